# revision 1
# baseline (speedup 1.0000x reference)
"""Trainium2 Bass kernel for nn_ContMixT (dense_cnn).

Data-parallel over batch: 8 samples -> 8 NeuronCores, no collectives.

v2 design notes:
- Conv tower (conv1 3x3 dil2 768->256, conv2 3x3 dil4 256->256) runs in
  fp8e4 with DoubleRow perf mode: K=256 contraction per matmul, halving
  matmul count vs bf16.  Numerically safe because the tower only feeds
  global average pooling (host-validated: end-to-end rel err ~1e-3).
- Conv matmuls stream 4D windows [p, 2, 8 rows, 56 cols] from padded
  [*, 64, 64] frames: one matmul covers 8 image rows, valid columns
  only (N=448).
- Conv biases fold into the ReLU activation (bias=[P,1] AP, scale=1/WS
  undoes the fp8 weight scaling).  Global pooling rides conv2's relu
  accum_out.
- fc2 is computed transposed (72 small matmuls) so the per-channel 3x3
  kernels land directly as [128, 18] columns - no DRAM bounce/scatter.
- f_prev = 0.5*(f_tm2+f_tm1) is computed on host (bf16), the blend is
  done in-place over it, and the output ships bf16 on the 64-wide grid
  (host strips pad columns and upcasts).
"""

import sys

if "/opt/trn_rl_repo" not in sys.path:
    sys.path.insert(0, "/opt/trn_rl_repo")

import numpy as np
import ml_dtypes

import concourse.bacc as bacc
import concourse.tile as tile
from concourse import mybir
from concourse.bass_utils import run_bass_kernel_spmd

BF16 = ml_dtypes.bfloat16
FP8 = ml_dtypes.float8_e4m3

B, C, H, W = 8, 256, 56, 56
HID = 256
P = 128
FW = 64          # frame width (56 + 2*4 pad)
GR = 64          # fp8 conv frame rows (ring of 4)
R0 = 4           # first image row
NR = 8           # rows per chunk
NCH = 7          # chunks (56 = 7*8)
WS = 64.0        # fp8 weight scale

USE_FP8 = True

LAST_INFO = {}


def _taps(d):
    return [(ky * 3 + kx, (ky - 1) * d, (kx - 1) * d) for ky in range(3) for kx in range(3)]


def build_nc():
    nc = bacc.Bacc()
    f32 = mybir.dt.float32
    bf16 = mybir.dt.bfloat16
    fp8 = mybir.dt.float8e4
    DR = mybir.MatmulPerfMode.DoubleRow if USE_FP8 else None
    cdt = fp8 if USE_FP8 else bf16

    Relu = mybir.ActivationFunctionType.Relu
    Sigmoid = mybir.ActivationFunctionType.Sigmoid
    mult = mybir.AluOpType.mult
    add = mybir.AluOpType.add

    # ---- dram I/O ----
    # conv tower input: 3 pairs (f_tm2, f_tm1, f_t), each [128, 2, 64, 64]
    xq = nc.dram_tensor("xq", [P, 3, 2, GR, FW], cdt, kind="ExternalInput")
    xt = nc.dram_tensor("xt", [P, 2, FW, FW], bf16, kind="ExternalInput")      # padded f_t
    xp = nc.dram_tensor("xp", [P, 2, H, FW], bf16, kind="ExternalInput")       # f_prev, 64-wide
    # conv weights (fp8 DoubleRow layout) packed in one tensor
    # w1: [9, 3, 2, 2, 128] -> 13824 elems/partition; w2: [9, 2, 2, 128] -> 4608
    wq = nc.dram_tensor("wq", [P, 9 * 3 * 2 * 2 * P + 9 * 2 * 2 * P], cdt,
                        kind="ExternalInput")
    # bf16 weights packed: gw [2,256]=512, fc1 [4,512]=2048, fc2 [18,4,128]=9216,
    # ident 128, awm 2, awp 2  -> 11908
    wb = nc.dram_tensor("wb", [P, 512 + 2048 + 9216 + P + 4], bf16, kind="ExternalInput")
    # fp32 consts: b1col 2, b2col 2, gbcol 2, fc1b 4, fc2bT 18 -> 28
    cf = nc.dram_tensor("cf", [P, 28], f32, kind="ExternalInput")
    onesr = nc.dram_tensor("onesr", [1, P], bf16, kind="ExternalInput")
    abt = nc.dram_tensor("abt", [1, 1], f32, kind="ExternalInput")

    yo = nc.dram_tensor("yo", [P, 2, H, FW], bf16, kind="ExternalOutput")

    W1SZ = 9 * 3 * 2 * 2 * P

    with tile.TileContext(nc) as tc:
        with (
            tc.tile_pool(name="mp", bufs=1) as mp,
            tc.tile_pool(name="psb", bufs=4, space="PSUM") as psb,
            tc.tile_pool(name="pss", bufs=2, space="PSUM") as pss,
            tc.tile_pool(name="psa", bufs=2, space="PSUM") as psa,
        ):
            xqs = mp.tile([P, 3, 2, GR, FW], cdt, name="xqs")
            y1s = mp.tile([P, 2, GR, FW], cdt, name="y1s")
            xts = mp.tile([P, 2, FW, FW], bf16, name="xts")
            xps = mp.tile([P, 2, H, FW], bf16, name="xps")
            fms = mp.tile([P, 2, H, FW], bf16, name="fms")
            w1s = mp.tile([P, 3, 9, 2, 2, P], cdt, name="w1s")
            w2s = mp.tile([P, 9, 2, 2, P], cdt, name="w2s")
            gws = mp.tile([P, 2, HID], bf16, name="gws")
            fc1ws = mp.tile([P, 4, 512], bf16, name="fc1ws")
            fc2ws = mp.tile([P, 18, 4, P], bf16, name="fc2ws")
            ident = mp.tile([P, P], bf16, name="ident")
            awm = mp.tile([P, 2], bf16, name="awm")
            awp = mp.tile([P, 2], bf16, name="awp")
            cfs = mp.tile([P, 28], f32, name="cfs")
            onesrs = mp.tile([1, P], bf16, name="onesrs")
            abts = mp.tile([1, 1], f32, name="abts")
            pacc = [mp.tile([P, NCH], f32, name=f"pacc{o}") for o in range(2)]
            gsum = mp.tile([P, 2], f32, name="gsum")
            lsum = mp.tile([P, 2], f32, name="lsum")
            gsumb = mp.tile([P, 2], bf16, name="gsumb")
            fcinb = mp.tile([P, 4], bf16, name="fcinb")
            hb = mp.tile([P, 4], bf16, name="hb")
            wkt = mp.tile([P, 18], f32, name="wkt")
            wks = mp.tile([P, 18], f32, name="wks")
            diag = [mp.tile([P, 9, P], bf16, name=f"diag{o}") for o in range(2)]

            # ---------- loads ----------
            KPSZ = 9 * 2 * 2 * P
            for kp in range(3):
                nc.sync.dma_start(
                    out=w1s[:, kp].rearrange("p b c d e -> p (b c d e)"),
                    in_=wq[:, kp * KPSZ:(kp + 1) * KPSZ])
                nc.sync.dma_start(out=xqs[:, kp, :, :, :], in_=xq[:, kp, :, :, :])
            nc.sync.dma_start(out=w2s.rearrange("p a b c d -> p (a b c d)"),
                              in_=wq[:, W1SZ:])
            nc.sync.dma_start(out=cfs, in_=cf[:, :])
            nc.scalar.memzero(y1s)

            b1c = [cfs[:, 0:1], cfs[:, 1:2]]
            b2c = [cfs[:, 2:3], cfs[:, 3:4]]
            gbc = cfs[:, 4:6]
            fc1b = cfs[:, 6:10]
            fc2bT = cfs[:, 10:28]

            taps1 = _taps(2)
            taps2 = _taps(4)
            taps3 = _taps(1)

            # ---------- conv1 ----------
            for c in range(NCH):
                for o in range(2):
                    ps = psb.tile([P, NR, W], f32, name=f"c1_{o}_{c}", tag="psb")
                    psl = ps.rearrange("p a b -> p (a b)")
                    mms = []
                    for kp in range(3):
                        for (t, dy, dx) in taps1:
                            r = R0 + NR * c + dy
                            if USE_FP8:
                                mms.append((w1s[:, kp, t, o, :, :],
                                            xqs[:, kp, :, r:r + NR, 4 + dx:60 + dx]))
                            else:
                                for i in range(2):
                                    mms.append((w1s[:, kp, t, o, i, :],
                                                xqs[:, kp, i, r:r + NR, 4 + dx:60 + dx]))
                    for n, (wv, xv) in enumerate(mms):
                        nc.tensor.matmul(psl, wv, xv, start=(n == 0),
                                         stop=(n == len(mms) - 1), perf_mode=DR)
                    nc.scalar.activation(
                        out=y1s[:, o, R0 + NR * c:R0 + NR * c + NR, 4:60],
                        in_=ps, func=Relu,
                        bias=b1c[o], scale=1.0 / WS,
                    )

            # late-use loads: emitted after conv1 so they queue behind the
            # conv-critical transfers on the DMA engines
            nc.sync.dma_start(out=xts, in_=xt[:, :, :, :])
            nc.sync.dma_start(out=xps, in_=xp[:, :, :, :])
            wbv = [(gws.rearrange("p a b -> p (a b)"), 2 * HID),
                   (fc1ws.rearrange("p a b -> p (a b)"), 4 * 512),
                   (fc2ws.rearrange("p a b c -> p (a b c)"), 18 * 4 * P),
                   (ident, P), (awm, 2), (awp, 2)]
            off = 0
            for v, n in wbv:
                nc.sync.dma_start(out=v, in_=wb[:, off:off + n])
                off += n
            nc.sync.dma_start(out=onesrs, in_=onesr[:, :])
            nc.sync.dma_start(out=abts, in_=abt[:, :])
            # local pooling of f_t (DVE, idle during conv)
            for o in range(2):
                nc.vector.tensor_reduce(
                    out=lsum[:, o:o + 1], in_=xts[:, o, 4:60, 4:60],
                    axis=mybir.AxisListType.XY, op=add,
                )

            # ---------- conv2 + pooled accumulation ----------
            for c in range(NCH):
                for o in range(2):
                    ps = psb.tile([P, NR, W], f32, name=f"c2_{o}_{c}", tag="psb")
                    psl = ps.rearrange("p a b -> p (a b)")
                    mms = []
                    for (t, dy, dx) in taps2:
                        r = R0 + NR * c + dy
                        if USE_FP8:
                            mms.append((w2s[:, t, o, :, :],
                                        y1s[:, :, r:r + NR, 4 + dx:60 + dx]))
                        else:
                            for i in range(2):
                                mms.append((w2s[:, t, o, i, :],
                                            y1s[:, i, r:r + NR, 4 + dx:60 + dx]))
                    for n, (wv, xv) in enumerate(mms):
                        nc.tensor.matmul(psl, wv, xv, start=(n == 0),
                                         stop=(n == len(mms) - 1), perf_mode=DR)
                    sc2 = mp.tile([P, NR, 56], bf16, name=f"sc2_{o}_{c}", tag="sc2", bufs=2)
                    nc.scalar.activation(
                        out=sc2, in_=ps, func=Relu,
                        bias=b2c[o], scale=1.0 / WS,
                        accum_out=pacc[o][:, c:c + 1],
                    )

            # ---------- pools -> fc chain ----------
            for o in range(2):
                nc.vector.tensor_reduce(
                    out=gsum[:, o:o + 1], in_=pacc[o],
                    axis=mybir.AxisListType.X, op=add,
                )
            nc.vector.tensor_copy(gsumb, gsum)

            psg = pss.tile([P, 2], f32, name="psg", tag="pss")
            for m in range(2):
                for k in range(2):
                    nc.tensor.matmul(
                        psg[:, m:m + 1], gws[:, k, m * P:(m + 1) * P],
                        gsumb[:, k:k + 1], start=(k == 0), stop=(k == 1),
                    )
            nc.vector.tensor_add(fcinb[:, 0:2], psg, gbc)
            nc.vector.tensor_copy(fcinb[:, 2:4], lsum)

            psh = pss.tile([P, 4], f32, name="psh", tag="pss")
            for m in range(4):
                for k in range(4):
                    nc.tensor.matmul(
                        psh[:, m:m + 1], fc1ws[:, k, m * P:(m + 1) * P],
                        fcinb[:, k:k + 1], start=(k == 0), stop=(k == 3),
                    )
            nc.vector.tensor_add(hb, psh, fc1b)

            psT = pss.tile([P, 18], f32, name="psT", tag="pss")
            for j in range(18):
                for kc in range(4):
                    nc.tensor.matmul(
                        psT[:, j:j + 1], fc2ws[:, j, kc, :],
                        hb[:, kc:kc + 1], start=(kc == 0), stop=(kc == 3),
                    )
            nc.vector.tensor_add(wkt, psT, fc2bT)
            # silu(z) = z * sigmoid(z) — CoreSim lacks a native Silu
            nc.scalar.activation(out=wks, in_=wkt, func=Sigmoid)
            nc.vector.tensor_mul(wks, wks, wkt)

            for j in range(18):
                nc.vector.tensor_scalar_mul(diag[j // 9][:, j % 9, :], ident,
                                            wks[:, j:j + 1])

            # ---------- depthwise + alpha + fusion (448-wide: valid cols only) ----------
            NV = NR * W  # 448
            for c in range(NCH):
                rows = slice(NR * c, NR * c + NR)
                for o in range(2):
                    ps = psb.tile([P, NR, W], f32, name=f"dw_{o}_{c}", tag="psb")
                    psl = ps.rearrange("p a b -> p (a b)")
                    for (t, dy, dx) in taps3:
                        nc.tensor.matmul(
                            psl, diag[o][:, t, :],
                            xts[:, o, 4 + NR * c + dy:4 + NR * c + dy + NR,
                                4 + dx:60 + dx],
                            start=(t == 0), stop=(t == 8),
                        )
                    nc.scalar.copy(fms[:, o, rows, 4:60], ps)

                pa = psa.tile([1, NR, W], f32, name=f"pa{c}", tag="psa")
                pal = pa.rearrange("p a b -> p (a b)")
                for o in range(2):
                    nc.tensor.matmul(
                        pal, awm[:, o:o + 1], fms[:, o, rows, 4:60],
                        start=(o == 0), stop=False,
                    )
                for o in range(2):
                    nc.tensor.matmul(
                        pal, awp[:, o:o + 1], xps[:, o, rows, 4:60],
                        start=False, stop=(o == 1),
                    )
                arow = mp.tile([1, NR, W], bf16, name=f"ar{c}", tag="ar", bufs=2)
                nc.scalar.activation(out=arow, in_=pa, func=Sigmoid, bias=abts[:, 0:1])
                nc.vector.tensor_scalar(arow, arow, 0.4, 0.3, op0=mult, op1=add)
                pb = psa.tile([P, NR, W], f32, name=f"pb{c}", tag="psa")
                nc.tensor.matmul(pb.rearrange("p a b -> p (a b)"), onesrs,
                                 arow.rearrange("p a b -> p (a b)"),
                                 start=True, stop=True)

                for o in range(2):
                    u = mp.tile([P, NR, W], f32, name=f"u{c}{o}", tag="u", bufs=3)
                    nc.vector.scalar_tensor_tensor(
                        u, xps[:, o, rows, 4:60], -1.0, fms[:, o, rows, 4:60],
                        op0=mult, op1=add,
                    )
                    nc.vector.tensor_mul(u, u, pb)
                    nc.vector.tensor_add(xps[:, o, rows, 4:60], xps[:, o, rows, 4:60], u)
                # stream this chunk's rows out while later chunks compute
                nc.sync.dma_start(out=yo[:, :, rows, :], in_=xps[:, :, rows, :])

    nc.compile()
    return nc


def _prep_shared(w1, b1, w2, b2, gw, gb, fc1_w, fc1_b, fc2_w, fc2_b, aw, ab):
    d = {}
    cdt = FP8 if USE_FP8 else BF16
    # conv1 weights: [k, t, kp, o, i, m]
    w1r = w1.reshape(2, P, 3, 2, P, 3, 3)            # o m kp i k ty tx
    w1q = np.ascontiguousarray(w1r.transpose(4, 2, 5, 6, 0, 3, 1))  # k kp ty tx o i m
    w1q = w1q.reshape(P, 9 * 3 * 2 * 2 * P)
    w2r = w2.reshape(2, P, 2, P, 3, 3)               # o m i k ty tx
    w2q = np.ascontiguousarray(w2r.transpose(3, 4, 5, 0, 2, 1))     # k ty tx o i m
    w2q = w2q.reshape(P, 9 * 2 * 2 * P)
    wqq = np.concatenate([w1q, w2q], axis=1).astype(np.float32) * WS
    d["wq"] = wqq.astype(cdt)

    gwt = np.ascontiguousarray((gw[:, :, 0, 0] / 3136.0).T).reshape(2, P, HID)
    gwb = np.ascontiguousarray(gwt.transpose(1, 0, 2)).reshape(P, 2 * HID)
    fc1t = fc1_w.T.copy()
    fc1t[C:, :] /= 3136.0
    fc1b4 = np.ascontiguousarray(fc1_b.reshape(4, P).T)              # [128, 4]
    fc1wb = np.ascontiguousarray(fc1t.reshape(4, P, 512).transpose(1, 0, 2)).reshape(P, 4 * 512)
    f2 = fc2_w.T.reshape(4, P, 2, P, 9)              # kc k bl p t
    fc2wb = np.ascontiguousarray(f2.transpose(1, 2, 4, 0, 3))        # k bl t kc p
    fc2wb = fc2wb.reshape(P, 18 * 4 * P)
    fc2bT = np.ascontiguousarray(fc2_b.reshape(2, P, 9).transpose(1, 0, 2)).reshape(P, 18)
    identm = np.eye(P, dtype=np.float32)
    awm = np.ascontiguousarray(aw[0, :C, 0, 0].reshape(2, P).T)      # [128, 2]
    awp = np.ascontiguousarray(aw[0, C:, 0, 0].reshape(2, P).T)
    d["wb"] = np.concatenate(
        [gwb, fc1wb, fc2wb, identm, awm, awp], axis=1).astype(BF16)
    b1c = b1.reshape(2, P).T                          # [128, 2]
    b2c = b2.reshape(2, P).T
    gbc = gb.reshape(2, P).T
    d["cf"] = np.concatenate([b1c, b2c, gbc, fc1b4, fc2bT], axis=1).astype(np.float32)
    d["onesr"] = np.ones((1, P), dtype=np.float32).astype(BF16)
    d["abt"] = ab.reshape(1, 1).astype(np.float32)
    return d


def _pad4(x, dtype):
    """[256, 56, 56] -> [128, 2, 64, 64] with ring of 4."""
    out = np.zeros((P, 2, FW, FW), dtype=np.float32)
    xr = x.reshape(2, P, H, W)
    out[:, 0, 4:60, 4:60] = xr[0]
    out[:, 1, 4:60, 4:60] = xr[1]
    return out.astype(dtype)


def kernel(f_tm2, f_tm1, f_t, w1, b1, w2, b2, gw, gb,
           fc1_w, fc1_b, fc2_w, fc2_b, aw, ab):
    import time

    args = [np.asarray(a, dtype=np.float32) for a in
            (f_tm2, f_tm1, f_t, w1, b1, w2, b2, gw, gb, fc1_w, fc1_b, fc2_w, fc2_b, aw, ab)]
    f_tm2, f_tm1, f_t = args[0], args[1], args[2]

    t0 = time.time()
    shared = _prep_shared(*args[3:])
    cdt = FP8 if USE_FP8 else BF16
    in_maps = []
    for b in range(B):
        m = dict(shared)
        m["xq"] = np.stack([_pad4(f_tm2[b], cdt), _pad4(f_tm1[b], cdt),
                            _pad4(f_t[b], cdt)], axis=1)   # [128, 3, 2, 64, 64]
        m["xt"] = _pad4(f_t[b], BF16)
        fp = (f_tm2[b] + f_tm1[b]) * 0.5
        xpm = np.zeros((P, 2, H, FW), dtype=np.float32)
        xpm[:, 0, :, 4:60] = fp.reshape(2, P, H, W)[0]
        xpm[:, 1, :, 4:60] = fp.reshape(2, P, H, W)[1]
        m["xp"] = xpm.astype(BF16)
        in_maps.append(m)
    t1 = time.time()

    nc = build_nc()
    t2 = time.time()
    res = run_bass_kernel_spmd(nc, in_maps, list(range(B)))
    t3 = time.time()

    out = np.empty((B, C, H, W), dtype=np.float32)
    for b in range(B):
        yb = res.results[b]["yo"].reshape(P, 2, H, FW).astype(np.float32)
        out[b] = yb[:, :, :, 4:60].transpose(1, 0, 2, 3).reshape(C, H, W)
    LAST_INFO.update(dict(prep_s=t1 - t0, build_s=t2 - t1, run_s=t3 - t2,
                          exec_time_ns=res.exec_time_ns))
    return out



# revision 2
# speedup vs baseline: 31.8086x; 31.8086x over previous
"""Trainium2 Bass kernel for nn_ContMixT (dense_cnn).

Data-parallel over batch: 8 samples -> 8 NeuronCores.

v3 design notes (tunnel-transfer-bound, so minimize bytes moved):
- Frames reparametrized: s=(f_tm2+f_tm1)/2 (bf16, also the blend base),
  d=(f_tm2-f_tm1)/2 (fp8, conv tower only), c=f_t (bf16).  conv1 weights
  are transformed host-side (W1s=W1a+W1b, W1d=W1a-W1b) so the tower
  consumes (s, d, c) directly.  5 B/pixel shipped vs 8 B/pixel in v2.
- All tensors ship tight (56x56); padding to the 64x64 conv frames is
  done on device (memzero + interior DMA).  fp8 tower copies of s and c
  are cast on device instead of shipped.
- The big weights (conv fp8 + fc bf16) are sharded 1/8 per core and
  AllGathered on device over NeuronLink: 5.4 MB total over the host
  tunnel instead of 43 MB replicated.
- identity / ones constants ride inside the NEFF (inline_tensor).
- Custom exec path (no donated zero output buffers -- the kernel writes
  every output element) with the compiled executable cached in a module
  global, so repeat calls skip build+compile.
- Conv tower runs fp8 DoubleRow as in v2; depthwise + blend read bf16.
"""

import sys

if "/opt/trn_rl_repo" not in sys.path:
    sys.path.insert(0, "/opt/trn_rl_repo")

import numpy as np
import ml_dtypes
import jax
from jax.sharding import Mesh, PartitionSpec
from jax.experimental.shard_map import shard_map

import concourse.bacc as bacc
import concourse.tile as tile
from concourse import mybir
from concourse import bass2jax

BF16 = ml_dtypes.bfloat16
FP8 = ml_dtypes.float8_e4m3

B, C, H, W = 8, 256, 56, 56
HID = 256
P = 128
FW = 64          # conv frame width (56 + 2*4 pad)
R0 = 4           # first image row in the padded frame
NR = 8           # rows per chunk
NCH = 7          # chunks (56 = 7*8)
WS = 64.0        # fp8 weight scale

QCOLS = 9 * 3 * 2 * 2 * P + 9 * 2 * 2 * P   # 13824 + 4608 = 18432
QSH = QCOLS // 8                             # 2304
W1SZ = 9 * 3 * 2 * 2 * P                     # 13824
BCOLS = 12288
BSH = BCOLS // 8                             # 1536
OFF_GW, OFF_FC1, OFF_FC2, OFF_AWM, OFF_AWP = 0, 512, 2560, 11776, 11778

LAST_INFO = {}
_CACHE = {}


def _taps(d):
    return [(ky * 3 + kx, (ky - 1) * d, (kx - 1) * d) for ky in range(3) for kx in range(3)]


def build_nc():
    nc = bacc.Bacc()
    f32 = mybir.dt.float32
    bf16 = mybir.dt.bfloat16
    fp8 = mybir.dt.float8e4
    DR = mybir.MatmulPerfMode.DoubleRow

    Relu = mybir.ActivationFunctionType.Relu
    Sigmoid = mybir.ActivationFunctionType.Sigmoid
    mult = mybir.AluOpType.mult
    add = mybir.AluOpType.add

    # ---- dram I/O (all tight 56x56; o = channel half) ----
    xf = nc.dram_tensor("xf", [4, P, H, W], bf16, kind="ExternalInput")   # [0:2]=s, [2:4]=c
    xd = nc.dram_tensor("xd", [2, P, H, W], fp8, kind="ExternalInput")    # d halves
    qsh = nc.dram_tensor("qsh", [P, QSH], fp8, kind="ExternalInput")      # conv-weight shard
    bsh = nc.dram_tensor("bsh", [P, BSH], bf16, kind="ExternalInput")     # fc-weight shard
    cf = nc.dram_tensor("cf", [P, 28], f32, kind="ExternalInput")         # biases (replicated)
    abt = nc.dram_tensor("abt", [1, 1], f32, kind="ExternalInput")

    y = nc.dram_tensor("y", [2, P, H, W], bf16, kind="ExternalOutput")

    identc = nc.inline_tensor(np.eye(P, dtype=np.float32).astype(BF16), name="identc")
    onesrc = nc.inline_tensor(np.ones((1, P), np.float32).astype(BF16), name="onesrc")

    with tile.TileContext(nc) as tc:
        with (
            tc.tile_pool(name="mp", bufs=1) as mp,
            tc.tile_pool(name="dp", bufs=1, space="DRAM") as dp,
            tc.tile_pool(name="psb", bufs=4, space="PSUM") as psb,
            tc.tile_pool(name="pss", bufs=2, space="PSUM") as pss,
            tc.tile_pool(name="psa", bufs=2, space="PSUM") as psa,
        ):
            xqs = mp.tile([P, 3, 2, FW, FW], fp8, name="xqs")   # tower input: s,d,c
            xss = mp.tile([P, 2, FW, FW], bf16, name="xss")     # s padded (blend base)
            xcs = mp.tile([P, 2, FW, FW], bf16, name="xcs")     # c padded (depthwise in)
            y1s = mp.tile([P, 2, FW, FW], fp8, name="y1s")
            fms = mp.tile([P, 2, H, FW], bf16, name="fms")
            w1s = mp.tile([P, 3, 9, 2, 2, P], fp8, name="w1s")
            w2s = mp.tile([P, 9, 2, 2, P], fp8, name="w2s")
            gws = mp.tile([P, 2, HID], bf16, name="gws")
            fc1ws = mp.tile([P, 4, 512], bf16, name="fc1ws")
            fc2ws = mp.tile([P, 18, 4, P], bf16, name="fc2ws")
            ident = mp.tile([P, P], bf16, name="ident")
            awm = mp.tile([P, 2], bf16, name="awm")
            awp = mp.tile([P, 2], bf16, name="awp")
            cfs = mp.tile([P, 28], f32, name="cfs")
            onesrs = mp.tile([1, P], bf16, name="onesrs")
            abts = mp.tile([1, 1], f32, name="abts")
            pacc = [mp.tile([P, NCH], f32, name=f"pacc{o}") for o in range(2)]
            gsum = mp.tile([P, 2], f32, name="gsum")
            lsum = mp.tile([P, 2], f32, name="lsum")
            gsumb = mp.tile([P, 2], bf16, name="gsumb")
            fcinb = mp.tile([P, 4], bf16, name="fcinb")
            hb = mp.tile([P, 4], bf16, name="hb")
            wkt = mp.tile([P, 18], f32, name="wkt")
            wks = mp.tile([P, 18], f32, name="wks")
            diag = [mp.tile([P, 9, P], bf16, name=f"diag{o}") for o in range(2)]

            qb = dp.tile([P, QSH], fp8, name="qb")
            qg = dp.tile([8, P, QSH], fp8, name="qg")
            bb = dp.tile([P, BSH], bf16, name="bb")
            bg = dp.tile([8, P, BSH], bf16, name="bg")

            # ---------- weight AllGather (NeuronLink, not host tunnel) ----------
            nc.gpsimd.dma_start(qb[:], qsh[:])
            nc.gpsimd.collective_compute(
                "AllGather", mybir.AluOpType.bypass,
                replica_groups=[list(range(8))], ins=[qb.opt()], outs=[qg.opt()])
            nc.gpsimd.dma_start(bb[:], bsh[:])
            nc.gpsimd.collective_compute(
                "AllGather", mybir.AluOpType.bypass,
                replica_groups=[list(range(8))], ins=[bb.opt()], outs=[bg.opt()])

            w1f = w1s.rearrange("p a b c d e -> p (a b c d e)")
            w2f = w2s.rearrange("p a b c d -> p (a b c d)")
            for r in range(6):
                nc.sync.dma_start(out=w1f[:, r * QSH:(r + 1) * QSH], in_=qg[r])
            for r in (6, 7):
                o0 = (r - 6) * QSH
                nc.sync.dma_start(out=w2f[:, o0:o0 + QSH], in_=qg[r])

            # ---------- frame staging ----------
            nc.scalar.memzero(xss)
            nc.scalar.memzero(xcs)
            nc.scalar.memzero(xqs)
            for o in range(2):
                nc.sync.dma_start(out=xss[:, o, R0:R0 + H, R0:R0 + W], in_=xf[o])
                nc.sync.dma_start(out=xcs[:, o, R0:R0 + H, R0:R0 + W], in_=xf[2 + o])
                nc.sync.dma_start(out=xqs[:, 1, o, R0:R0 + H, R0:R0 + W], in_=xd[o])
            nc.vector.tensor_copy(xqs[:, 0], xss)    # s -> fp8 (pad ring is zero)
            nc.vector.tensor_copy(xqs[:, 2], xcs)    # c -> fp8
            nc.sync.dma_start(out=cfs, in_=cf[:, :])
            nc.sync.dma_start(out=abts, in_=abt[:, :])
            nc.sync.dma_start(out=ident, in_=identc[:, :])
            nc.sync.dma_start(out=onesrs, in_=onesrc[:, :])
            nc.scalar.memzero(y1s)

            b1c = [cfs[:, 0:1], cfs[:, 1:2]]
            b2c = [cfs[:, 2:3], cfs[:, 3:4]]
            gbc = cfs[:, 4:6]
            fc1b = cfs[:, 6:10]
            fc2bT = cfs[:, 10:28]

            taps1 = _taps(2)
            taps2 = _taps(4)
            taps3 = _taps(1)

            # ---------- conv1 ----------
            for c in range(NCH):
                for o in range(2):
                    ps = psb.tile([P, NR, W], f32, name=f"c1_{o}_{c}", tag="psb")
                    psl = ps.rearrange("p a b -> p (a b)")
                    mms = []
                    for kp in range(3):
                        for (t, dy, dx) in taps1:
                            r = R0 + NR * c + dy
                            mms.append((w1s[:, kp, t, o, :, :],
                                        xqs[:, kp, :, r:r + NR, 4 + dx:60 + dx]))
                    for n, (wv, xv) in enumerate(mms):
                        nc.tensor.matmul(psl, wv, xv, start=(n == 0),
                                         stop=(n == len(mms) - 1), perf_mode=DR)
                    nc.scalar.activation(
                        out=y1s[:, o, R0 + NR * c:R0 + NR * c + NR, 4:60],
                        in_=ps, func=Relu,
                        bias=b1c[o], scale=1.0 / WS,
                    )

            # late-use loads: emitted after conv1 so they queue behind the
            # conv-critical transfers on the DMA engines
            regions = [
                (gws.rearrange("p a b -> p (a b)"), OFF_GW, 2 * HID),
                (fc1ws.rearrange("p a b -> p (a b)"), OFF_FC1, 4 * 512),
                (fc2ws.rearrange("p a b c -> p (a b c)"), OFF_FC2, 18 * 4 * P),
                (awm, OFF_AWM, 2),
                (awp, OFF_AWP, 2),
            ]
            for dst, roff, rlen in regions:
                for r in range(8):
                    a = max(roff, r * BSH)
                    b2 = min(roff + rlen, (r + 1) * BSH)
                    if a < b2:
                        nc.sync.dma_start(out=dst[:, a - roff:b2 - roff],
                                          in_=bg[r][:, a - r * BSH:b2 - r * BSH])
            # local pooling of f_t (DVE, idle during conv)
            for o in range(2):
                nc.vector.tensor_reduce(
                    out=lsum[:, o:o + 1], in_=xcs[:, o, 4:60, 4:60],
                    axis=mybir.AxisListType.XY, op=add,
                )

            # ---------- conv2 + pooled accumulation ----------
            for c in range(NCH):
                for o in range(2):
                    ps = psb.tile([P, NR, W], f32, name=f"c2_{o}_{c}", tag="psb")
                    psl = ps.rearrange("p a b -> p (a b)")
                    mms = []
                    for (t, dy, dx) in taps2:
                        r = R0 + NR * c + dy
                        mms.append((w2s[:, t, o, :, :],
                                    y1s[:, :, r:r + NR, 4 + dx:60 + dx]))
                    for n, (wv, xv) in enumerate(mms):
                        nc.tensor.matmul(psl, wv, xv, start=(n == 0),
                                         stop=(n == len(mms) - 1), perf_mode=DR)
                    sc2 = mp.tile([P, NR, 56], bf16, name=f"sc2_{o}_{c}", tag="sc2", bufs=2)
                    nc.scalar.activation(
                        out=sc2, in_=ps, func=Relu,
                        bias=b2c[o], scale=1.0 / WS,
                        accum_out=pacc[o][:, c:c + 1],
                    )

            # ---------- pools -> fc chain ----------
            for o in range(2):
                nc.vector.tensor_reduce(
                    out=gsum[:, o:o + 1], in_=pacc[o],
                    axis=mybir.AxisListType.X, op=add,
                )
            nc.vector.tensor_copy(gsumb, gsum)

            psg = pss.tile([P, 2], f32, name="psg", tag="pss")
            for m in range(2):
                for k in range(2):
                    nc.tensor.matmul(
                        psg[:, m:m + 1], gws[:, k, m * P:(m + 1) * P],
                        gsumb[:, k:k + 1], start=(k == 0), stop=(k == 1),
                    )
            nc.vector.tensor_add(fcinb[:, 0:2], psg, gbc)
            nc.vector.tensor_copy(fcinb[:, 2:4], lsum)

            psh = pss.tile([P, 4], f32, name="psh", tag="pss")
            for m in range(4):
                for k in range(4):
                    nc.tensor.matmul(
                        psh[:, m:m + 1], fc1ws[:, k, m * P:(m + 1) * P],
                        fcinb[:, k:k + 1], start=(k == 0), stop=(k == 3),
                    )
            nc.vector.tensor_add(hb, psh, fc1b)

            psT = pss.tile([P, 18], f32, name="psT", tag="pss")
            for j in range(18):
                for kc in range(4):
                    nc.tensor.matmul(
                        psT[:, j:j + 1], fc2ws[:, j, kc, :],
                        hb[:, kc:kc + 1], start=(kc == 0), stop=(kc == 3),
                    )
            nc.vector.tensor_add(wkt, psT, fc2bT)
            # silu(z) = z * sigmoid(z)
            nc.scalar.activation(out=wks, in_=wkt, func=Sigmoid)
            nc.vector.tensor_mul(wks, wks, wkt)

            for j in range(18):
                nc.vector.tensor_scalar_mul(diag[j // 9][:, j % 9, :], ident,
                                            wks[:, j:j + 1])

            # ---------- depthwise + alpha + fusion (valid cols only) ----------
            for c in range(NCH):
                rows = slice(NR * c, NR * c + NR)
                prow = slice(R0 + NR * c, R0 + NR * c + NR)   # rows in padded tiles
                for o in range(2):
                    ps = psb.tile([P, NR, W], f32, name=f"dw_{o}_{c}", tag="psb")
                    psl = ps.rearrange("p a b -> p (a b)")
                    for (t, dy, dx) in taps3:
                        nc.tensor.matmul(
                            psl, diag[o][:, t, :],
                            xcs[:, o, R0 + NR * c + dy:R0 + NR * c + dy + NR,
                                4 + dx:60 + dx],
                            start=(t == 0), stop=(t == 8),
                        )
                    nc.scalar.copy(fms[:, o, rows, 4:60], ps)

                pa = psa.tile([1, NR, W], f32, name=f"pa{c}", tag="psa")
                pal = pa.rearrange("p a b -> p (a b)")
                for o in range(2):
                    nc.tensor.matmul(
                        pal, awm[:, o:o + 1], fms[:, o, rows, 4:60],
                        start=(o == 0), stop=False,
                    )
                for o in range(2):
                    nc.tensor.matmul(
                        pal, awp[:, o:o + 1], xss[:, o, prow, 4:60],
                        start=False, stop=(o == 1),
                    )
                arow = mp.tile([1, NR, W], bf16, name=f"ar{c}", tag="ar", bufs=2)
                nc.scalar.activation(out=arow, in_=pa, func=Sigmoid, bias=abts[:, 0:1])
                nc.vector.tensor_scalar(arow, arow, 0.4, 0.3, op0=mult, op1=add)
                pb = psa.tile([P, NR, W], f32, name=f"pb{c}", tag="psa")
                nc.tensor.matmul(pb.rearrange("p a b -> p (a b)"), onesrs,
                                 arow.rearrange("p a b -> p (a b)"),
                                 start=True, stop=True)

                for o in range(2):
                    u = mp.tile([P, NR, W], f32, name=f"u{c}{o}", tag="u", bufs=3)
                    nc.vector.scalar_tensor_tensor(
                        u, xss[:, o, prow, 4:60], -1.0, fms[:, o, rows, 4:60],
                        op0=mult, op1=add,
                    )
                    nc.vector.tensor_mul(u, u, pb)
                    nc.vector.tensor_add(xss[:, o, prow, 4:60], xss[:, o, prow, 4:60], u)
                    # stream this chunk's rows out while later chunks compute
                    nc.sync.dma_start(out=y[o, :, rows, :],
                                      in_=xss[:, o, prow, 4:60])

    nc.compile()
    return nc


def _get_exec():
    if "jitted" in _CACHE:
        return _CACHE
    nc = build_nc()
    bass2jax.install_neuronx_cc_hook()
    partition_name = nc.partition_id_tensor.name if nc.partition_id_tensor else None
    in_names, out_names, out_avals = [], [], []
    for alloc in nc.m.functions[0].allocations:
        if not isinstance(alloc, mybir.MemoryLocationSet):
            continue
        name = alloc.memorylocations[0].name
        if alloc.kind == "ExternalInput":
            if name != partition_name:
                in_names.append(name)
        elif alloc.kind == "ExternalOutput":
            out_names.append(name)
            out_avals.append(jax.core.ShapedArray(
                tuple(alloc.tensor_shape), mybir.dt.np(alloc.dtype)))
    n_params = len(in_names)
    param_names = list(in_names)
    if partition_name is not None:
        in_names.append(partition_name)

    def _body(*args):
        operands = list(args)
        if partition_name is not None:
            operands.append(bass2jax.partition_id_tensor())
        return tuple(bass2jax._bass_exec_p.bind(
            *operands, out_avals=tuple(out_avals),
            in_names=tuple(in_names), out_names=tuple(out_names),
            lowering_input_output_aliases=(), sim_require_finite=True,
            sim_require_nnan=True, nc=nc))

    devices = jax.devices()[:8]
    mesh = Mesh(np.asarray(devices), ("core",))
    jitted = jax.jit(shard_map(
        _body, mesh=mesh,
        in_specs=(PartitionSpec("core"),) * n_params,
        out_specs=(PartitionSpec("core"),) * len(out_names), check_rep=False))
    _CACHE.update(dict(jitted=jitted, param_names=param_names,
                       out_names=out_names, out_avals=out_avals))
    return _CACHE


def _prep_shared(w1, b1, w2, b2, gw, gb, fc1_w, fc1_b, fc2_w, fc2_b, aw, ab):
    """Returns global (8-core concat) arrays for the shared-weight inputs."""
    # conv1 weights with the (a,b,c)->(s,d,c) frame transform on axis kp
    w1r = w1.reshape(2, P, 3, 2, P, 3, 3)            # o m kp i k ty tx
    w1t = np.empty_like(w1r)
    w1t[:, :, 0] = w1r[:, :, 0] + w1r[:, :, 1]       # applies to s
    w1t[:, :, 1] = w1r[:, :, 0] - w1r[:, :, 1]       # applies to d
    w1t[:, :, 2] = w1r[:, :, 2]                      # applies to c
    w1q = np.ascontiguousarray(w1t.transpose(4, 2, 5, 6, 0, 3, 1))  # k kp ty tx o i m
    w1q = w1q.reshape(P, W1SZ)
    w2r = w2.reshape(2, P, 2, P, 3, 3)               # o m i k ty tx
    w2q = np.ascontiguousarray(w2r.transpose(3, 4, 5, 0, 2, 1))     # k ty tx o i m
    w2q = w2q.reshape(P, 9 * 2 * 2 * P)
    wq_full = (np.concatenate([w1q, w2q], axis=1) * WS).astype(FP8)  # [P, QCOLS]
    qsh_g = np.ascontiguousarray(
        wq_full.reshape(P, 8, QSH).transpose(1, 0, 2)).reshape(8 * P, QSH)

    gwt = np.ascontiguousarray((gw[:, :, 0, 0] / 3136.0).T).reshape(2, P, HID)
    gwb = np.ascontiguousarray(gwt.transpose(1, 0, 2)).reshape(P, 2 * HID)
    fc1t = fc1_w.T.copy()
    fc1t[C:, :] /= 3136.0
    fc1wb = np.ascontiguousarray(
        fc1t.reshape(4, P, 512).transpose(1, 0, 2)).reshape(P, 4 * 512)
    f2 = fc2_w.T.reshape(4, P, 2, P, 9)              # kc k bl p t
    fc2wb = np.ascontiguousarray(f2.transpose(1, 2, 4, 0, 3)).reshape(P, 18 * 4 * P)
    awm = np.ascontiguousarray(aw[0, :C, 0, 0].reshape(2, P).T)      # [128, 2]
    awp = np.ascontiguousarray(aw[0, C:, 0, 0].reshape(2, P).T)
    wb_full = np.zeros((P, BCOLS), dtype=np.float32)
    wb_full[:, OFF_GW:OFF_GW + 512] = gwb
    wb_full[:, OFF_FC1:OFF_FC1 + 2048] = fc1wb
    wb_full[:, OFF_FC2:OFF_FC2 + 9216] = fc2wb
    wb_full[:, OFF_AWM:OFF_AWM + 2] = awm
    wb_full[:, OFF_AWP:OFF_AWP + 2] = awp
    wb_full = wb_full.astype(BF16)
    bsh_g = np.ascontiguousarray(
        wb_full.reshape(P, 8, BSH).transpose(1, 0, 2)).reshape(8 * P, BSH)

    fc1b4 = np.ascontiguousarray(fc1_b.reshape(4, P).T)              # [128, 4]
    b1c = b1.reshape(2, P).T
    b2c = b2.reshape(2, P).T
    gbc = gb.reshape(2, P).T
    fc2bT = np.ascontiguousarray(fc2_b.reshape(2, P, 9).transpose(1, 0, 2)).reshape(P, 18)
    cf1 = np.concatenate([b1c, b2c, gbc, fc1b4, fc2bT], axis=1).astype(np.float32)
    cf_g = np.ascontiguousarray(np.broadcast_to(cf1, (8,) + cf1.shape)).reshape(8 * P, 28)
    abt_g = np.broadcast_to(ab.reshape(1, 1).astype(np.float32), (8, 1)).reshape(8, 1)
    return {"qsh": qsh_g, "bsh": bsh_g, "cf": cf_g, "abt": abt_g}


def kernel(f_tm2, f_tm1, f_t, w1, b1, w2, b2, gw, gb,
           fc1_w, fc1_b, fc2_w, fc2_b, aw, ab):
    import time

    args = [np.asarray(a, dtype=np.float32) for a in
            (f_tm2, f_tm1, f_t, w1, b1, w2, b2, gw, gb, fc1_w, fc1_b, fc2_w, fc2_b, aw, ab)]
    f_tm2, f_tm1, f_t = args[0], args[1], args[2]

    t0 = time.time()
    g = _get_exec()
    t1 = time.time()

    s = (f_tm2 + f_tm1) * np.float32(0.5)
    d = f_tm2 - f_tm1
    d *= np.float32(0.5)
    xf_g = np.empty((B, 4, P, H, W), dtype=BF16)
    xf_g[:, 0:2] = s.reshape(B, 2, P, H, W)
    xf_g[:, 2:4] = f_t.reshape(B, 2, P, H, W)
    xf_g = xf_g.reshape(B * 4, P, H, W)
    xd_g = d.reshape(B * 2, P, H, W).astype(FP8)
    ins = _prep_shared(*args[3:])
    ins["xf"] = xf_g
    ins["xd"] = xd_g
    t2 = time.time()

    concat_in = [ins[name] for name in g["param_names"]]
    out_arrs = g["jitted"](*concat_in)
    yv = np.asarray(out_arrs[0])                      # [8*2, 128, 56, 56] bf16
    t3 = time.time()

    out = yv.reshape(B, C, H, W).astype(np.float32)
    t4 = time.time()
    LAST_INFO.update(dict(build_s=t1 - t0, prep_s=t2 - t1, run_s=t3 - t2,
                          post_s=t4 - t3, exec_time_ns=None))
    return out


# revision 10
# speedup vs baseline: 34.0974x; 1.0720x over previous
"""Trainium2 Bass kernel for nn_ContMixT (dense_cnn).

Data-parallel over batch: 8 samples -> 8 NeuronCores.

v3 design notes (tunnel-transfer-bound, so minimize bytes moved):
- Frames reparametrized: s=(f_tm2+f_tm1)/2 (bf16, also the blend base),
  d=(f_tm2-f_tm1)/2 (fp8, conv tower only), c=f_t (bf16).  conv1 weights
  are transformed host-side (W1s=W1a+W1b, W1d=W1a-W1b) so the tower
  consumes (s, d, c) directly.  5 B/pixel shipped vs 8 B/pixel in v2.
- All tensors ship tight (56x56); padding to the 64x64 conv frames is
  done on device (memzero + interior DMA).  fp8 tower copies of s and c
  are cast on device instead of shipped.
- The big weights (conv fp8 + fc bf16) are sharded 1/8 per core and
  AllGathered on device over NeuronLink: 5.4 MB total over the host
  tunnel instead of 43 MB replicated.
- identity / ones constants ride inside the NEFF (inline_tensor).
- Custom exec path (no donated zero output buffers -- the kernel writes
  every output element) with the compiled executable cached in a module
  global, so repeat calls skip build+compile.
- Conv tower runs fp8 DoubleRow as in v2; depthwise + blend read bf16.
"""

import sys

if "/opt/trn_rl_repo" not in sys.path:
    sys.path.insert(0, "/opt/trn_rl_repo")

import numpy as np
import ml_dtypes
import jax
from jax.sharding import Mesh, PartitionSpec
from jax.experimental.shard_map import shard_map

import concourse.bacc as bacc
import concourse.tile as tile
from concourse import mybir
from concourse import bass2jax

BF16 = ml_dtypes.bfloat16
FP8 = ml_dtypes.float8_e4m3

B, C, H, W = 8, 256, 56, 56
HID = 256
P = 128
FW = 64          # conv frame width (56 + 2*4 pad)
R0 = 4           # first image row in the padded frame
NR = 8           # rows per chunk
NCH = 7          # chunks (56 = 7*8)
WS = 64.0        # fp8 weight scale

QCOLS = 9 * 3 * 2 * 2 * P + 9 * 2 * 2 * P   # 13824 + 4608 = 18432
QSH = QCOLS // 8                             # 2304
W1SZ = 9 * 3 * 2 * 2 * P                     # 13824
BCOLS = 12288
BSH = BCOLS // 8                             # 1536
OFF_GW, OFF_FC1, OFF_FC2, OFF_AWM, OFF_AWP = 0, 512, 2560, 11776, 11778

# single-blob input layout (byte offsets per core); one jax array per call
# minimizes the per-array tunnel dispatch overhead (~85 ms/array)
FRB = 2 * P * H * W            # bytes of one fp8 frame pair = 802816
SOFF = 0                       # s bf16 [2,128,56,56]
COFF = SOFF + 2 * FRB          # c bf16
DOFF = COFF + 2 * FRB          # d fp8
QOFF = DOFF + FRB              # qsh fp8 [128,2304]
BOFF = QOFF + P * QSH          # bsh bf16 [128,1536]
CFOFF = BOFF + P * BSH * 2     # cf f32 [128,28]
ABOFF = CFOFF + P * 28 * 4     # ab f32 [1,1]
NB = ABOFF + 4

LAST_INFO = {}
_CACHE = {}


def _taps(d):
    return [(ky * 3 + kx, (ky - 1) * d, (kx - 1) * d) for ky in range(3) for kx in range(3)]


def build_nc():
    nc = bacc.Bacc()
    f32 = mybir.dt.float32
    bf16 = mybir.dt.bfloat16
    fp8 = mybir.dt.float8e4
    DR = mybir.MatmulPerfMode.DoubleRow

    Relu = mybir.ActivationFunctionType.Relu
    Sigmoid = mybir.ActivationFunctionType.Sigmoid
    mult = mybir.AluOpType.mult
    add = mybir.AluOpType.add

    # ---- dram I/O: ONE uint8 blob in (regions bitcast on device), bf16 out ----
    xin = nc.dram_tensor("xin", [NB], mybir.dt.uint8, kind="ExternalInput")

    y = nc.dram_tensor("y", [2, P, H, W], bf16, kind="ExternalOutput")

    identc = nc.inline_tensor(np.eye(P, dtype=np.float32).astype(BF16), name="identc")
    onesrc = nc.inline_tensor(np.ones((1, P), np.float32).astype(BF16), name="onesrc")

    with tile.TileContext(nc) as tc:
        with (
            tc.tile_pool(name="mp", bufs=1) as mp,
            tc.tile_pool(name="dp", bufs=1, space="DRAM") as dp,
            tc.tile_pool(name="psb", bufs=4, space="PSUM") as psb,
            tc.tile_pool(name="pss", bufs=2, space="PSUM") as pss,
            tc.tile_pool(name="psa", bufs=2, space="PSUM") as psa,
        ):
            xqs = mp.tile([P, 3, 2, FW, FW], fp8, name="xqs")   # tower input: s,d,c
            xss = mp.tile([P, 2, FW, FW], bf16, name="xss")     # s padded (blend base)
            xcs = mp.tile([P, 2, FW, FW], bf16, name="xcs")     # c padded (depthwise in)
            y1s = mp.tile([P, 2, FW, FW], fp8, name="y1s")
            fms = mp.tile([P, 2, H, FW], bf16, name="fms")
            w1s = mp.tile([P, 3, 9, 2, 2, P], fp8, name="w1s")
            w2s = mp.tile([P, 9, 2, 2, P], fp8, name="w2s")
            gws = mp.tile([P, 2, HID], bf16, name="gws")
            fc1ws = mp.tile([P, 4, 512], bf16, name="fc1ws")
            fc2ws = mp.tile([P, 18, 4, P], bf16, name="fc2ws")
            ident = mp.tile([P, P], bf16, name="ident")
            awm = mp.tile([P, 2], bf16, name="awm")
            awp = mp.tile([P, 2], bf16, name="awp")
            cfs = mp.tile([P, 28], f32, name="cfs")
            onesrs = mp.tile([1, P], bf16, name="onesrs")
            abts = mp.tile([1, 1], f32, name="abts")
            pacc = [mp.tile([P, NCH], f32, name=f"pacc{o}") for o in range(2)]
            gsum = mp.tile([P, 2], f32, name="gsum")
            lsum = mp.tile([P, 2], f32, name="lsum")
            gsumb = mp.tile([P, 2], bf16, name="gsumb")
            fcinb = mp.tile([P, 4], bf16, name="fcinb")
            hb = mp.tile([P, 4], bf16, name="hb")
            wkt = mp.tile([P, 18], f32, name="wkt")
            wks = mp.tile([P, 18], f32, name="wks")
            diag = [mp.tile([P, 9, P], bf16, name=f"diag{o}") for o in range(2)]

            qb = dp.tile([P, QSH], fp8, name="qb")
            qg = dp.tile([8, P, QSH], fp8, name="qg")
            bb = dp.tile([P, BSH], bf16, name="bb")
            bg = dp.tile([8, P, BSH], bf16, name="bg")

            # ---------- weight AllGather (NeuronLink, not host tunnel) ----------
            nc.gpsimd.dma_start(qb[:], xin[QOFF:QOFF + P * QSH].bitcast(fp8))
            nc.gpsimd.collective_compute(
                "AllGather", mybir.AluOpType.bypass,
                replica_groups=[list(range(8))], ins=[qb.opt()], outs=[qg.opt()])
            nc.gpsimd.dma_start(bb[:], xin[BOFF:BOFF + P * BSH * 2].bitcast(bf16))
            nc.gpsimd.collective_compute(
                "AllGather", mybir.AluOpType.bypass,
                replica_groups=[list(range(8))], ins=[bb.opt()], outs=[bg.opt()])

            w1f = w1s.rearrange("p a b c d e -> p (a b c d e)")
            w2f = w2s.rearrange("p a b c d -> p (a b c d)")
            for r in range(6):
                nc.sync.dma_start(out=w1f[:, r * QSH:(r + 1) * QSH], in_=qg[r])
            for r in (6, 7):
                o0 = (r - 6) * QSH
                nc.sync.dma_start(out=w2f[:, o0:o0 + QSH], in_=qg[r])

            # ---------- frame staging ----------
            nc.scalar.memzero(xss)
            nc.scalar.memzero(xcs)
            nc.scalar.memzero(xqs)
            for o in range(2):
                nc.sync.dma_start(
                    out=xss[:, o, R0:R0 + H, R0:R0 + W],
                    in_=xin[SOFF + o * FRB:SOFF + (o + 1) * FRB].bitcast(bf16))
                nc.sync.dma_start(
                    out=xcs[:, o, R0:R0 + H, R0:R0 + W],
                    in_=xin[COFF + o * FRB:COFF + (o + 1) * FRB].bitcast(bf16))
                nc.sync.dma_start(
                    out=xqs[:, 1, o, R0:R0 + H, R0:R0 + W],
                    in_=xin[DOFF + o * (FRB // 2):DOFF + (o + 1) * (FRB // 2)].bitcast(fp8))
            nc.vector.tensor_copy(xqs[:, 0], xss)    # s -> fp8 (pad ring is zero)
            nc.vector.tensor_copy(xqs[:, 2], xcs)    # c -> fp8
            nc.sync.dma_start(out=cfs, in_=xin[CFOFF:CFOFF + P * 28 * 4].bitcast(f32))
            nc.sync.dma_start(out=abts, in_=xin[ABOFF:ABOFF + 4].bitcast(f32))
            nc.sync.dma_start(out=ident, in_=identc[:, :])
            nc.sync.dma_start(out=onesrs, in_=onesrc[:, :])
            nc.scalar.memzero(y1s)

            b1c = [cfs[:, 0:1], cfs[:, 1:2]]
            b2c = [cfs[:, 2:3], cfs[:, 3:4]]
            gbc = cfs[:, 4:6]
            fc1b = cfs[:, 6:10]
            fc2bT = cfs[:, 10:28]

            taps1 = _taps(2)
            taps2 = _taps(4)
            taps3 = _taps(1)

            # ---------- conv1 ----------
            for c in range(NCH):
                for o in range(2):
                    ps = psb.tile([P, NR, W], f32, name=f"c1_{o}_{c}", tag="psb")
                    psl = ps.rearrange("p a b -> p (a b)")
                    mms = []
                    for kp in range(3):
                        for (t, dy, dx) in taps1:
                            r = R0 + NR * c + dy
                            mms.append((w1s[:, kp, t, o, :, :],
                                        xqs[:, kp, :, r:r + NR, 4 + dx:60 + dx]))
                    for n, (wv, xv) in enumerate(mms):
                        nc.tensor.matmul(psl, wv, xv, start=(n == 0),
                                         stop=(n == len(mms) - 1), perf_mode=DR)
                    nc.scalar.activation(
                        out=y1s[:, o, R0 + NR * c:R0 + NR * c + NR, 4:60],
                        in_=ps, func=Relu,
                        bias=b1c[o], scale=1.0 / WS,
                    )

            # late-use loads: emitted after conv1 so they queue behind the
            # conv-critical transfers on the DMA engines
            regions = [
                (gws.rearrange("p a b -> p (a b)"), OFF_GW, 2 * HID),
                (fc1ws.rearrange("p a b -> p (a b)"), OFF_FC1, 4 * 512),
                (fc2ws.rearrange("p a b c -> p (a b c)"), OFF_FC2, 18 * 4 * P),
                (awm, OFF_AWM, 2),
                (awp, OFF_AWP, 2),
            ]
            for dst, roff, rlen in regions:
                for r in range(8):
                    a = max(roff, r * BSH)
                    b2 = min(roff + rlen, (r + 1) * BSH)
                    if a < b2:
                        nc.sync.dma_start(out=dst[:, a - roff:b2 - roff],
                                          in_=bg[r][:, a - r * BSH:b2 - r * BSH])
            # local pooling of f_t (DVE, idle during conv)
            for o in range(2):
                nc.vector.tensor_reduce(
                    out=lsum[:, o:o + 1], in_=xcs[:, o, 4:60, 4:60],
                    axis=mybir.AxisListType.XY, op=add,
                )

            # ---------- conv2 + pooled accumulation ----------
            for c in range(NCH):
                for o in range(2):
                    ps = psb.tile([P, NR, W], f32, name=f"c2_{o}_{c}", tag="psb")
                    psl = ps.rearrange("p a b -> p (a b)")
                    mms = []
                    for (t, dy, dx) in taps2:
                        r = R0 + NR * c + dy
                        mms.append((w2s[:, t, o, :, :],
                                    y1s[:, :, r:r + NR, 4 + dx:60 + dx]))
                    for n, (wv, xv) in enumerate(mms):
                        nc.tensor.matmul(psl, wv, xv, start=(n == 0),
                                         stop=(n == len(mms) - 1), perf_mode=DR)
                    sc2 = mp.tile([P, NR, 56], bf16, name=f"sc2_{o}_{c}", tag="sc2", bufs=2)
                    nc.scalar.activation(
                        out=sc2, in_=ps, func=Relu,
                        bias=b2c[o], scale=1.0 / WS,
                        accum_out=pacc[o][:, c:c + 1],
                    )

            # ---------- pools -> fc chain ----------
            for o in range(2):
                nc.vector.tensor_reduce(
                    out=gsum[:, o:o + 1], in_=pacc[o],
                    axis=mybir.AxisListType.X, op=add,
                )
            nc.vector.tensor_copy(gsumb, gsum)

            psg = pss.tile([P, 2], f32, name="psg", tag="pss")
            for m in range(2):
                for k in range(2):
                    nc.tensor.matmul(
                        psg[:, m:m + 1], gws[:, k, m * P:(m + 1) * P],
                        gsumb[:, k:k + 1], start=(k == 0), stop=(k == 1),
                    )
            nc.vector.tensor_add(fcinb[:, 0:2], psg, gbc)
            nc.vector.tensor_copy(fcinb[:, 2:4], lsum)

            psh = pss.tile([P, 4], f32, name="psh", tag="pss")
            for m in range(4):
                for k in range(4):
                    nc.tensor.matmul(
                        psh[:, m:m + 1], fc1ws[:, k, m * P:(m + 1) * P],
                        fcinb[:, k:k + 1], start=(k == 0), stop=(k == 3),
                    )
            nc.vector.tensor_add(hb, psh, fc1b)

            psT = pss.tile([P, 18], f32, name="psT", tag="pss")
            for j in range(18):
                for kc in range(4):
                    nc.tensor.matmul(
                        psT[:, j:j + 1], fc2ws[:, j, kc, :],
                        hb[:, kc:kc + 1], start=(kc == 0), stop=(kc == 3),
                    )
            nc.vector.tensor_add(wkt, psT, fc2bT)
            # silu(z) = z * sigmoid(z)
            nc.scalar.activation(out=wks, in_=wkt, func=Sigmoid)
            nc.vector.tensor_mul(wks, wks, wkt)

            for j in range(18):
                nc.vector.tensor_scalar_mul(diag[j // 9][:, j % 9, :], ident,
                                            wks[:, j:j + 1])

            # ---------- depthwise + alpha + fusion (valid cols only) ----------
            for c in range(NCH):
                rows = slice(NR * c, NR * c + NR)
                prow = slice(R0 + NR * c, R0 + NR * c + NR)   # rows in padded tiles
                for o in range(2):
                    ps = psb.tile([P, NR, W], f32, name=f"dw_{o}_{c}", tag="psb")
                    psl = ps.rearrange("p a b -> p (a b)")
                    for (t, dy, dx) in taps3:
                        nc.tensor.matmul(
                            psl, diag[o][:, t, :],
                            xcs[:, o, R0 + NR * c + dy:R0 + NR * c + dy + NR,
                                4 + dx:60 + dx],
                            start=(t == 0), stop=(t == 8),
                        )
                    nc.scalar.copy(fms[:, o, rows, 4:60], ps)

                pa = psa.tile([1, NR, W], f32, name=f"pa{c}", tag="psa")
                pal = pa.rearrange("p a b -> p (a b)")
                for o in range(2):
                    nc.tensor.matmul(
                        pal, awm[:, o:o + 1], fms[:, o, rows, 4:60],
                        start=(o == 0), stop=False,
                    )
                for o in range(2):
                    nc.tensor.matmul(
                        pal, awp[:, o:o + 1], xss[:, o, prow, 4:60],
                        start=False, stop=(o == 1),
                    )
                arow = mp.tile([1, NR, W], bf16, name=f"ar{c}", tag="ar", bufs=2)
                nc.scalar.activation(out=arow, in_=pa, func=Sigmoid, bias=abts[:, 0:1])
                nc.vector.tensor_scalar(arow, arow, 0.4, 0.3, op0=mult, op1=add)
                pb = psa.tile([P, NR, W], f32, name=f"pb{c}", tag="psa")
                nc.tensor.matmul(pb.rearrange("p a b -> p (a b)"), onesrs,
                                 arow.rearrange("p a b -> p (a b)"),
                                 start=True, stop=True)

                for o in range(2):
                    u = mp.tile([P, NR, W], f32, name=f"u{c}{o}", tag="u", bufs=3)
                    nc.vector.scalar_tensor_tensor(
                        u, xss[:, o, prow, 4:60], -1.0, fms[:, o, rows, 4:60],
                        op0=mult, op1=add,
                    )
                    nc.vector.tensor_mul(u, u, pb)
                    nc.vector.tensor_add(xss[:, o, prow, 4:60], xss[:, o, prow, 4:60], u)
                    # stream this chunk's rows out while later chunks compute
                    nc.sync.dma_start(out=y[o, :, rows, :],
                                      in_=xss[:, o, prow, 4:60])

    nc.compile()
    return nc


def _get_exec():
    if "jitted" in _CACHE:
        return _CACHE
    nc = build_nc()
    bass2jax.install_neuronx_cc_hook()
    partition_name = nc.partition_id_tensor.name if nc.partition_id_tensor else None
    in_names, out_names, out_avals = [], [], []
    for alloc in nc.m.functions[0].allocations:
        if not isinstance(alloc, mybir.MemoryLocationSet):
            continue
        name = alloc.memorylocations[0].name
        if alloc.kind == "ExternalInput":
            if name != partition_name:
                in_names.append(name)
        elif alloc.kind == "ExternalOutput":
            out_names.append(name)
            out_avals.append(jax.core.ShapedArray(
                tuple(alloc.tensor_shape), mybir.dt.np(alloc.dtype)))
    n_params = len(in_names)
    param_names = list(in_names)
    if partition_name is not None:
        in_names.append(partition_name)

    def _body(*args):
        operands = list(args)
        if partition_name is not None:
            operands.append(bass2jax.partition_id_tensor())
        return tuple(bass2jax._bass_exec_p.bind(
            *operands, out_avals=tuple(out_avals),
            in_names=tuple(in_names), out_names=tuple(out_names),
            lowering_input_output_aliases=(), sim_require_finite=True,
            sim_require_nnan=True, nc=nc))

    devices = jax.devices()[:8]
    mesh = Mesh(np.asarray(devices), ("core",))
    jitted = jax.jit(shard_map(
        _body, mesh=mesh,
        in_specs=(PartitionSpec("core"),) * n_params,
        out_specs=(PartitionSpec("core"),) * len(out_names), check_rep=False))
    _CACHE.update(dict(jitted=jitted, param_names=param_names,
                       out_names=out_names, out_avals=out_avals))
    return _CACHE


def _prep_shared(w1, b1, w2, b2, gw, gb, fc1_w, fc1_b, fc2_w, fc2_b, aw, ab):
    """Returns per-region [8, nbytes] uint8 arrays for the shared weights."""
    # conv1 weights with the (a,b,c)->(s,d,c) frame transform on axis kp
    w1r = w1.reshape(2, P, 3, 2, P, 3, 3)            # o m kp i k ty tx
    w1t = np.empty_like(w1r)
    w1t[:, :, 0] = w1r[:, :, 0] + w1r[:, :, 1]       # applies to s
    w1t[:, :, 1] = w1r[:, :, 0] - w1r[:, :, 1]       # applies to d
    w1t[:, :, 2] = w1r[:, :, 2]                      # applies to c
    w1q = np.ascontiguousarray(w1t.transpose(4, 2, 5, 6, 0, 3, 1))  # k kp ty tx o i m
    w1q = w1q.reshape(P, W1SZ)
    w2r = w2.reshape(2, P, 2, P, 3, 3)               # o m i k ty tx
    w2q = np.ascontiguousarray(w2r.transpose(3, 4, 5, 0, 2, 1))     # k ty tx o i m
    w2q = w2q.reshape(P, 9 * 2 * 2 * P)
    wq_full = (np.concatenate([w1q, w2q], axis=1) * WS).astype(FP8)  # [P, QCOLS]
    qsh_g = np.ascontiguousarray(
        wq_full.reshape(P, 8, QSH).transpose(1, 0, 2)).view(np.uint8).reshape(8, P * QSH)

    gwt = np.ascontiguousarray((gw[:, :, 0, 0] / 3136.0).T).reshape(2, P, HID)
    gwb = np.ascontiguousarray(gwt.transpose(1, 0, 2)).reshape(P, 2 * HID)
    fc1t = fc1_w.T.copy()
    fc1t[C:, :] /= 3136.0
    fc1wb = np.ascontiguousarray(
        fc1t.reshape(4, P, 512).transpose(1, 0, 2)).reshape(P, 4 * 512)
    f2 = fc2_w.T.reshape(4, P, 2, P, 9)              # kc k bl p t
    fc2wb = np.ascontiguousarray(f2.transpose(1, 2, 4, 0, 3)).reshape(P, 18 * 4 * P)
    awm = np.ascontiguousarray(aw[0, :C, 0, 0].reshape(2, P).T)      # [128, 2]
    awp = np.ascontiguousarray(aw[0, C:, 0, 0].reshape(2, P).T)
    wb_full = np.zeros((P, BCOLS), dtype=np.float32)
    wb_full[:, OFF_GW:OFF_GW + 512] = gwb
    wb_full[:, OFF_FC1:OFF_FC1 + 2048] = fc1wb
    wb_full[:, OFF_FC2:OFF_FC2 + 9216] = fc2wb
    wb_full[:, OFF_AWM:OFF_AWM + 2] = awm
    wb_full[:, OFF_AWP:OFF_AWP + 2] = awp
    wb_full = wb_full.astype(BF16)
    bsh_g = np.ascontiguousarray(
        wb_full.reshape(P, 8, BSH).transpose(1, 0, 2)).view(np.uint8).reshape(8, P * BSH * 2)

    fc1b4 = np.ascontiguousarray(fc1_b.reshape(4, P).T)              # [128, 4]
    b1c = b1.reshape(2, P).T
    b2c = b2.reshape(2, P).T
    gbc = gb.reshape(2, P).T
    fc2bT = np.ascontiguousarray(fc2_b.reshape(2, P, 9).transpose(1, 0, 2)).reshape(P, 18)
    cf1 = np.concatenate([b1c, b2c, gbc, fc1b4, fc2bT], axis=1).astype(np.float32)
    cf_g = np.ascontiguousarray(
        np.broadcast_to(cf1, (8,) + cf1.shape)).view(np.uint8).reshape(8, P * 28 * 4)
    abt_g = np.ascontiguousarray(np.broadcast_to(
        ab.reshape(1).astype(np.float32), (8, 1))).view(np.uint8).reshape(8, 4)
    return qsh_g, bsh_g, cf_g, abt_g


def kernel(f_tm2, f_tm1, f_t, w1, b1, w2, b2, gw, gb,
           fc1_w, fc1_b, fc2_w, fc2_b, aw, ab):
    import time

    args = [np.asarray(a, dtype=np.float32) for a in
            (f_tm2, f_tm1, f_t, w1, b1, w2, b2, gw, gb, fc1_w, fc1_b, fc2_w, fc2_b, aw, ab)]
    f_tm2, f_tm1, f_t = args[0], args[1], args[2]

    t0 = time.time()
    g = _get_exec()
    t1 = time.time()

    s = (f_tm2 + f_tm1) * np.float32(0.5)
    d = f_tm2 - f_tm1
    d *= np.float32(0.5)
    s8 = s.astype(BF16).view(np.uint8).reshape(B, 2 * FRB)
    c8 = f_t.astype(BF16).view(np.uint8).reshape(B, 2 * FRB)
    d8 = d.astype(FP8).view(np.uint8).reshape(B, FRB)
    qsh_g, bsh_g, cf_g, abt_g = _prep_shared(*args[3:])
    blob = np.concatenate([s8, c8, d8, qsh_g, bsh_g, cf_g, abt_g], axis=1)
    assert blob.shape == (B, NB), blob.shape
    blob = blob.reshape(B * NB)
    t2 = time.time()

    out_arrs = g["jitted"](blob)
    yv = np.asarray(out_arrs[0])                      # [8*2, 128, 56, 56] bf16
    t3 = time.time()

    out = (yv.view(np.uint16).astype(np.uint32) << np.uint32(16)).view(np.float32)
    out = out.reshape(B, C, H, W)
    t4 = time.time()
    LAST_INFO.update(dict(build_s=t1 - t0, prep_s=t2 - t1, run_s=t3 - t2,
                          post_s=t4 - t3, exec_time_ns=None))
    return out


# revision 14
# speedup vs baseline: 35.1967x; 1.0322x over previous
"""Trainium2 Bass kernel for nn_ContMixT (dense_cnn).

Data-parallel over batch: 8 samples -> 8 NeuronCores.

v3 design notes (tunnel-transfer-bound, so minimize bytes moved):
- Frames reparametrized: s=(f_tm2+f_tm1)/2 (bf16, also the blend base),
  d=(f_tm2-f_tm1)/2 (fp8, conv tower only), c=f_t (bf16).  conv1 weights
  are transformed host-side (W1s=W1a+W1b, W1d=W1a-W1b) so the tower
  consumes (s, d, c) directly.  5 B/pixel shipped vs 8 B/pixel in v2.
- All tensors ship tight (56x56); padding to the 64x64 conv frames is
  done on device (memzero + interior DMA).  fp8 tower copies of s and c
  are cast on device instead of shipped.
- The big weights (conv fp8 + fc bf16) are sharded 1/8 per core and
  AllGathered on device over NeuronLink: 5.4 MB total over the host
  tunnel instead of 43 MB replicated.
- identity / ones constants ride inside the NEFF (inline_tensor).
- Custom exec path (no donated zero output buffers -- the kernel writes
  every output element) with the compiled executable cached in a module
  global, so repeat calls skip build+compile.
- Conv tower runs fp8 DoubleRow as in v2; depthwise + blend read bf16.
"""

import sys

if "/opt/trn_rl_repo" not in sys.path:
    sys.path.insert(0, "/opt/trn_rl_repo")

import numpy as np
import ml_dtypes
import jax
from jax.sharding import Mesh, PartitionSpec
from jax.experimental.shard_map import shard_map

import concourse.bacc as bacc
import concourse.tile as tile
from concourse import mybir
from concourse import bass2jax

BF16 = ml_dtypes.bfloat16
FP8 = ml_dtypes.float8_e4m3

B, C, H, W = 8, 256, 56, 56
HID = 256
P = 128
FW = 64          # conv frame width (56 + 2*4 pad)
R0 = 4           # first image row in the padded frame
NR = 8           # rows per chunk
NCH = 7          # chunks (56 = 7*8)
WS = 64.0        # fp8 weight scale

QCOLS = 9 * 3 * 2 * 2 * P + 9 * 2 * 2 * P   # 13824 + 4608 = 18432
QSH = QCOLS // 8                             # 2304
W1SZ = 9 * 3 * 2 * 2 * P                     # 13824
BCOLS = 12288
BSH = BCOLS // 8                             # 1536
OFF_GW, OFF_FC1, OFF_FC2, OFF_AWM, OFF_AWP = 0, 512, 2560, 11776, 11778

# single-blob input layout (byte offsets per core); one jax array per call
# minimizes the per-array tunnel dispatch overhead (~85 ms/array)
FRB = 2 * P * H * W            # bytes of one fp8 frame pair = 802816
SOFF = 0                       # s bf16 [2,128,56,56]
COFF = SOFF + 2 * FRB          # c fp8 (f_t; depthwise reads the exact upcast)
DOFF = COFF + FRB              # d fp8
QOFF = DOFF + FRB              # qsh fp8 [128,2304]
BOFF = QOFF + P * QSH          # bsh bf16 [128,1536]
CFOFF = BOFF + P * BSH * 2     # cf f32 [128,28]
ABOFF = CFOFF + P * 28 * 4     # ab f32 [1,1]
NB = ABOFF + 4

LAST_INFO = {}
_CACHE = {}


def _taps(d):
    return [(ky * 3 + kx, (ky - 1) * d, (kx - 1) * d) for ky in range(3) for kx in range(3)]


def build_nc():
    nc = bacc.Bacc()
    f32 = mybir.dt.float32
    bf16 = mybir.dt.bfloat16
    fp8 = mybir.dt.float8e4
    DR = mybir.MatmulPerfMode.DoubleRow

    Relu = mybir.ActivationFunctionType.Relu
    Sigmoid = mybir.ActivationFunctionType.Sigmoid
    mult = mybir.AluOpType.mult
    add = mybir.AluOpType.add

    # ---- dram I/O: ONE uint8 blob in (regions bitcast on device), bf16 out ----
    xin = nc.dram_tensor("xin", [NB], mybir.dt.uint8, kind="ExternalInput")

    y = nc.dram_tensor("y", [2, P, H, W], bf16, kind="ExternalOutput")

    identc = nc.inline_tensor(np.eye(P, dtype=np.float32).astype(BF16), name="identc")
    onesrc = nc.inline_tensor(np.ones((1, P), np.float32).astype(BF16), name="onesrc")

    with tile.TileContext(nc) as tc:
        with (
            tc.tile_pool(name="mp", bufs=1) as mp,
            tc.tile_pool(name="dp", bufs=1, space="DRAM") as dp,
            tc.tile_pool(name="psb", bufs=4, space="PSUM") as psb,
            tc.tile_pool(name="pss", bufs=2, space="PSUM") as pss,
            tc.tile_pool(name="psa", bufs=2, space="PSUM") as psa,
        ):
            xqs = mp.tile([P, 3, 2, FW, FW], fp8, name="xqs")   # tower input: s,d,c
            xss = mp.tile([P, 2, FW, FW], bf16, name="xss")     # s padded (blend base)
            xcs = mp.tile([P, 2, FW, FW], bf16, name="xcs")     # c padded (depthwise in)
            y1s = mp.tile([P, 2, FW, FW], fp8, name="y1s")
            fms = mp.tile([P, 2, H, FW], bf16, name="fms")
            w1s = mp.tile([P, 3, 9, 2, 2, P], fp8, name="w1s")
            w2s = mp.tile([P, 9, 2, 2, P], fp8, name="w2s")
            gws = mp.tile([P, 2, HID], bf16, name="gws")
            fc1ws = mp.tile([P, 4, 512], bf16, name="fc1ws")
            fc2ws = mp.tile([P, 18, 4, P], bf16, name="fc2ws")
            ident = mp.tile([P, P], bf16, name="ident")
            awm = mp.tile([P, 2], bf16, name="awm")
            awp = mp.tile([P, 2], bf16, name="awp")
            cfs = mp.tile([P, 28], f32, name="cfs")
            onesrs = mp.tile([1, P], bf16, name="onesrs")
            abts = mp.tile([1, 1], f32, name="abts")
            pacc = [mp.tile([P, NCH], f32, name=f"pacc{o}") for o in range(2)]
            gsum = mp.tile([P, 2], f32, name="gsum")
            lsum = mp.tile([P, 2], f32, name="lsum")
            gsumb = mp.tile([P, 2], bf16, name="gsumb")
            fcinb = mp.tile([P, 4], bf16, name="fcinb")
            hb = mp.tile([P, 4], bf16, name="hb")
            wkt = mp.tile([P, 18], f32, name="wkt")
            wks = mp.tile([P, 18], f32, name="wks")
            diag = [mp.tile([P, 9, P], bf16, name=f"diag{o}") for o in range(2)]

            qb = dp.tile([P, QSH], fp8, name="qb")
            qg = dp.tile([8, P, QSH], fp8, name="qg")
            bb = dp.tile([P, BSH], bf16, name="bb")
            bg = dp.tile([8, P, BSH], bf16, name="bg")

            # ---------- weight AllGather (NeuronLink, not host tunnel) ----------
            nc.gpsimd.dma_start(qb[:], xin[QOFF:QOFF + P * QSH].bitcast(fp8))
            nc.gpsimd.collective_compute(
                "AllGather", mybir.AluOpType.bypass,
                replica_groups=[list(range(8))], ins=[qb.opt()], outs=[qg.opt()])
            nc.gpsimd.dma_start(bb[:], xin[BOFF:BOFF + P * BSH * 2].bitcast(bf16))
            nc.gpsimd.collective_compute(
                "AllGather", mybir.AluOpType.bypass,
                replica_groups=[list(range(8))], ins=[bb.opt()], outs=[bg.opt()])

            w1f = w1s.rearrange("p a b c d e -> p (a b c d e)")
            w2f = w2s.rearrange("p a b c d -> p (a b c d)")
            for r in range(6):
                nc.sync.dma_start(out=w1f[:, r * QSH:(r + 1) * QSH], in_=qg[r])
            for r in (6, 7):
                o0 = (r - 6) * QSH
                nc.sync.dma_start(out=w2f[:, o0:o0 + QSH], in_=qg[r])

            # ---------- frame staging ----------
            nc.scalar.memzero(xss)
            nc.scalar.memzero(xqs)
            HFR = FRB // 2
            for o in range(2):
                nc.sync.dma_start(
                    out=xss[:, o, R0:R0 + H, R0:R0 + W],
                    in_=xin[SOFF + o * FRB:SOFF + (o + 1) * FRB].bitcast(bf16))
                nc.sync.dma_start(
                    out=xqs[:, 2, o, R0:R0 + H, R0:R0 + W],
                    in_=xin[COFF + o * HFR:COFF + (o + 1) * HFR].bitcast(fp8))
                nc.sync.dma_start(
                    out=xqs[:, 1, o, R0:R0 + H, R0:R0 + W],
                    in_=xin[DOFF + o * HFR:DOFF + (o + 1) * HFR].bitcast(fp8))
            nc.vector.tensor_copy(xqs[:, 0], xss)    # s -> fp8 (pad ring is zero)
            nc.vector.tensor_copy(xcs, xqs[:, 2])    # c fp8 -> bf16 (exact upcast)
            nc.sync.dma_start(out=cfs, in_=xin[CFOFF:CFOFF + P * 28 * 4].bitcast(f32))
            nc.sync.dma_start(out=abts, in_=xin[ABOFF:ABOFF + 4].bitcast(f32))
            nc.sync.dma_start(out=ident, in_=identc[:, :])
            nc.sync.dma_start(out=onesrs, in_=onesrc[:, :])
            nc.scalar.memzero(y1s)

            b1c = [cfs[:, 0:1], cfs[:, 1:2]]
            b2c = [cfs[:, 2:3], cfs[:, 3:4]]
            gbc = cfs[:, 4:6]
            fc1b = cfs[:, 6:10]
            fc2bT = cfs[:, 10:28]

            taps1 = _taps(2)
            taps2 = _taps(4)
            taps3 = _taps(1)

            # ---------- conv1 ----------
            for c in range(NCH):
                for o in range(2):
                    ps = psb.tile([P, NR, W], f32, name=f"c1_{o}_{c}", tag="psb")
                    psl = ps.rearrange("p a b -> p (a b)")
                    mms = []
                    for kp in range(3):
                        for (t, dy, dx) in taps1:
                            r = R0 + NR * c + dy
                            mms.append((w1s[:, kp, t, o, :, :],
                                        xqs[:, kp, :, r:r + NR, 4 + dx:60 + dx]))
                    for n, (wv, xv) in enumerate(mms):
                        nc.tensor.matmul(psl, wv, xv, start=(n == 0),
                                         stop=(n == len(mms) - 1), perf_mode=DR)
                    nc.scalar.activation(
                        out=y1s[:, o, R0 + NR * c:R0 + NR * c + NR, 4:60],
                        in_=ps, func=Relu,
                        bias=b1c[o], scale=1.0 / WS,
                    )

            # late-use loads: emitted after conv1 so they queue behind the
            # conv-critical transfers on the DMA engines
            regions = [
                (gws.rearrange("p a b -> p (a b)"), OFF_GW, 2 * HID),
                (fc1ws.rearrange("p a b -> p (a b)"), OFF_FC1, 4 * 512),
                (fc2ws.rearrange("p a b c -> p (a b c)"), OFF_FC2, 18 * 4 * P),
                (awm, OFF_AWM, 2),
                (awp, OFF_AWP, 2),
            ]
            for dst, roff, rlen in regions:
                for r in range(8):
                    a = max(roff, r * BSH)
                    b2 = min(roff + rlen, (r + 1) * BSH)
                    if a < b2:
                        nc.sync.dma_start(out=dst[:, a - roff:b2 - roff],
                                          in_=bg[r][:, a - r * BSH:b2 - r * BSH])
            # local pooling of f_t (DVE, idle during conv)
            for o in range(2):
                nc.vector.tensor_reduce(
                    out=lsum[:, o:o + 1], in_=xcs[:, o, 4:60, 4:60],
                    axis=mybir.AxisListType.XY, op=add,
                )

            # ---------- conv2 + pooled accumulation ----------
            for c in range(NCH):
                for o in range(2):
                    ps = psb.tile([P, NR, W], f32, name=f"c2_{o}_{c}", tag="psb")
                    psl = ps.rearrange("p a b -> p (a b)")
                    mms = []
                    for (t, dy, dx) in taps2:
                        r = R0 + NR * c + dy
                        mms.append((w2s[:, t, o, :, :],
                                    y1s[:, :, r:r + NR, 4 + dx:60 + dx]))
                    for n, (wv, xv) in enumerate(mms):
                        nc.tensor.matmul(psl, wv, xv, start=(n == 0),
                                         stop=(n == len(mms) - 1), perf_mode=DR)
                    sc2 = mp.tile([P, NR, 56], bf16, name=f"sc2_{o}_{c}", tag="sc2", bufs=2)
                    nc.scalar.activation(
                        out=sc2, in_=ps, func=Relu,
                        bias=b2c[o], scale=1.0 / WS,
                        accum_out=pacc[o][:, c:c + 1],
                    )

            # ---------- pools -> fc chain ----------
            for o in range(2):
                nc.vector.tensor_reduce(
                    out=gsum[:, o:o + 1], in_=pacc[o],
                    axis=mybir.AxisListType.X, op=add,
                )
            nc.vector.tensor_copy(gsumb, gsum)

            psg = pss.tile([P, 2], f32, name="psg", tag="pss")
            for m in range(2):
                for k in range(2):
                    nc.tensor.matmul(
                        psg[:, m:m + 1], gws[:, k, m * P:(m + 1) * P],
                        gsumb[:, k:k + 1], start=(k == 0), stop=(k == 1),
                    )
            nc.vector.tensor_add(fcinb[:, 0:2], psg, gbc)
            nc.vector.tensor_copy(fcinb[:, 2:4], lsum)

            psh = pss.tile([P, 4], f32, name="psh", tag="pss")
            for m in range(4):
                for k in range(4):
                    nc.tensor.matmul(
                        psh[:, m:m + 1], fc1ws[:, k, m * P:(m + 1) * P],
                        fcinb[:, k:k + 1], start=(k == 0), stop=(k == 3),
                    )
            nc.vector.tensor_add(hb, psh, fc1b)

            psT = pss.tile([P, 18], f32, name="psT", tag="pss")
            for j in range(18):
                for kc in range(4):
                    nc.tensor.matmul(
                        psT[:, j:j + 1], fc2ws[:, j, kc, :],
                        hb[:, kc:kc + 1], start=(kc == 0), stop=(kc == 3),
                    )
            nc.vector.tensor_add(wkt, psT, fc2bT)
            # silu(z) = z * sigmoid(z)
            nc.scalar.activation(out=wks, in_=wkt, func=Sigmoid)
            nc.vector.tensor_mul(wks, wks, wkt)

            for j in range(18):
                nc.vector.tensor_scalar_mul(diag[j // 9][:, j % 9, :], ident,
                                            wks[:, j:j + 1])

            # ---------- depthwise + alpha + fusion (valid cols only) ----------
            for c in range(NCH):
                rows = slice(NR * c, NR * c + NR)
                prow = slice(R0 + NR * c, R0 + NR * c + NR)   # rows in padded tiles
                for o in range(2):
                    ps = psb.tile([P, NR, W], f32, name=f"dw_{o}_{c}", tag="psb")
                    psl = ps.rearrange("p a b -> p (a b)")
                    for (t, dy, dx) in taps3:
                        nc.tensor.matmul(
                            psl, diag[o][:, t, :],
                            xcs[:, o, R0 + NR * c + dy:R0 + NR * c + dy + NR,
                                4 + dx:60 + dx],
                            start=(t == 0), stop=(t == 8),
                        )
                    nc.scalar.copy(fms[:, o, rows, 4:60], ps)

                pa = psa.tile([1, NR, W], f32, name=f"pa{c}", tag="psa")
                pal = pa.rearrange("p a b -> p (a b)")
                for o in range(2):
                    nc.tensor.matmul(
                        pal, awm[:, o:o + 1], fms[:, o, rows, 4:60],
                        start=(o == 0), stop=False,
                    )
                for o in range(2):
                    nc.tensor.matmul(
                        pal, awp[:, o:o + 1], xss[:, o, prow, 4:60],
                        start=False, stop=(o == 1),
                    )
                arow = mp.tile([1, NR, W], bf16, name=f"ar{c}", tag="ar", bufs=2)
                nc.scalar.activation(out=arow, in_=pa, func=Sigmoid, bias=abts[:, 0:1])
                nc.vector.tensor_scalar(arow, arow, 0.4, 0.3, op0=mult, op1=add)
                pb = psa.tile([P, NR, W], f32, name=f"pb{c}", tag="psa")
                nc.tensor.matmul(pb.rearrange("p a b -> p (a b)"), onesrs,
                                 arow.rearrange("p a b -> p (a b)"),
                                 start=True, stop=True)

                for o in range(2):
                    u = mp.tile([P, NR, W], f32, name=f"u{c}{o}", tag="u", bufs=3)
                    nc.vector.scalar_tensor_tensor(
                        u, xss[:, o, prow, 4:60], -1.0, fms[:, o, rows, 4:60],
                        op0=mult, op1=add,
                    )
                    nc.vector.tensor_mul(u, u, pb)
                    nc.vector.tensor_add(xss[:, o, prow, 4:60], xss[:, o, prow, 4:60], u)
                    # stream this chunk's rows out while later chunks compute
                    nc.sync.dma_start(out=y[o, :, rows, :],
                                      in_=xss[:, o, prow, 4:60])

    nc.compile()
    return nc


def _get_exec():
    if "jitted" in _CACHE:
        return _CACHE
    nc = build_nc()
    bass2jax.install_neuronx_cc_hook()
    partition_name = nc.partition_id_tensor.name if nc.partition_id_tensor else None
    in_names, out_names, out_avals = [], [], []
    for alloc in nc.m.functions[0].allocations:
        if not isinstance(alloc, mybir.MemoryLocationSet):
            continue
        name = alloc.memorylocations[0].name
        if alloc.kind == "ExternalInput":
            if name != partition_name:
                in_names.append(name)
        elif alloc.kind == "ExternalOutput":
            out_names.append(name)
            out_avals.append(jax.core.ShapedArray(
                tuple(alloc.tensor_shape), mybir.dt.np(alloc.dtype)))
    n_params = len(in_names)
    param_names = list(in_names)
    if partition_name is not None:
        in_names.append(partition_name)

    def _body(*args):
        operands = list(args)
        if partition_name is not None:
            operands.append(bass2jax.partition_id_tensor())
        return tuple(bass2jax._bass_exec_p.bind(
            *operands, out_avals=tuple(out_avals),
            in_names=tuple(in_names), out_names=tuple(out_names),
            lowering_input_output_aliases=(), sim_require_finite=True,
            sim_require_nnan=True, nc=nc))

    devices = jax.devices()[:8]
    mesh = Mesh(np.asarray(devices), ("core",))
    jitted = jax.jit(shard_map(
        _body, mesh=mesh,
        in_specs=(PartitionSpec("core"),) * n_params,
        out_specs=(PartitionSpec("core"),) * len(out_names), check_rep=False))
    _CACHE.update(dict(jitted=jitted, param_names=param_names,
                       out_names=out_names, out_avals=out_avals))
    return _CACHE


def _prep_shared(w1, b1, w2, b2, gw, gb, fc1_w, fc1_b, fc2_w, fc2_b, aw, ab):
    """Returns per-region [8, nbytes] uint8 arrays for the shared weights."""
    # conv1 weights with the (a,b,c)->(s,d,c) frame transform on axis kp
    w1r = w1.reshape(2, P, 3, 2, P, 3, 3)            # o m kp i k ty tx
    w1t = np.empty_like(w1r)
    w1t[:, :, 0] = w1r[:, :, 0] + w1r[:, :, 1]       # applies to s
    w1t[:, :, 1] = w1r[:, :, 0] - w1r[:, :, 1]       # applies to d
    w1t[:, :, 2] = w1r[:, :, 2]                      # applies to c
    w1q = np.ascontiguousarray(w1t.transpose(4, 2, 5, 6, 0, 3, 1))  # k kp ty tx o i m
    w1q = w1q.reshape(P, W1SZ)
    w2r = w2.reshape(2, P, 2, P, 3, 3)               # o m i k ty tx
    w2q = np.ascontiguousarray(w2r.transpose(3, 4, 5, 0, 2, 1))     # k ty tx o i m
    w2q = w2q.reshape(P, 9 * 2 * 2 * P)
    wq_full = (np.concatenate([w1q, w2q], axis=1) * WS).astype(FP8)  # [P, QCOLS]
    qsh_g = np.ascontiguousarray(
        wq_full.reshape(P, 8, QSH).transpose(1, 0, 2)).view(np.uint8).reshape(8, P * QSH)

    gwt = np.ascontiguousarray((gw[:, :, 0, 0] / 3136.0).T).reshape(2, P, HID)
    gwb = np.ascontiguousarray(gwt.transpose(1, 0, 2)).reshape(P, 2 * HID)
    fc1t = fc1_w.T.copy()
    fc1t[C:, :] /= 3136.0
    fc1wb = np.ascontiguousarray(
        fc1t.reshape(4, P, 512).transpose(1, 0, 2)).reshape(P, 4 * 512)
    f2 = fc2_w.T.reshape(4, P, 2, P, 9)              # kc k bl p t
    fc2wb = np.ascontiguousarray(f2.transpose(1, 2, 4, 0, 3)).reshape(P, 18 * 4 * P)
    awm = np.ascontiguousarray(aw[0, :C, 0, 0].reshape(2, P).T)      # [128, 2]
    awp = np.ascontiguousarray(aw[0, C:, 0, 0].reshape(2, P).T)
    wb_full = np.zeros((P, BCOLS), dtype=np.float32)
    wb_full[:, OFF_GW:OFF_GW + 512] = gwb
    wb_full[:, OFF_FC1:OFF_FC1 + 2048] = fc1wb
    wb_full[:, OFF_FC2:OFF_FC2 + 9216] = fc2wb
    wb_full[:, OFF_AWM:OFF_AWM + 2] = awm
    wb_full[:, OFF_AWP:OFF_AWP + 2] = awp
    wb_full = wb_full.astype(BF16)
    bsh_g = np.ascontiguousarray(
        wb_full.reshape(P, 8, BSH).transpose(1, 0, 2)).view(np.uint8).reshape(8, P * BSH * 2)

    fc1b4 = np.ascontiguousarray(fc1_b.reshape(4, P).T)              # [128, 4]
    b1c = b1.reshape(2, P).T
    b2c = b2.reshape(2, P).T
    gbc = gb.reshape(2, P).T
    fc2bT = np.ascontiguousarray(fc2_b.reshape(2, P, 9).transpose(1, 0, 2)).reshape(P, 18)
    cf1 = np.concatenate([b1c, b2c, gbc, fc1b4, fc2bT], axis=1).astype(np.float32)
    cf_g = np.ascontiguousarray(
        np.broadcast_to(cf1, (8,) + cf1.shape)).view(np.uint8).reshape(8, P * 28 * 4)
    abt_g = np.ascontiguousarray(np.broadcast_to(
        ab.reshape(1).astype(np.float32), (8, 1))).view(np.uint8).reshape(8, 4)
    return qsh_g, bsh_g, cf_g, abt_g


def kernel(f_tm2, f_tm1, f_t, w1, b1, w2, b2, gw, gb,
           fc1_w, fc1_b, fc2_w, fc2_b, aw, ab):
    import time

    args = [np.asarray(a, dtype=np.float32) for a in
            (f_tm2, f_tm1, f_t, w1, b1, w2, b2, gw, gb, fc1_w, fc1_b, fc2_w, fc2_b, aw, ab)]
    f_tm2, f_tm1, f_t = args[0], args[1], args[2]

    t0 = time.time()
    g = _get_exec()
    t1 = time.time()

    s = (f_tm2 + f_tm1) * np.float32(0.5)
    d = f_tm2 - f_tm1
    d *= np.float32(0.5)
    s8 = s.astype(BF16).view(np.uint8).reshape(B, 2 * FRB)
    c8 = f_t.astype(FP8).view(np.uint8).reshape(B, FRB)
    d8 = d.astype(FP8).view(np.uint8).reshape(B, FRB)
    qsh_g, bsh_g, cf_g, abt_g = _prep_shared(*args[3:])
    blob = np.concatenate([s8, c8, d8, qsh_g, bsh_g, cf_g, abt_g], axis=1)
    assert blob.shape == (B, NB), blob.shape
    blob = blob.reshape(B * NB)
    t2 = time.time()

    out_arrs = g["jitted"](blob)
    yv = np.asarray(out_arrs[0])                      # [8*2, 128, 56, 56] bf16
    t3 = time.time()

    o32 = yv.view(np.uint16).astype(np.uint32)
    np.left_shift(o32, 16, out=o32)
    out = o32.view(np.float32).reshape(B, C, H, W)
    t4 = time.time()
    LAST_INFO.update(dict(build_s=t1 - t0, prep_s=t2 - t1, run_s=t3 - t2,
                          post_s=t4 - t3, exec_time_ns=None))
    return out


# revision 16
# speedup vs baseline: 40.3599x; 1.1467x over previous
"""Trainium2 Bass kernel for nn_ContMixT (dense_cnn).

Data-parallel over batch: 8 samples -> 8 NeuronCores.

v3 design notes (tunnel-transfer-bound, so minimize bytes moved):
- Frames reparametrized: s=(f_tm2+f_tm1)/2 (bf16, also the blend base),
  d=(f_tm2-f_tm1)/2 (fp8, conv tower only), c=f_t (bf16).  conv1 weights
  are transformed host-side (W1s=W1a+W1b, W1d=W1a-W1b) so the tower
  consumes (s, d, c) directly.  5 B/pixel shipped vs 8 B/pixel in v2.
- All tensors ship tight (56x56); padding to the 64x64 conv frames is
  done on device (memzero + interior DMA).  fp8 tower copies of s and c
  are cast on device instead of shipped.
- The big weights (conv fp8 + fc bf16) are sharded 1/8 per core and
  AllGathered on device over NeuronLink: 5.4 MB total over the host
  tunnel instead of 43 MB replicated.
- identity / ones constants ride inside the NEFF (inline_tensor).
- Custom exec path (no donated zero output buffers -- the kernel writes
  every output element) with the compiled executable cached in a module
  global, so repeat calls skip build+compile.
- Conv tower runs fp8 DoubleRow as in v2; depthwise + blend read bf16.
"""

import sys

if "/opt/trn_rl_repo" not in sys.path:
    sys.path.insert(0, "/opt/trn_rl_repo")

import numpy as np
import ml_dtypes
import jax
from jax.sharding import Mesh, PartitionSpec
from jax.experimental.shard_map import shard_map

import concourse.bacc as bacc
import concourse.tile as tile
from concourse import mybir
from concourse import bass2jax

BF16 = ml_dtypes.bfloat16
FP8 = ml_dtypes.float8_e4m3

B, C, H, W = 8, 256, 56, 56
HID = 256
P = 128
FW = 64          # conv frame width (56 + 2*4 pad)
R0 = 4           # first image row in the padded frame
NR = 8           # rows per chunk
NCH = 7          # chunks (56 = 7*8)
WS = 64.0        # fp8 weight scale

QCOLS = 9 * 3 * 2 * 2 * P + 9 * 2 * 2 * P   # 13824 + 4608 = 18432
QSH = QCOLS // 8                             # 2304
W1SZ = 9 * 3 * 2 * 2 * P                     # 13824
BCOLS = 12288
BSH = BCOLS // 8                             # 1536
OFF_GW, OFF_FC1, OFF_FC2, OFF_AWM, OFF_AWP = 0, 512, 2560, 11776, 11778

# single-blob input layout (byte offsets per core); one jax array per call
# minimizes the per-array tunnel dispatch overhead (~85 ms/array)
FRB = 2 * P * H * W            # bytes of one fp8 frame pair = 802816
SOFF = 0                       # s bf16 [2,128,56,56]
COFF = SOFF + 2 * FRB          # c fp8 (f_t; depthwise reads the exact upcast)
DOFF = COFF + FRB              # d fp8
QOFF = DOFF + FRB              # qsh fp8 [128,2304]
BOFF = QOFF + P * QSH          # bsh bf16 [128,1536]
CFOFF = BOFF + P * BSH * 2     # cf f32 [128,28]
ABOFF = CFOFF + P * 28 * 4     # ab f32 [1,1]
NB = ABOFF + 4

LAST_INFO = {}
_CACHE = {}
_POOL = None


def _pool():
    global _POOL
    if _POOL is None:
        from concurrent.futures import ThreadPoolExecutor
        _POOL = ThreadPoolExecutor(max_workers=8)
    return _POOL


def _taps(d):
    return [(ky * 3 + kx, (ky - 1) * d, (kx - 1) * d) for ky in range(3) for kx in range(3)]


def build_nc():
    nc = bacc.Bacc()
    f32 = mybir.dt.float32
    bf16 = mybir.dt.bfloat16
    fp8 = mybir.dt.float8e4
    DR = mybir.MatmulPerfMode.DoubleRow

    Relu = mybir.ActivationFunctionType.Relu
    Sigmoid = mybir.ActivationFunctionType.Sigmoid
    mult = mybir.AluOpType.mult
    add = mybir.AluOpType.add

    # ---- dram I/O: ONE uint8 blob in (regions bitcast on device), bf16 out ----
    xin = nc.dram_tensor("xin", [NB], mybir.dt.uint8, kind="ExternalInput")

    y = nc.dram_tensor("y", [2, P, H, W], bf16, kind="ExternalOutput")

    identc = nc.inline_tensor(np.eye(P, dtype=np.float32).astype(BF16), name="identc")
    onesrc = nc.inline_tensor(np.ones((1, P), np.float32).astype(BF16), name="onesrc")

    with tile.TileContext(nc) as tc:
        with (
            tc.tile_pool(name="mp", bufs=1) as mp,
            tc.tile_pool(name="dp", bufs=1, space="DRAM") as dp,
            tc.tile_pool(name="psb", bufs=4, space="PSUM") as psb,
            tc.tile_pool(name="pss", bufs=2, space="PSUM") as pss,
            tc.tile_pool(name="psa", bufs=2, space="PSUM") as psa,
        ):
            xqs = mp.tile([P, 3, 2, FW, FW], fp8, name="xqs")   # tower input: s,d,c
            xss = mp.tile([P, 2, FW, FW], bf16, name="xss")     # s padded (blend base)
            xcs = mp.tile([P, 2, FW, FW], bf16, name="xcs")     # c padded (depthwise in)
            y1s = mp.tile([P, 2, FW, FW], fp8, name="y1s")
            fms = mp.tile([P, 2, H, FW], bf16, name="fms")
            w1s = mp.tile([P, 3, 9, 2, 2, P], fp8, name="w1s")
            w2s = mp.tile([P, 9, 2, 2, P], fp8, name="w2s")
            gws = mp.tile([P, 2, HID], bf16, name="gws")
            fc1ws = mp.tile([P, 4, 512], bf16, name="fc1ws")
            fc2ws = mp.tile([P, 18, 4, P], bf16, name="fc2ws")
            ident = mp.tile([P, P], bf16, name="ident")
            awm = mp.tile([P, 2], bf16, name="awm")
            awp = mp.tile([P, 2], bf16, name="awp")
            cfs = mp.tile([P, 28], f32, name="cfs")
            onesrs = mp.tile([1, P], bf16, name="onesrs")
            abts = mp.tile([1, 1], f32, name="abts")
            pacc = [mp.tile([P, NCH], f32, name=f"pacc{o}") for o in range(2)]
            gsum = mp.tile([P, 2], f32, name="gsum")
            lsum = mp.tile([P, 2], f32, name="lsum")
            gsumb = mp.tile([P, 2], bf16, name="gsumb")
            fcinb = mp.tile([P, 4], bf16, name="fcinb")
            hb = mp.tile([P, 4], bf16, name="hb")
            wkt = mp.tile([P, 18], f32, name="wkt")
            wks = mp.tile([P, 18], f32, name="wks")
            diag = [mp.tile([P, 9, P], bf16, name=f"diag{o}") for o in range(2)]

            qb = dp.tile([P, QSH], fp8, name="qb")
            qg = dp.tile([8, P, QSH], fp8, name="qg")
            bb = dp.tile([P, BSH], bf16, name="bb")
            bg = dp.tile([8, P, BSH], bf16, name="bg")

            # ---------- weight AllGather (NeuronLink, not host tunnel) ----------
            nc.gpsimd.dma_start(qb[:], xin[QOFF:QOFF + P * QSH].bitcast(fp8))
            nc.gpsimd.collective_compute(
                "AllGather", mybir.AluOpType.bypass,
                replica_groups=[list(range(8))], ins=[qb.opt()], outs=[qg.opt()])
            nc.gpsimd.dma_start(bb[:], xin[BOFF:BOFF + P * BSH * 2].bitcast(bf16))
            nc.gpsimd.collective_compute(
                "AllGather", mybir.AluOpType.bypass,
                replica_groups=[list(range(8))], ins=[bb.opt()], outs=[bg.opt()])

            w1f = w1s.rearrange("p a b c d e -> p (a b c d e)")
            w2f = w2s.rearrange("p a b c d -> p (a b c d)")
            for r in range(6):
                nc.sync.dma_start(out=w1f[:, r * QSH:(r + 1) * QSH], in_=qg[r])
            for r in (6, 7):
                o0 = (r - 6) * QSH
                nc.sync.dma_start(out=w2f[:, o0:o0 + QSH], in_=qg[r])

            # ---------- frame staging ----------
            nc.scalar.memzero(xss)
            nc.scalar.memzero(xqs)
            HFR = FRB // 2
            for o in range(2):
                nc.sync.dma_start(
                    out=xss[:, o, R0:R0 + H, R0:R0 + W],
                    in_=xin[SOFF + o * FRB:SOFF + (o + 1) * FRB].bitcast(bf16))
                nc.sync.dma_start(
                    out=xqs[:, 2, o, R0:R0 + H, R0:R0 + W],
                    in_=xin[COFF + o * HFR:COFF + (o + 1) * HFR].bitcast(fp8))
                nc.sync.dma_start(
                    out=xqs[:, 1, o, R0:R0 + H, R0:R0 + W],
                    in_=xin[DOFF + o * HFR:DOFF + (o + 1) * HFR].bitcast(fp8))
            nc.vector.tensor_copy(xqs[:, 0], xss)    # s -> fp8 (pad ring is zero)
            nc.vector.tensor_copy(xcs, xqs[:, 2])    # c fp8 -> bf16 (exact upcast)
            nc.sync.dma_start(out=cfs, in_=xin[CFOFF:CFOFF + P * 28 * 4].bitcast(f32))
            nc.sync.dma_start(out=abts, in_=xin[ABOFF:ABOFF + 4].bitcast(f32))
            nc.sync.dma_start(out=ident, in_=identc[:, :])
            nc.sync.dma_start(out=onesrs, in_=onesrc[:, :])
            nc.scalar.memzero(y1s)

            b1c = [cfs[:, 0:1], cfs[:, 1:2]]
            b2c = [cfs[:, 2:3], cfs[:, 3:4]]
            gbc = cfs[:, 4:6]
            fc1b = cfs[:, 6:10]
            fc2bT = cfs[:, 10:28]

            taps1 = _taps(2)
            taps2 = _taps(4)
            taps3 = _taps(1)

            # ---------- conv1 ----------
            for c in range(NCH):
                for o in range(2):
                    ps = psb.tile([P, NR, W], f32, name=f"c1_{o}_{c}", tag="psb")
                    psl = ps.rearrange("p a b -> p (a b)")
                    mms = []
                    for kp in range(3):
                        for (t, dy, dx) in taps1:
                            r = R0 + NR * c + dy
                            mms.append((w1s[:, kp, t, o, :, :],
                                        xqs[:, kp, :, r:r + NR, 4 + dx:60 + dx]))
                    for n, (wv, xv) in enumerate(mms):
                        nc.tensor.matmul(psl, wv, xv, start=(n == 0),
                                         stop=(n == len(mms) - 1), perf_mode=DR)
                    nc.scalar.activation(
                        out=y1s[:, o, R0 + NR * c:R0 + NR * c + NR, 4:60],
                        in_=ps, func=Relu,
                        bias=b1c[o], scale=1.0 / WS,
                    )

            # late-use loads: emitted after conv1 so they queue behind the
            # conv-critical transfers on the DMA engines
            regions = [
                (gws.rearrange("p a b -> p (a b)"), OFF_GW, 2 * HID),
                (fc1ws.rearrange("p a b -> p (a b)"), OFF_FC1, 4 * 512),
                (fc2ws.rearrange("p a b c -> p (a b c)"), OFF_FC2, 18 * 4 * P),
                (awm, OFF_AWM, 2),
                (awp, OFF_AWP, 2),
            ]
            for dst, roff, rlen in regions:
                for r in range(8):
                    a = max(roff, r * BSH)
                    b2 = min(roff + rlen, (r + 1) * BSH)
                    if a < b2:
                        nc.sync.dma_start(out=dst[:, a - roff:b2 - roff],
                                          in_=bg[r][:, a - r * BSH:b2 - r * BSH])
            # local pooling of f_t (DVE, idle during conv)
            for o in range(2):
                nc.vector.tensor_reduce(
                    out=lsum[:, o:o + 1], in_=xcs[:, o, 4:60, 4:60],
                    axis=mybir.AxisListType.XY, op=add,
                )

            # ---------- conv2 + pooled accumulation ----------
            for c in range(NCH):
                for o in range(2):
                    ps = psb.tile([P, NR, W], f32, name=f"c2_{o}_{c}", tag="psb")
                    psl = ps.rearrange("p a b -> p (a b)")
                    mms = []
                    for (t, dy, dx) in taps2:
                        r = R0 + NR * c + dy
                        mms.append((w2s[:, t, o, :, :],
                                    y1s[:, :, r:r + NR, 4 + dx:60 + dx]))
                    for n, (wv, xv) in enumerate(mms):
                        nc.tensor.matmul(psl, wv, xv, start=(n == 0),
                                         stop=(n == len(mms) - 1), perf_mode=DR)
                    sc2 = mp.tile([P, NR, 56], bf16, name=f"sc2_{o}_{c}", tag="sc2", bufs=2)
                    nc.scalar.activation(
                        out=sc2, in_=ps, func=Relu,
                        bias=b2c[o], scale=1.0 / WS,
                        accum_out=pacc[o][:, c:c + 1],
                    )

            # ---------- pools -> fc chain ----------
            for o in range(2):
                nc.vector.tensor_reduce(
                    out=gsum[:, o:o + 1], in_=pacc[o],
                    axis=mybir.AxisListType.X, op=add,
                )
            nc.vector.tensor_copy(gsumb, gsum)

            psg = pss.tile([P, 2], f32, name="psg", tag="pss")
            for m in range(2):
                for k in range(2):
                    nc.tensor.matmul(
                        psg[:, m:m + 1], gws[:, k, m * P:(m + 1) * P],
                        gsumb[:, k:k + 1], start=(k == 0), stop=(k == 1),
                    )
            nc.vector.tensor_add(fcinb[:, 0:2], psg, gbc)
            nc.vector.tensor_copy(fcinb[:, 2:4], lsum)

            psh = pss.tile([P, 4], f32, name="psh", tag="pss")
            for m in range(4):
                for k in range(4):
                    nc.tensor.matmul(
                        psh[:, m:m + 1], fc1ws[:, k, m * P:(m + 1) * P],
                        fcinb[:, k:k + 1], start=(k == 0), stop=(k == 3),
                    )
            nc.vector.tensor_add(hb, psh, fc1b)

            psT = pss.tile([P, 18], f32, name="psT", tag="pss")
            for j in range(18):
                for kc in range(4):
                    nc.tensor.matmul(
                        psT[:, j:j + 1], fc2ws[:, j, kc, :],
                        hb[:, kc:kc + 1], start=(kc == 0), stop=(kc == 3),
                    )
            nc.vector.tensor_add(wkt, psT, fc2bT)
            # silu(z) = z * sigmoid(z)
            nc.scalar.activation(out=wks, in_=wkt, func=Sigmoid)
            nc.vector.tensor_mul(wks, wks, wkt)

            for j in range(18):
                nc.vector.tensor_scalar_mul(diag[j // 9][:, j % 9, :], ident,
                                            wks[:, j:j + 1])

            # ---------- depthwise + alpha + fusion (valid cols only) ----------
            for c in range(NCH):
                rows = slice(NR * c, NR * c + NR)
                prow = slice(R0 + NR * c, R0 + NR * c + NR)   # rows in padded tiles
                for o in range(2):
                    ps = psb.tile([P, NR, W], f32, name=f"dw_{o}_{c}", tag="psb")
                    psl = ps.rearrange("p a b -> p (a b)")
                    for (t, dy, dx) in taps3:
                        nc.tensor.matmul(
                            psl, diag[o][:, t, :],
                            xcs[:, o, R0 + NR * c + dy:R0 + NR * c + dy + NR,
                                4 + dx:60 + dx],
                            start=(t == 0), stop=(t == 8),
                        )
                    nc.scalar.copy(fms[:, o, rows, 4:60], ps)

                pa = psa.tile([1, NR, W], f32, name=f"pa{c}", tag="psa")
                pal = pa.rearrange("p a b -> p (a b)")
                for o in range(2):
                    nc.tensor.matmul(
                        pal, awm[:, o:o + 1], fms[:, o, rows, 4:60],
                        start=(o == 0), stop=False,
                    )
                for o in range(2):
                    nc.tensor.matmul(
                        pal, awp[:, o:o + 1], xss[:, o, prow, 4:60],
                        start=False, stop=(o == 1),
                    )
                arow = mp.tile([1, NR, W], bf16, name=f"ar{c}", tag="ar", bufs=2)
                nc.scalar.activation(out=arow, in_=pa, func=Sigmoid, bias=abts[:, 0:1])
                nc.vector.tensor_scalar(arow, arow, 0.4, 0.3, op0=mult, op1=add)
                pb = psa.tile([P, NR, W], f32, name=f"pb{c}", tag="psa")
                nc.tensor.matmul(pb.rearrange("p a b -> p (a b)"), onesrs,
                                 arow.rearrange("p a b -> p (a b)"),
                                 start=True, stop=True)

                for o in range(2):
                    u = mp.tile([P, NR, W], f32, name=f"u{c}{o}", tag="u", bufs=3)
                    nc.vector.scalar_tensor_tensor(
                        u, xss[:, o, prow, 4:60], -1.0, fms[:, o, rows, 4:60],
                        op0=mult, op1=add,
                    )
                    nc.vector.tensor_mul(u, u, pb)
                    nc.vector.tensor_add(xss[:, o, prow, 4:60], xss[:, o, prow, 4:60], u)
                    # stream this chunk's rows out while later chunks compute
                    nc.sync.dma_start(out=y[o, :, rows, :],
                                      in_=xss[:, o, prow, 4:60])

    nc.compile()
    return nc


def _get_exec():
    if "jitted" in _CACHE:
        return _CACHE
    nc = build_nc()
    bass2jax.install_neuronx_cc_hook()
    partition_name = nc.partition_id_tensor.name if nc.partition_id_tensor else None
    in_names, out_names, out_avals = [], [], []
    for alloc in nc.m.functions[0].allocations:
        if not isinstance(alloc, mybir.MemoryLocationSet):
            continue
        name = alloc.memorylocations[0].name
        if alloc.kind == "ExternalInput":
            if name != partition_name:
                in_names.append(name)
        elif alloc.kind == "ExternalOutput":
            out_names.append(name)
            out_avals.append(jax.core.ShapedArray(
                tuple(alloc.tensor_shape), mybir.dt.np(alloc.dtype)))
    n_params = len(in_names)
    param_names = list(in_names)
    if partition_name is not None:
        in_names.append(partition_name)

    def _body(*args):
        operands = list(args)
        if partition_name is not None:
            operands.append(bass2jax.partition_id_tensor())
        return tuple(bass2jax._bass_exec_p.bind(
            *operands, out_avals=tuple(out_avals),
            in_names=tuple(in_names), out_names=tuple(out_names),
            lowering_input_output_aliases=(), sim_require_finite=True,
            sim_require_nnan=True, nc=nc))

    devices = jax.devices()[:8]
    mesh = Mesh(np.asarray(devices), ("core",))
    jitted = jax.jit(shard_map(
        _body, mesh=mesh,
        in_specs=(PartitionSpec("core"),) * n_params,
        out_specs=(PartitionSpec("core"),) * len(out_names), check_rep=False))
    _CACHE.update(dict(jitted=jitted, param_names=param_names,
                       out_names=out_names, out_avals=out_avals))
    return _CACHE


def _prep_shared(w1, b1, w2, b2, gw, gb, fc1_w, fc1_b, fc2_w, fc2_b, aw, ab):
    """Returns per-region [8, nbytes] uint8 arrays for the shared weights."""
    # conv1 weights with the (a,b,c)->(s,d,c) frame transform on axis kp
    w1r = w1.reshape(2, P, 3, 2, P, 3, 3)            # o m kp i k ty tx
    w1t = np.empty_like(w1r)
    w1t[:, :, 0] = w1r[:, :, 0] + w1r[:, :, 1]       # applies to s
    w1t[:, :, 1] = w1r[:, :, 0] - w1r[:, :, 1]       # applies to d
    w1t[:, :, 2] = w1r[:, :, 2]                      # applies to c
    w1q = np.ascontiguousarray(w1t.transpose(4, 2, 5, 6, 0, 3, 1))  # k kp ty tx o i m
    w1q = w1q.reshape(P, W1SZ)
    w2r = w2.reshape(2, P, 2, P, 3, 3)               # o m i k ty tx
    w2q = np.ascontiguousarray(w2r.transpose(3, 4, 5, 0, 2, 1))     # k ty tx o i m
    w2q = w2q.reshape(P, 9 * 2 * 2 * P)
    wq_full = (np.concatenate([w1q, w2q], axis=1) * WS).astype(FP8)  # [P, QCOLS]
    qsh_g = np.ascontiguousarray(
        wq_full.reshape(P, 8, QSH).transpose(1, 0, 2)).view(np.uint8).reshape(8, P * QSH)

    gwt = np.ascontiguousarray((gw[:, :, 0, 0] / 3136.0).T).reshape(2, P, HID)
    gwb = np.ascontiguousarray(gwt.transpose(1, 0, 2)).reshape(P, 2 * HID)
    fc1t = fc1_w.T.copy()
    fc1t[C:, :] /= 3136.0
    fc1wb = np.ascontiguousarray(
        fc1t.reshape(4, P, 512).transpose(1, 0, 2)).reshape(P, 4 * 512)
    f2 = fc2_w.T.reshape(4, P, 2, P, 9)              # kc k bl p t
    fc2wb = np.ascontiguousarray(f2.transpose(1, 2, 4, 0, 3)).reshape(P, 18 * 4 * P)
    awm = np.ascontiguousarray(aw[0, :C, 0, 0].reshape(2, P).T)      # [128, 2]
    awp = np.ascontiguousarray(aw[0, C:, 0, 0].reshape(2, P).T)
    wb_full = np.zeros((P, BCOLS), dtype=np.float32)
    wb_full[:, OFF_GW:OFF_GW + 512] = gwb
    wb_full[:, OFF_FC1:OFF_FC1 + 2048] = fc1wb
    wb_full[:, OFF_FC2:OFF_FC2 + 9216] = fc2wb
    wb_full[:, OFF_AWM:OFF_AWM + 2] = awm
    wb_full[:, OFF_AWP:OFF_AWP + 2] = awp
    wb_full = wb_full.astype(BF16)
    bsh_g = np.ascontiguousarray(
        wb_full.reshape(P, 8, BSH).transpose(1, 0, 2)).view(np.uint8).reshape(8, P * BSH * 2)

    fc1b4 = np.ascontiguousarray(fc1_b.reshape(4, P).T)              # [128, 4]
    b1c = b1.reshape(2, P).T
    b2c = b2.reshape(2, P).T
    gbc = gb.reshape(2, P).T
    fc2bT = np.ascontiguousarray(fc2_b.reshape(2, P, 9).transpose(1, 0, 2)).reshape(P, 18)
    cf1 = np.concatenate([b1c, b2c, gbc, fc1b4, fc2bT], axis=1).astype(np.float32)
    cf_g = np.ascontiguousarray(
        np.broadcast_to(cf1, (8,) + cf1.shape)).view(np.uint8).reshape(8, P * 28 * 4)
    abt_g = np.ascontiguousarray(np.broadcast_to(
        ab.reshape(1).astype(np.float32), (8, 1))).view(np.uint8).reshape(8, 4)
    return qsh_g, bsh_g, cf_g, abt_g


def kernel(f_tm2, f_tm1, f_t, w1, b1, w2, b2, gw, gb,
           fc1_w, fc1_b, fc2_w, fc2_b, aw, ab):
    import time

    args = [np.asarray(a, dtype=np.float32) for a in
            (f_tm2, f_tm1, f_t, w1, b1, w2, b2, gw, gb, fc1_w, fc1_b, fc2_w, fc2_b, aw, ab)]
    f_tm2, f_tm1, f_t = args[0], args[1], args[2]

    t0 = time.time()
    g = _get_exec()
    pool = _pool()
    t1 = time.time()

    blob = np.empty((B, NB), dtype=np.uint8)
    fw = pool.submit(_prep_shared, *args[3:])

    def _fill_row(bi):
        row = blob[bi]
        t = f_tm2[bi] + f_tm1[bi]
        t *= np.float32(0.5)
        row[SOFF:COFF].view(BF16).reshape(C, H, W)[:] = t
        np.subtract(f_tm2[bi], f_tm1[bi], out=t)
        t *= np.float32(0.5)
        row[DOFF:QOFF].view(FP8).reshape(C, H, W)[:] = t
        row[COFF:DOFF].view(FP8).reshape(C, H, W)[:] = f_t[bi]

    futs = [pool.submit(_fill_row, bi) for bi in range(B)]
    qsh_g, bsh_g, cf_g, abt_g = fw.result()
    blob[:, QOFF:BOFF] = qsh_g
    blob[:, BOFF:CFOFF] = bsh_g
    blob[:, CFOFF:ABOFF] = cf_g
    blob[:, ABOFF:NB] = abt_g
    for f in futs:
        f.result()
    blob = blob.reshape(B * NB)
    t2 = time.time()

    out_arrs = g["jitted"](blob)
    yv = np.asarray(out_arrs[0])                      # [8*2, 128, 56, 56] bf16
    t3 = time.time()

    out = np.empty((B, C, H, W), dtype=np.float32)

    def _post_row(bi):
        dst = out[bi].view(np.uint32).reshape(2, P, H, W)
        dst[:] = yv[2 * bi:2 * bi + 2].view(np.uint16)
        dst <<= np.uint32(16)

    for f in [pool.submit(_post_row, bi) for bi in range(B)]:
        f.result()
    t4 = time.time()
    LAST_INFO.update(dict(build_s=t1 - t0, prep_s=t2 - t1, run_s=t3 - t2,
                          post_s=t4 - t3, exec_time_ns=None))
    return out


# revision 18
# speedup vs baseline: 41.2865x; 1.0230x over previous
"""Trainium2 Bass kernel for nn_ContMixT (dense_cnn).

Data-parallel over batch: 8 samples -> 8 NeuronCores.

v3 design notes (tunnel-transfer-bound, so minimize bytes moved):
- Frames reparametrized: s=(f_tm2+f_tm1)/2 (bf16, also the blend base),
  d=(f_tm2-f_tm1)/2 (fp8, conv tower only), c=f_t (bf16).  conv1 weights
  are transformed host-side (W1s=W1a+W1b, W1d=W1a-W1b) so the tower
  consumes (s, d, c) directly.  5 B/pixel shipped vs 8 B/pixel in v2.
- All tensors ship tight (56x56); padding to the 64x64 conv frames is
  done on device (memzero + interior DMA).  fp8 tower copies of s and c
  are cast on device instead of shipped.
- The big weights (conv fp8 + fc bf16) are sharded 1/8 per core and
  AllGathered on device over NeuronLink: 5.4 MB total over the host
  tunnel instead of 43 MB replicated.
- identity / ones constants ride inside the NEFF (inline_tensor).
- Custom exec path (no donated zero output buffers -- the kernel writes
  every output element) with the compiled executable cached in a module
  global, so repeat calls skip build+compile.
- Conv tower runs fp8 DoubleRow as in v2; depthwise + blend read bf16.
"""

import sys

if "/opt/trn_rl_repo" not in sys.path:
    sys.path.insert(0, "/opt/trn_rl_repo")

import numpy as np
import ml_dtypes
import jax
from jax.sharding import Mesh, PartitionSpec
from jax.experimental.shard_map import shard_map

import concourse.bacc as bacc
import concourse.tile as tile
from concourse import mybir
from concourse import bass2jax

BF16 = ml_dtypes.bfloat16
FP8 = ml_dtypes.float8_e4m3

B, C, H, W = 8, 256, 56, 56
HID = 256
P = 128
FW = 64          # conv frame width (56 + 2*4 pad)
R0 = 4           # first image row in the padded frame
NR = 8           # rows per chunk
NCH = 7          # chunks (56 = 7*8)
WS = 64.0        # fp8 weight scale

QCOLS = 9 * 3 * 2 * 2 * P + 9 * 2 * 2 * P   # 13824 + 4608 = 18432
QSH = QCOLS // 8                             # 2304
W1SZ = 9 * 3 * 2 * 2 * P                     # 13824
BCOLS = 12288
BSH = BCOLS // 8                             # 1536
OFF_GW, OFF_FC1, OFF_FC2, OFF_AWM, OFF_AWP = 0, 512, 2560, 11776, 11778

# single-blob input layout (byte offsets per core); one jax array per call
# minimizes the per-array tunnel dispatch overhead (~85 ms/array)
FRB = 2 * P * H * W            # bytes of one fp8 frame pair = 802816
SOFF = 0                       # s bf16 [2,128,56,56]
COFF = SOFF + 2 * FRB          # c fp8 (f_t; depthwise reads the exact upcast)
DOFF = COFF + FRB              # d fp8
QOFF = DOFF + FRB              # qsh fp8 [128,2304]
BOFF = QOFF + P * QSH          # bsh bf16 [128,1536]
CFOFF = BOFF + P * BSH * 2     # cf f32 [128,28]
ABOFF = CFOFF + P * 28 * 4     # ab f32 [1,1]
NB = ABOFF + 4

LAST_INFO = {}
_CACHE = {}
_POOL = None


def _pool():
    global _POOL
    if _POOL is None:
        from concurrent.futures import ThreadPoolExecutor
        _POOL = ThreadPoolExecutor(max_workers=8)
    return _POOL


def _taps(d):
    return [(ky * 3 + kx, (ky - 1) * d, (kx - 1) * d) for ky in range(3) for kx in range(3)]


def build_nc():
    nc = bacc.Bacc()
    f32 = mybir.dt.float32
    bf16 = mybir.dt.bfloat16
    fp8 = mybir.dt.float8e4
    DR = mybir.MatmulPerfMode.DoubleRow

    Relu = mybir.ActivationFunctionType.Relu
    Sigmoid = mybir.ActivationFunctionType.Sigmoid
    mult = mybir.AluOpType.mult
    add = mybir.AluOpType.add

    # ---- dram I/O: ONE uint8 blob in (regions bitcast on device), bf16 out ----
    xin = nc.dram_tensor("xin", [NB], mybir.dt.uint8, kind="ExternalInput")

    y = nc.dram_tensor("y", [2, P, H, W], bf16, kind="ExternalOutput")

    identc = nc.inline_tensor(np.eye(P, dtype=np.float32).astype(BF16), name="identc")
    onesrc = nc.inline_tensor(np.ones((1, P), np.float32).astype(BF16), name="onesrc")

    with tile.TileContext(nc) as tc:
        with (
            tc.tile_pool(name="mp", bufs=1) as mp,
            tc.tile_pool(name="dp", bufs=1, space="DRAM") as dp,
            tc.tile_pool(name="psb", bufs=4, space="PSUM") as psb,
            tc.tile_pool(name="pss", bufs=2, space="PSUM") as pss,
            tc.tile_pool(name="psa", bufs=2, space="PSUM") as psa,
        ):
            xqs = mp.tile([P, 3, 2, FW, FW], fp8, name="xqs")   # tower input: s,d,c
            xss = mp.tile([P, 2, FW, FW], bf16, name="xss")     # s padded (blend base)
            xcs = mp.tile([P, 2, FW, FW], bf16, name="xcs")     # c padded (depthwise in)
            y1s = mp.tile([P, 2, FW, FW], fp8, name="y1s")
            fms = mp.tile([P, 2, H, FW], bf16, name="fms")
            w1s = mp.tile([P, 3, 9, 2, 2, P], fp8, name="w1s")
            w2s = mp.tile([P, 9, 2, 2, P], fp8, name="w2s")
            gws = mp.tile([P, 2, HID], bf16, name="gws")
            fc1ws = mp.tile([P, 4, 512], bf16, name="fc1ws")
            fc2ws = mp.tile([P, 18, 4, P], bf16, name="fc2ws")
            ident = mp.tile([P, P], bf16, name="ident")
            awm = mp.tile([P, 2], bf16, name="awm")
            awp = mp.tile([P, 2], bf16, name="awp")
            cfs = mp.tile([P, 28], f32, name="cfs")
            onesrs = mp.tile([1, P], bf16, name="onesrs")
            abts = mp.tile([1, 1], f32, name="abts")
            pacc = [mp.tile([P, NCH], f32, name=f"pacc{o}") for o in range(2)]
            gsum = mp.tile([P, 2], f32, name="gsum")
            lsum = mp.tile([P, 2], f32, name="lsum")
            gsumb = mp.tile([P, 2], bf16, name="gsumb")
            fcinb = mp.tile([P, 4], bf16, name="fcinb")
            hb = mp.tile([P, 4], bf16, name="hb")
            wkt = mp.tile([P, 18], f32, name="wkt")
            wks = mp.tile([P, 18], f32, name="wks")
            diag = [mp.tile([P, 9, P], bf16, name=f"diag{o}") for o in range(2)]

            qb = dp.tile([P, QSH], fp8, name="qb")
            qg = dp.tile([8, P, QSH], fp8, name="qg")
            bb = dp.tile([P, BSH], bf16, name="bb")
            bg = dp.tile([8, P, BSH], bf16, name="bg")

            # ---------- weight AllGather (NeuronLink, not host tunnel) ----------
            nc.gpsimd.dma_start(qb[:], xin[QOFF:QOFF + P * QSH].bitcast(fp8))
            nc.gpsimd.collective_compute(
                "AllGather", mybir.AluOpType.bypass,
                replica_groups=[list(range(8))], ins=[qb.opt()], outs=[qg.opt()])
            nc.gpsimd.dma_start(bb[:], xin[BOFF:BOFF + P * BSH * 2].bitcast(bf16))
            nc.gpsimd.collective_compute(
                "AllGather", mybir.AluOpType.bypass,
                replica_groups=[list(range(8))], ins=[bb.opt()], outs=[bg.opt()])

            w1f = w1s.rearrange("p a b c d e -> p (a b c d e)")
            w2f = w2s.rearrange("p a b c d -> p (a b c d)")
            for r in range(6):
                nc.sync.dma_start(out=w1f[:, r * QSH:(r + 1) * QSH], in_=qg[r])
            for r in (6, 7):
                o0 = (r - 6) * QSH
                nc.sync.dma_start(out=w2f[:, o0:o0 + QSH], in_=qg[r])

            # ---------- frame staging ----------
            nc.scalar.memzero(xss)
            nc.scalar.memzero(xqs)
            HFR = FRB // 2
            for o in range(2):
                nc.sync.dma_start(
                    out=xss[:, o, R0:R0 + H, R0:R0 + W],
                    in_=xin[SOFF + o * FRB:SOFF + (o + 1) * FRB].bitcast(bf16))
                nc.sync.dma_start(
                    out=xqs[:, 2, o, R0:R0 + H, R0:R0 + W],
                    in_=xin[COFF + o * HFR:COFF + (o + 1) * HFR].bitcast(fp8))
                nc.sync.dma_start(
                    out=xqs[:, 1, o, R0:R0 + H, R0:R0 + W],
                    in_=xin[DOFF + o * HFR:DOFF + (o + 1) * HFR].bitcast(fp8))
            nc.vector.tensor_copy(xqs[:, 0], xss)    # s -> fp8 (pad ring is zero)
            nc.vector.tensor_copy(xcs, xqs[:, 2])    # c fp8 -> bf16 (exact upcast)
            nc.sync.dma_start(out=cfs, in_=xin[CFOFF:CFOFF + P * 28 * 4].bitcast(f32))
            nc.sync.dma_start(out=abts, in_=xin[ABOFF:ABOFF + 4].bitcast(f32))
            nc.sync.dma_start(out=ident, in_=identc[:, :])
            nc.sync.dma_start(out=onesrs, in_=onesrc[:, :])
            nc.scalar.memzero(y1s)

            b1c = [cfs[:, 0:1], cfs[:, 1:2]]
            b2c = [cfs[:, 2:3], cfs[:, 3:4]]
            gbc = cfs[:, 4:6]
            fc1b = cfs[:, 6:10]
            fc2bT = cfs[:, 10:28]

            taps1 = _taps(2)
            taps2 = _taps(4)
            taps3 = _taps(1)

            # ---------- conv1 ----------
            for c in range(NCH):
                for o in range(2):
                    ps = psb.tile([P, NR, W], f32, name=f"c1_{o}_{c}", tag="psb")
                    psl = ps.rearrange("p a b -> p (a b)")
                    mms = []
                    for kp in range(3):
                        for (t, dy, dx) in taps1:
                            r = R0 + NR * c + dy
                            mms.append((w1s[:, kp, t, o, :, :],
                                        xqs[:, kp, :, r:r + NR, 4 + dx:60 + dx]))
                    for n, (wv, xv) in enumerate(mms):
                        nc.tensor.matmul(psl, wv, xv, start=(n == 0),
                                         stop=(n == len(mms) - 1), perf_mode=DR)
                    nc.scalar.activation(
                        out=y1s[:, o, R0 + NR * c:R0 + NR * c + NR, 4:60],
                        in_=ps, func=Relu,
                        bias=b1c[o], scale=1.0 / WS,
                    )

            # late-use loads: emitted after conv1 so they queue behind the
            # conv-critical transfers on the DMA engines
            regions = [
                (gws.rearrange("p a b -> p (a b)"), OFF_GW, 2 * HID),
                (fc1ws.rearrange("p a b -> p (a b)"), OFF_FC1, 4 * 512),
                (fc2ws.rearrange("p a b c -> p (a b c)"), OFF_FC2, 18 * 4 * P),
                (awm, OFF_AWM, 2),
                (awp, OFF_AWP, 2),
            ]
            for dst, roff, rlen in regions:
                for r in range(8):
                    a = max(roff, r * BSH)
                    b2 = min(roff + rlen, (r + 1) * BSH)
                    if a < b2:
                        nc.sync.dma_start(out=dst[:, a - roff:b2 - roff],
                                          in_=bg[r][:, a - r * BSH:b2 - r * BSH])
            # local pooling of f_t (DVE, idle during conv)
            for o in range(2):
                nc.vector.tensor_reduce(
                    out=lsum[:, o:o + 1], in_=xcs[:, o, 4:60, 4:60],
                    axis=mybir.AxisListType.XY, op=add,
                )

            # ---------- conv2 + pooled accumulation ----------
            for c in range(NCH):
                for o in range(2):
                    ps = psb.tile([P, NR, W], f32, name=f"c2_{o}_{c}", tag="psb")
                    psl = ps.rearrange("p a b -> p (a b)")
                    mms = []
                    for (t, dy, dx) in taps2:
                        r = R0 + NR * c + dy
                        mms.append((w2s[:, t, o, :, :],
                                    y1s[:, :, r:r + NR, 4 + dx:60 + dx]))
                    for n, (wv, xv) in enumerate(mms):
                        nc.tensor.matmul(psl, wv, xv, start=(n == 0),
                                         stop=(n == len(mms) - 1), perf_mode=DR)
                    sc2 = mp.tile([P, NR, 56], bf16, name=f"sc2_{o}_{c}", tag="sc2", bufs=2)
                    nc.scalar.activation(
                        out=sc2, in_=ps, func=Relu,
                        bias=b2c[o], scale=1.0 / WS,
                        accum_out=pacc[o][:, c:c + 1],
                    )

            # ---------- pools -> fc chain ----------
            for o in range(2):
                nc.vector.tensor_reduce(
                    out=gsum[:, o:o + 1], in_=pacc[o],
                    axis=mybir.AxisListType.X, op=add,
                )
            nc.vector.tensor_copy(gsumb, gsum)

            psg = pss.tile([P, 2], f32, name="psg", tag="pss")
            for m in range(2):
                for k in range(2):
                    nc.tensor.matmul(
                        psg[:, m:m + 1], gws[:, k, m * P:(m + 1) * P],
                        gsumb[:, k:k + 1], start=(k == 0), stop=(k == 1),
                    )
            nc.vector.tensor_add(fcinb[:, 0:2], psg, gbc)
            nc.vector.tensor_copy(fcinb[:, 2:4], lsum)

            psh = pss.tile([P, 4], f32, name="psh", tag="pss")
            for m in range(4):
                for k in range(4):
                    nc.tensor.matmul(
                        psh[:, m:m + 1], fc1ws[:, k, m * P:(m + 1) * P],
                        fcinb[:, k:k + 1], start=(k == 0), stop=(k == 3),
                    )
            nc.vector.tensor_add(hb, psh, fc1b)

            psT = pss.tile([P, 18], f32, name="psT", tag="pss")
            for j in range(18):
                for kc in range(4):
                    nc.tensor.matmul(
                        psT[:, j:j + 1], fc2ws[:, j, kc, :],
                        hb[:, kc:kc + 1], start=(kc == 0), stop=(kc == 3),
                    )
            nc.vector.tensor_add(wkt, psT, fc2bT)
            # silu(z) = z * sigmoid(z)
            nc.scalar.activation(out=wks, in_=wkt, func=Sigmoid)
            nc.vector.tensor_mul(wks, wks, wkt)

            for j in range(18):
                nc.vector.tensor_scalar_mul(diag[j // 9][:, j % 9, :], ident,
                                            wks[:, j:j + 1])

            # ---------- depthwise + alpha + fusion (valid cols only) ----------
            for c in range(NCH):
                rows = slice(NR * c, NR * c + NR)
                prow = slice(R0 + NR * c, R0 + NR * c + NR)   # rows in padded tiles
                for o in range(2):
                    ps = psb.tile([P, NR, W], f32, name=f"dw_{o}_{c}", tag="psb")
                    psl = ps.rearrange("p a b -> p (a b)")
                    for (t, dy, dx) in taps3:
                        nc.tensor.matmul(
                            psl, diag[o][:, t, :],
                            xcs[:, o, R0 + NR * c + dy:R0 + NR * c + dy + NR,
                                4 + dx:60 + dx],
                            start=(t == 0), stop=(t == 8),
                        )
                    nc.scalar.copy(fms[:, o, rows, 4:60], ps)

                pa = psa.tile([1, NR, W], f32, name=f"pa{c}", tag="psa")
                pal = pa.rearrange("p a b -> p (a b)")
                for o in range(2):
                    nc.tensor.matmul(
                        pal, awm[:, o:o + 1], fms[:, o, rows, 4:60],
                        start=(o == 0), stop=False,
                    )
                for o in range(2):
                    nc.tensor.matmul(
                        pal, awp[:, o:o + 1], xss[:, o, prow, 4:60],
                        start=False, stop=(o == 1),
                    )
                arow = mp.tile([1, NR, W], bf16, name=f"ar{c}", tag="ar", bufs=2)
                nc.scalar.activation(out=arow, in_=pa, func=Sigmoid, bias=abts[:, 0:1])
                nc.vector.tensor_scalar(arow, arow, 0.4, 0.3, op0=mult, op1=add)
                pb = psa.tile([P, NR, W], f32, name=f"pb{c}", tag="psa")
                nc.tensor.matmul(pb.rearrange("p a b -> p (a b)"), onesrs,
                                 arow.rearrange("p a b -> p (a b)"),
                                 start=True, stop=True)

                for o in range(2):
                    u = mp.tile([P, NR, W], f32, name=f"u{c}{o}", tag="u", bufs=3)
                    nc.vector.scalar_tensor_tensor(
                        u, xss[:, o, prow, 4:60], -1.0, fms[:, o, rows, 4:60],
                        op0=mult, op1=add,
                    )
                    nc.vector.tensor_mul(u, u, pb)
                    nc.vector.tensor_add(xss[:, o, prow, 4:60], xss[:, o, prow, 4:60], u)
                    # stream this chunk's rows out while later chunks compute
                    nc.sync.dma_start(out=y[o, :, rows, :],
                                      in_=xss[:, o, prow, 4:60])

    nc.compile()
    return nc


def _get_exec():
    if "jitted" in _CACHE:
        return _CACHE
    nc = build_nc()
    bass2jax.install_neuronx_cc_hook()
    partition_name = nc.partition_id_tensor.name if nc.partition_id_tensor else None
    in_names, out_names, out_avals = [], [], []
    for alloc in nc.m.functions[0].allocations:
        if not isinstance(alloc, mybir.MemoryLocationSet):
            continue
        name = alloc.memorylocations[0].name
        if alloc.kind == "ExternalInput":
            if name != partition_name:
                in_names.append(name)
        elif alloc.kind == "ExternalOutput":
            out_names.append(name)
            out_avals.append(jax.core.ShapedArray(
                tuple(alloc.tensor_shape), mybir.dt.np(alloc.dtype)))
    n_params = len(in_names)
    param_names = list(in_names)
    if partition_name is not None:
        in_names.append(partition_name)

    def _body(*args):
        operands = list(args)
        if partition_name is not None:
            operands.append(bass2jax.partition_id_tensor())
        return tuple(bass2jax._bass_exec_p.bind(
            *operands, out_avals=tuple(out_avals),
            in_names=tuple(in_names), out_names=tuple(out_names),
            lowering_input_output_aliases=(), sim_require_finite=True,
            sim_require_nnan=True, nc=nc))

    devices = jax.devices()[:8]
    mesh = Mesh(np.asarray(devices), ("core",))
    jitted = jax.jit(shard_map(
        _body, mesh=mesh,
        in_specs=(PartitionSpec("core"),) * n_params,
        out_specs=(PartitionSpec("core"),) * len(out_names), check_rep=False))

    import jax.numpy as jnp

    def _frames(a, b, c):
        s = (a + b) * np.float32(0.5)
        d = (a - b) * np.float32(0.5)
        sb = jax.lax.bitcast_convert_type(
            s.astype(jnp.bfloat16), jnp.uint8).reshape(B, 2 * FRB)
        cb = jax.lax.bitcast_convert_type(
            c.astype(jnp.float8_e4m3fn), jnp.uint8).reshape(B, FRB)
        db = jax.lax.bitcast_convert_type(
            d.astype(jnp.float8_e4m3fn), jnp.uint8).reshape(B, FRB)
        return jnp.concatenate([sb, cb, db], axis=1)

    cpu = jax.devices("cpu")[0]
    frames_jit = jax.jit(_frames, device=cpu)
    _CACHE.update(dict(jitted=jitted, param_names=param_names,
                       out_names=out_names, out_avals=out_avals,
                       frames_jit=frames_jit))
    return _CACHE


def _prep_shared(w1, b1, w2, b2, gw, gb, fc1_w, fc1_b, fc2_w, fc2_b, aw, ab):
    """Returns per-region [8, nbytes] uint8 arrays for the shared weights."""
    # conv1 weights with the (a,b,c)->(s,d,c) frame transform on axis kp
    w1r = w1.reshape(2, P, 3, 2, P, 3, 3)            # o m kp i k ty tx
    w1t = np.empty_like(w1r)
    w1t[:, :, 0] = w1r[:, :, 0] + w1r[:, :, 1]       # applies to s
    w1t[:, :, 1] = w1r[:, :, 0] - w1r[:, :, 1]       # applies to d
    w1t[:, :, 2] = w1r[:, :, 2]                      # applies to c
    w1q = np.ascontiguousarray(w1t.transpose(4, 2, 5, 6, 0, 3, 1))  # k kp ty tx o i m
    w1q = w1q.reshape(P, W1SZ)
    w2r = w2.reshape(2, P, 2, P, 3, 3)               # o m i k ty tx
    w2q = np.ascontiguousarray(w2r.transpose(3, 4, 5, 0, 2, 1))     # k ty tx o i m
    w2q = w2q.reshape(P, 9 * 2 * 2 * P)
    wq_full = (np.concatenate([w1q, w2q], axis=1) * WS).astype(FP8)  # [P, QCOLS]
    qsh_g = np.ascontiguousarray(
        wq_full.reshape(P, 8, QSH).transpose(1, 0, 2)).view(np.uint8).reshape(8, P * QSH)

    gwt = np.ascontiguousarray((gw[:, :, 0, 0] / 3136.0).T).reshape(2, P, HID)
    gwb = np.ascontiguousarray(gwt.transpose(1, 0, 2)).reshape(P, 2 * HID)
    fc1t = fc1_w.T.copy()
    fc1t[C:, :] /= 3136.0
    fc1wb = np.ascontiguousarray(
        fc1t.reshape(4, P, 512).transpose(1, 0, 2)).reshape(P, 4 * 512)
    f2 = fc2_w.T.reshape(4, P, 2, P, 9)              # kc k bl p t
    fc2wb = np.ascontiguousarray(f2.transpose(1, 2, 4, 0, 3)).reshape(P, 18 * 4 * P)
    awm = np.ascontiguousarray(aw[0, :C, 0, 0].reshape(2, P).T)      # [128, 2]
    awp = np.ascontiguousarray(aw[0, C:, 0, 0].reshape(2, P).T)
    wb_full = np.zeros((P, BCOLS), dtype=np.float32)
    wb_full[:, OFF_GW:OFF_GW + 512] = gwb
    wb_full[:, OFF_FC1:OFF_FC1 + 2048] = fc1wb
    wb_full[:, OFF_FC2:OFF_FC2 + 9216] = fc2wb
    wb_full[:, OFF_AWM:OFF_AWM + 2] = awm
    wb_full[:, OFF_AWP:OFF_AWP + 2] = awp
    wb_full = wb_full.astype(BF16)
    bsh_g = np.ascontiguousarray(
        wb_full.reshape(P, 8, BSH).transpose(1, 0, 2)).view(np.uint8).reshape(8, P * BSH * 2)

    fc1b4 = np.ascontiguousarray(fc1_b.reshape(4, P).T)              # [128, 4]
    b1c = b1.reshape(2, P).T
    b2c = b2.reshape(2, P).T
    gbc = gb.reshape(2, P).T
    fc2bT = np.ascontiguousarray(fc2_b.reshape(2, P, 9).transpose(1, 0, 2)).reshape(P, 18)
    cf1 = np.concatenate([b1c, b2c, gbc, fc1b4, fc2bT], axis=1).astype(np.float32)
    cf_g = np.ascontiguousarray(
        np.broadcast_to(cf1, (8,) + cf1.shape)).view(np.uint8).reshape(8, P * 28 * 4)
    abt_g = np.ascontiguousarray(np.broadcast_to(
        ab.reshape(1).astype(np.float32), (8, 1))).view(np.uint8).reshape(8, 4)
    return qsh_g, bsh_g, cf_g, abt_g


def kernel(f_tm2, f_tm1, f_t, w1, b1, w2, b2, gw, gb,
           fc1_w, fc1_b, fc2_w, fc2_b, aw, ab):
    import time

    args = [np.asarray(a, dtype=np.float32) for a in
            (f_tm2, f_tm1, f_t, w1, b1, w2, b2, gw, gb, fc1_w, fc1_b, fc2_w, fc2_b, aw, ab)]
    f_tm2, f_tm1, f_t = args[0], args[1], args[2]

    t0 = time.time()
    g = _get_exec()
    pool = _pool()
    t1 = time.time()

    fr = g["frames_jit"](f_tm2, f_tm1, f_t)           # async on XLA CPU pool
    blob = np.empty((B, NB), dtype=np.uint8)
    fw = pool.submit(_prep_shared, *args[3:])
    qsh_g, bsh_g, cf_g, abt_g = fw.result()
    blob[:, QOFF:BOFF] = qsh_g
    blob[:, BOFF:CFOFF] = bsh_g
    blob[:, CFOFF:ABOFF] = cf_g
    blob[:, ABOFF:NB] = abt_g
    blob[:, :QOFF] = np.asarray(fr)
    blob = blob.reshape(B * NB)
    t2 = time.time()

    out_arrs = g["jitted"](blob)
    yv = np.asarray(out_arrs[0])                      # [8*2, 128, 56, 56] bf16
    t3 = time.time()

    out = np.empty((B, C, H, W), dtype=np.float32)

    def _post_row(bi):
        dst = out[bi].view(np.uint32).reshape(2, P, H, W)
        dst[:] = yv[2 * bi:2 * bi + 2].view(np.uint16)
        dst <<= np.uint32(16)

    for f in [pool.submit(_post_row, bi) for bi in range(B)]:
        f.result()
    t4 = time.time()
    LAST_INFO.update(dict(build_s=t1 - t0, prep_s=t2 - t1, run_s=t3 - t2,
                          post_s=t4 - t3, exec_time_ns=None))
    return out


# revision 20
# speedup vs baseline: 42.8209x; 1.0372x over previous
"""Trainium2 Bass kernel for nn_ContMixT (dense_cnn).

Data-parallel over batch: 8 samples -> 8 NeuronCores.

v3 design notes (tunnel-transfer-bound, so minimize bytes moved):
- Frames reparametrized: s=(f_tm2+f_tm1)/2 (bf16, also the blend base),
  d=(f_tm2-f_tm1)/2 (fp8, conv tower only), c=f_t (bf16).  conv1 weights
  are transformed host-side (W1s=W1a+W1b, W1d=W1a-W1b) so the tower
  consumes (s, d, c) directly.  5 B/pixel shipped vs 8 B/pixel in v2.
- All tensors ship tight (56x56); padding to the 64x64 conv frames is
  done on device (memzero + interior DMA).  fp8 tower copies of s and c
  are cast on device instead of shipped.
- The big weights (conv fp8 + fc bf16) are sharded 1/8 per core and
  AllGathered on device over NeuronLink: 5.4 MB total over the host
  tunnel instead of 43 MB replicated.
- identity / ones constants ride inside the NEFF (inline_tensor).
- Custom exec path (no donated zero output buffers -- the kernel writes
  every output element) with the compiled executable cached in a module
  global, so repeat calls skip build+compile.
- Conv tower runs fp8 DoubleRow as in v2; depthwise + blend read bf16.
"""

import sys

if "/opt/trn_rl_repo" not in sys.path:
    sys.path.insert(0, "/opt/trn_rl_repo")

import numpy as np
import ml_dtypes
import jax
from jax.sharding import Mesh, PartitionSpec
from jax.experimental.shard_map import shard_map

import concourse.bacc as bacc
import concourse.tile as tile
from concourse import mybir
from concourse import bass2jax

BF16 = ml_dtypes.bfloat16
FP8 = ml_dtypes.float8_e4m3

B, C, H, W = 8, 256, 56, 56
HID = 256
P = 128
FW = 64          # conv frame width (56 + 2*4 pad)
R0 = 4           # first image row in the padded frame
NR = 8           # rows per chunk
NCH = 7          # chunks (56 = 7*8)
WS = 64.0        # fp8 weight scale

QCOLS = 9 * 3 * 2 * 2 * P + 9 * 2 * 2 * P   # 13824 + 4608 = 18432
QSH = QCOLS // 8                             # 2304
W1SZ = 9 * 3 * 2 * 2 * P                     # 13824
BCOLS = 12288
BSH = BCOLS // 8                             # 1536
OFF_GW, OFF_FC1, OFF_FC2, OFF_AWM, OFF_AWP = 0, 512, 2560, 11776, 11778

# single-blob input layout (byte offsets per core); one jax array per call
# minimizes the per-array tunnel dispatch overhead (~85 ms/array)
FRB = 2 * P * H * W            # bytes of one fp8 frame pair = 802816
SOFF = 0                       # s bf16 [2,128,56,56]
COFF = SOFF + 2 * FRB          # c fp8 (f_t; depthwise reads the exact upcast)
DOFF = COFF + FRB              # d fp8
QOFF = DOFF + FRB              # qsh fp8 [128,2304]
BOFF = QOFF + P * QSH          # bsh bf16 [128,1536]
CFOFF = BOFF + P * BSH * 2     # cf f32 [128,28]
ABOFF = CFOFF + P * 28 * 4     # ab f32 [1,1]
NB = ABOFF + 4

LAST_INFO = {}
_CACHE = {}
_POOL = None


def _pool():
    global _POOL
    if _POOL is None:
        from concurrent.futures import ThreadPoolExecutor
        _POOL = ThreadPoolExecutor(max_workers=8)
    return _POOL


def _taps(d):
    return [(ky * 3 + kx, (ky - 1) * d, (kx - 1) * d) for ky in range(3) for kx in range(3)]


def build_nc():
    nc = bacc.Bacc()
    f32 = mybir.dt.float32
    bf16 = mybir.dt.bfloat16
    fp8 = mybir.dt.float8e4
    DR = mybir.MatmulPerfMode.DoubleRow

    Relu = mybir.ActivationFunctionType.Relu
    Sigmoid = mybir.ActivationFunctionType.Sigmoid
    mult = mybir.AluOpType.mult
    add = mybir.AluOpType.add

    # ---- dram I/O: ONE uint8 blob in (regions bitcast on device), bf16 out ----
    xin = nc.dram_tensor("xin", [NB], mybir.dt.uint8, kind="ExternalInput")

    y = nc.dram_tensor("y", [2, P, H, W], bf16, kind="ExternalOutput")

    identc = nc.inline_tensor(np.eye(P, dtype=np.float32).astype(BF16), name="identc")
    onesrc = nc.inline_tensor(np.ones((1, P), np.float32).astype(BF16), name="onesrc")

    with tile.TileContext(nc) as tc:
        with (
            tc.tile_pool(name="mp", bufs=1) as mp,
            tc.tile_pool(name="dp", bufs=1, space="DRAM") as dp,
            tc.tile_pool(name="psb", bufs=4, space="PSUM") as psb,
            tc.tile_pool(name="pss", bufs=2, space="PSUM") as pss,
            tc.tile_pool(name="psa", bufs=2, space="PSUM") as psa,
        ):
            xqs = mp.tile([P, 3, 2, FW, FW], fp8, name="xqs")   # tower input: s,d,c
            xss = mp.tile([P, 2, FW, FW], bf16, name="xss")     # s padded (blend base)
            xcs = mp.tile([P, 2, FW, FW], bf16, name="xcs")     # c padded (depthwise in)
            y1s = mp.tile([P, 2, FW, FW], fp8, name="y1s")
            fms = mp.tile([P, 2, H, FW], bf16, name="fms")
            w1s = mp.tile([P, 3, 9, 2, 2, P], fp8, name="w1s")
            w2s = mp.tile([P, 9, 2, 2, P], fp8, name="w2s")
            gws = mp.tile([P, 2, HID], bf16, name="gws")
            fc1ws = mp.tile([P, 4, 512], bf16, name="fc1ws")
            fc2ws = mp.tile([P, 18, 4, P], bf16, name="fc2ws")
            ident = mp.tile([P, P], bf16, name="ident")
            awm = mp.tile([P, 2], bf16, name="awm")
            awp = mp.tile([P, 2], bf16, name="awp")
            cfs = mp.tile([P, 28], f32, name="cfs")
            onesrs = mp.tile([1, P], bf16, name="onesrs")
            abts = mp.tile([1, 1], f32, name="abts")
            pacc = [mp.tile([P, NCH], f32, name=f"pacc{o}") for o in range(2)]
            gsum = mp.tile([P, 2], f32, name="gsum")
            lsum = mp.tile([P, 2], f32, name="lsum")
            gsumb = mp.tile([P, 2], bf16, name="gsumb")
            fcinb = mp.tile([P, 4], bf16, name="fcinb")
            hb = mp.tile([P, 4], bf16, name="hb")
            wkt = mp.tile([P, 18], f32, name="wkt")
            wks = mp.tile([P, 18], f32, name="wks")
            diag = [mp.tile([P, 9, P], bf16, name=f"diag{o}") for o in range(2)]

            qb = dp.tile([P, QSH], fp8, name="qb")
            qg = dp.tile([8, P, QSH], fp8, name="qg")
            bb = dp.tile([P, BSH], bf16, name="bb")
            bg = dp.tile([8, P, BSH], bf16, name="bg")

            # ---------- weight AllGather (NeuronLink, not host tunnel) ----------
            nc.gpsimd.dma_start(qb[:], xin[QOFF:QOFF + P * QSH].bitcast(fp8))
            nc.gpsimd.collective_compute(
                "AllGather", mybir.AluOpType.bypass,
                replica_groups=[list(range(8))], ins=[qb.opt()], outs=[qg.opt()])
            nc.gpsimd.dma_start(bb[:], xin[BOFF:BOFF + P * BSH * 2].bitcast(bf16))
            nc.gpsimd.collective_compute(
                "AllGather", mybir.AluOpType.bypass,
                replica_groups=[list(range(8))], ins=[bb.opt()], outs=[bg.opt()])

            w1f = w1s.rearrange("p a b c d e -> p (a b c d e)")
            w2f = w2s.rearrange("p a b c d -> p (a b c d)")
            for r in range(6):
                nc.sync.dma_start(out=w1f[:, r * QSH:(r + 1) * QSH], in_=qg[r])
            for r in (6, 7):
                o0 = (r - 6) * QSH
                nc.sync.dma_start(out=w2f[:, o0:o0 + QSH], in_=qg[r])

            # ---------- frame staging ----------
            nc.scalar.memzero(xss)
            nc.scalar.memzero(xqs)
            HFR = FRB // 2
            for o in range(2):
                nc.sync.dma_start(
                    out=xss[:, o, R0:R0 + H, R0:R0 + W],
                    in_=xin[SOFF + o * FRB:SOFF + (o + 1) * FRB].bitcast(bf16))
                nc.sync.dma_start(
                    out=xqs[:, 2, o, R0:R0 + H, R0:R0 + W],
                    in_=xin[COFF + o * HFR:COFF + (o + 1) * HFR].bitcast(fp8))
                nc.sync.dma_start(
                    out=xqs[:, 1, o, R0:R0 + H, R0:R0 + W],
                    in_=xin[DOFF + o * HFR:DOFF + (o + 1) * HFR].bitcast(fp8))
            nc.vector.tensor_copy(xqs[:, 0], xss)    # s -> fp8 (pad ring is zero)
            nc.vector.tensor_copy(xcs, xqs[:, 2])    # c fp8 -> bf16 (exact upcast)
            nc.sync.dma_start(out=cfs, in_=xin[CFOFF:CFOFF + P * 28 * 4].bitcast(f32))
            nc.sync.dma_start(out=abts, in_=xin[ABOFF:ABOFF + 4].bitcast(f32))
            nc.sync.dma_start(out=ident, in_=identc[:, :])
            nc.sync.dma_start(out=onesrs, in_=onesrc[:, :])
            nc.scalar.memzero(y1s)

            b1c = [cfs[:, 0:1], cfs[:, 1:2]]
            b2c = [cfs[:, 2:3], cfs[:, 3:4]]
            gbc = cfs[:, 4:6]
            fc1b = cfs[:, 6:10]
            fc2bT = cfs[:, 10:28]

            taps1 = _taps(2)
            taps2 = _taps(4)
            taps3 = _taps(1)

            # ---------- conv1 ----------
            for c in range(NCH):
                for o in range(2):
                    ps = psb.tile([P, NR, W], f32, name=f"c1_{o}_{c}", tag="psb")
                    psl = ps.rearrange("p a b -> p (a b)")
                    mms = []
                    for kp in range(3):
                        for (t, dy, dx) in taps1:
                            r = R0 + NR * c + dy
                            mms.append((w1s[:, kp, t, o, :, :],
                                        xqs[:, kp, :, r:r + NR, 4 + dx:60 + dx]))
                    for n, (wv, xv) in enumerate(mms):
                        nc.tensor.matmul(psl, wv, xv, start=(n == 0),
                                         stop=(n == len(mms) - 1), perf_mode=DR)
                    nc.scalar.activation(
                        out=y1s[:, o, R0 + NR * c:R0 + NR * c + NR, 4:60],
                        in_=ps, func=Relu,
                        bias=b1c[o], scale=1.0 / WS,
                    )

            # late-use loads: emitted after conv1 so they queue behind the
            # conv-critical transfers on the DMA engines
            regions = [
                (gws.rearrange("p a b -> p (a b)"), OFF_GW, 2 * HID),
                (fc1ws.rearrange("p a b -> p (a b)"), OFF_FC1, 4 * 512),
                (fc2ws.rearrange("p a b c -> p (a b c)"), OFF_FC2, 18 * 4 * P),
                (awm, OFF_AWM, 2),
                (awp, OFF_AWP, 2),
            ]
            for dst, roff, rlen in regions:
                for r in range(8):
                    a = max(roff, r * BSH)
                    b2 = min(roff + rlen, (r + 1) * BSH)
                    if a < b2:
                        nc.sync.dma_start(out=dst[:, a - roff:b2 - roff],
                                          in_=bg[r][:, a - r * BSH:b2 - r * BSH])
            # local pooling of f_t (DVE, idle during conv)
            for o in range(2):
                nc.vector.tensor_reduce(
                    out=lsum[:, o:o + 1], in_=xcs[:, o, 4:60, 4:60],
                    axis=mybir.AxisListType.XY, op=add,
                )

            # ---------- conv2 + pooled accumulation ----------
            for c in range(NCH):
                for o in range(2):
                    ps = psb.tile([P, NR, W], f32, name=f"c2_{o}_{c}", tag="psb")
                    psl = ps.rearrange("p a b -> p (a b)")
                    mms = []
                    for (t, dy, dx) in taps2:
                        r = R0 + NR * c + dy
                        mms.append((w2s[:, t, o, :, :],
                                    y1s[:, :, r:r + NR, 4 + dx:60 + dx]))
                    for n, (wv, xv) in enumerate(mms):
                        nc.tensor.matmul(psl, wv, xv, start=(n == 0),
                                         stop=(n == len(mms) - 1), perf_mode=DR)
                    sc2 = mp.tile([P, NR, 56], bf16, name=f"sc2_{o}_{c}", tag="sc2", bufs=2)
                    nc.scalar.activation(
                        out=sc2, in_=ps, func=Relu,
                        bias=b2c[o], scale=1.0 / WS,
                        accum_out=pacc[o][:, c:c + 1],
                    )

            # ---------- pools -> fc chain ----------
            for o in range(2):
                nc.vector.tensor_reduce(
                    out=gsum[:, o:o + 1], in_=pacc[o],
                    axis=mybir.AxisListType.X, op=add,
                )
            nc.vector.tensor_copy(gsumb, gsum)

            psg = pss.tile([P, 2], f32, name="psg", tag="pss")
            for m in range(2):
                for k in range(2):
                    nc.tensor.matmul(
                        psg[:, m:m + 1], gws[:, k, m * P:(m + 1) * P],
                        gsumb[:, k:k + 1], start=(k == 0), stop=(k == 1),
                    )
            nc.vector.tensor_add(fcinb[:, 0:2], psg, gbc)
            nc.vector.tensor_copy(fcinb[:, 2:4], lsum)

            psh = pss.tile([P, 4], f32, name="psh", tag="pss")
            for m in range(4):
                for k in range(4):
                    nc.tensor.matmul(
                        psh[:, m:m + 1], fc1ws[:, k, m * P:(m + 1) * P],
                        fcinb[:, k:k + 1], start=(k == 0), stop=(k == 3),
                    )
            nc.vector.tensor_add(hb, psh, fc1b)

            psT = pss.tile([P, 18], f32, name="psT", tag="pss")
            for j in range(18):
                for kc in range(4):
                    nc.tensor.matmul(
                        psT[:, j:j + 1], fc2ws[:, j, kc, :],
                        hb[:, kc:kc + 1], start=(kc == 0), stop=(kc == 3),
                    )
            nc.vector.tensor_add(wkt, psT, fc2bT)
            # silu(z) = z * sigmoid(z)
            nc.scalar.activation(out=wks, in_=wkt, func=Sigmoid)
            nc.vector.tensor_mul(wks, wks, wkt)

            for j in range(18):
                nc.vector.tensor_scalar_mul(diag[j // 9][:, j % 9, :], ident,
                                            wks[:, j:j + 1])

            # ---------- depthwise + alpha + fusion (valid cols only) ----------
            for c in range(NCH):
                rows = slice(NR * c, NR * c + NR)
                prow = slice(R0 + NR * c, R0 + NR * c + NR)   # rows in padded tiles
                for o in range(2):
                    ps = psb.tile([P, NR, W], f32, name=f"dw_{o}_{c}", tag="psb")
                    psl = ps.rearrange("p a b -> p (a b)")
                    for (t, dy, dx) in taps3:
                        nc.tensor.matmul(
                            psl, diag[o][:, t, :],
                            xcs[:, o, R0 + NR * c + dy:R0 + NR * c + dy + NR,
                                4 + dx:60 + dx],
                            start=(t == 0), stop=(t == 8),
                        )
                    nc.scalar.copy(fms[:, o, rows, 4:60], ps)

                pa = psa.tile([1, NR, W], f32, name=f"pa{c}", tag="psa")
                pal = pa.rearrange("p a b -> p (a b)")
                for o in range(2):
                    nc.tensor.matmul(
                        pal, awm[:, o:o + 1], fms[:, o, rows, 4:60],
                        start=(o == 0), stop=False,
                    )
                for o in range(2):
                    nc.tensor.matmul(
                        pal, awp[:, o:o + 1], xss[:, o, prow, 4:60],
                        start=False, stop=(o == 1),
                    )
                arow = mp.tile([1, NR, W], bf16, name=f"ar{c}", tag="ar", bufs=2)
                nc.scalar.activation(out=arow, in_=pa, func=Sigmoid, bias=abts[:, 0:1])
                nc.vector.tensor_scalar(arow, arow, 0.4, 0.3, op0=mult, op1=add)
                pb = psa.tile([P, NR, W], f32, name=f"pb{c}", tag="psa")
                nc.tensor.matmul(pb.rearrange("p a b -> p (a b)"), onesrs,
                                 arow.rearrange("p a b -> p (a b)"),
                                 start=True, stop=True)

                for o in range(2):
                    u = mp.tile([P, NR, W], f32, name=f"u{c}{o}", tag="u", bufs=3)
                    nc.vector.scalar_tensor_tensor(
                        u, xss[:, o, prow, 4:60], -1.0, fms[:, o, rows, 4:60],
                        op0=mult, op1=add,
                    )
                    nc.vector.tensor_mul(u, u, pb)
                    nc.vector.tensor_add(xss[:, o, prow, 4:60], xss[:, o, prow, 4:60], u)
                    # stream this chunk's rows out while later chunks compute
                    nc.sync.dma_start(out=y[o, :, rows, :],
                                      in_=xss[:, o, prow, 4:60])

    nc.compile()
    return nc


def _get_exec():
    if "jitted" in _CACHE:
        return _CACHE
    nc = build_nc()
    bass2jax.install_neuronx_cc_hook()
    partition_name = nc.partition_id_tensor.name if nc.partition_id_tensor else None
    in_names, out_names, out_avals = [], [], []
    for alloc in nc.m.functions[0].allocations:
        if not isinstance(alloc, mybir.MemoryLocationSet):
            continue
        name = alloc.memorylocations[0].name
        if alloc.kind == "ExternalInput":
            if name != partition_name:
                in_names.append(name)
        elif alloc.kind == "ExternalOutput":
            out_names.append(name)
            out_avals.append(jax.core.ShapedArray(
                tuple(alloc.tensor_shape), mybir.dt.np(alloc.dtype)))
    n_params = len(in_names)
    param_names = list(in_names)
    if partition_name is not None:
        in_names.append(partition_name)

    def _body(*args):
        operands = list(args)
        if partition_name is not None:
            operands.append(bass2jax.partition_id_tensor())
        return tuple(bass2jax._bass_exec_p.bind(
            *operands, out_avals=tuple(out_avals),
            in_names=tuple(in_names), out_names=tuple(out_names),
            lowering_input_output_aliases=(), sim_require_finite=True,
            sim_require_nnan=True, nc=nc))

    devices = jax.devices()[:8]
    mesh = Mesh(np.asarray(devices), ("core",))
    jitted = jax.jit(shard_map(
        _body, mesh=mesh,
        in_specs=(PartitionSpec("core"),) * n_params,
        out_specs=(PartitionSpec("core"),) * len(out_names), check_rep=False))

    import jax.numpy as jnp

    def _bc(v):
        return jax.lax.bitcast_convert_type(v, jnp.uint8)

    def _prep_all(a, b, c, w1, b1, w2, b2, gw, gb, fc1_w, fc1_b, fc2_w, fc2_b, aw, ab):
        s = (a + b) * np.float32(0.5)
        d = (a - b) * np.float32(0.5)
        sb = _bc(s.astype(jnp.bfloat16)).reshape(B, 2 * FRB)
        cb = _bc(c.astype(jnp.float8_e4m3fn)).reshape(B, FRB)
        db = _bc(d.astype(jnp.float8_e4m3fn)).reshape(B, FRB)

        w1r = w1.reshape(2, P, 3, 2, P, 3, 3)        # o m kp i k ty tx
        w1t = jnp.stack([w1r[:, :, 0] + w1r[:, :, 1],
                         w1r[:, :, 0] - w1r[:, :, 1],
                         w1r[:, :, 2]], axis=2)
        w1q = w1t.transpose(4, 2, 5, 6, 0, 3, 1).reshape(P, W1SZ)   # k kp ty tx o i m
        w2q = w2.reshape(2, P, 2, P, 3, 3).transpose(3, 4, 5, 0, 2, 1).reshape(P, 9 * 2 * 2 * P)
        wq_full = (jnp.concatenate([w1q, w2q], axis=1) * WS).astype(jnp.float8_e4m3fn)
        qsh_g = _bc(wq_full.reshape(P, 8, QSH).transpose(1, 0, 2)).reshape(8, P * QSH)

        gwb = (gw[:, :, 0, 0] / 3136.0).T.reshape(2, P, HID).transpose(1, 0, 2).reshape(P, 2 * HID)
        fc1t = jnp.concatenate([fc1_w.T[:C], fc1_w.T[C:] / 3136.0], axis=0)
        fc1wb = fc1t.reshape(4, P, 512).transpose(1, 0, 2).reshape(P, 4 * 512)
        fc2wb = fc2_w.T.reshape(4, P, 2, P, 9).transpose(1, 2, 4, 0, 3).reshape(P, 18 * 4 * P)
        awm = aw[0, :C, 0, 0].reshape(2, P).T
        awp = aw[0, C:, 0, 0].reshape(2, P).T
        wb_full = jnp.concatenate(
            [gwb, fc1wb, fc2wb, awm, awp,
             jnp.zeros((P, BCOLS - (OFF_AWP + 2)), jnp.float32)], axis=1)
        bsh_g = _bc(wb_full.astype(jnp.bfloat16).reshape(P, 8, BSH).transpose(1, 0, 2)
                    ).reshape(8, P * BSH * 2)

        b1c = b1.reshape(2, P).T
        b2c = b2.reshape(2, P).T
        gbc = gb.reshape(2, P).T
        fc1b4 = fc1_b.reshape(4, P).T
        fc2bT = fc2_b.reshape(2, P, 9).transpose(1, 0, 2).reshape(P, 18)
        cf1 = jnp.concatenate([b1c, b2c, gbc, fc1b4, fc2bT], axis=1).astype(jnp.float32)
        cfb = _bc(jnp.broadcast_to(cf1, (8, P, 28))).reshape(8, P * 28 * 4)
        abb = _bc(jnp.broadcast_to(ab.reshape(1, 1).astype(jnp.float32), (8, 1))).reshape(8, 4)
        return jnp.concatenate([sb, cb, db, qsh_g, bsh_g, cfb, abb], axis=1).reshape(B * NB)

    cpu = jax.devices("cpu")[0]
    prep_jit = jax.jit(_prep_all, device=cpu)
    _CACHE.update(dict(jitted=jitted, param_names=param_names,
                       out_names=out_names, out_avals=out_avals,
                       prep_jit=prep_jit))
    return _CACHE


def _prep_shared(w1, b1, w2, b2, gw, gb, fc1_w, fc1_b, fc2_w, fc2_b, aw, ab):
    """Returns per-region [8, nbytes] uint8 arrays for the shared weights."""
    # conv1 weights with the (a,b,c)->(s,d,c) frame transform on axis kp
    w1r = w1.reshape(2, P, 3, 2, P, 3, 3)            # o m kp i k ty tx
    w1t = np.empty_like(w1r)
    w1t[:, :, 0] = w1r[:, :, 0] + w1r[:, :, 1]       # applies to s
    w1t[:, :, 1] = w1r[:, :, 0] - w1r[:, :, 1]       # applies to d
    w1t[:, :, 2] = w1r[:, :, 2]                      # applies to c
    w1q = np.ascontiguousarray(w1t.transpose(4, 2, 5, 6, 0, 3, 1))  # k kp ty tx o i m
    w1q = w1q.reshape(P, W1SZ)
    w2r = w2.reshape(2, P, 2, P, 3, 3)               # o m i k ty tx
    w2q = np.ascontiguousarray(w2r.transpose(3, 4, 5, 0, 2, 1))     # k ty tx o i m
    w2q = w2q.reshape(P, 9 * 2 * 2 * P)
    wq_full = (np.concatenate([w1q, w2q], axis=1) * WS).astype(FP8)  # [P, QCOLS]
    qsh_g = np.ascontiguousarray(
        wq_full.reshape(P, 8, QSH).transpose(1, 0, 2)).view(np.uint8).reshape(8, P * QSH)

    gwt = np.ascontiguousarray((gw[:, :, 0, 0] / 3136.0).T).reshape(2, P, HID)
    gwb = np.ascontiguousarray(gwt.transpose(1, 0, 2)).reshape(P, 2 * HID)
    fc1t = fc1_w.T.copy()
    fc1t[C:, :] /= 3136.0
    fc1wb = np.ascontiguousarray(
        fc1t.reshape(4, P, 512).transpose(1, 0, 2)).reshape(P, 4 * 512)
    f2 = fc2_w.T.reshape(4, P, 2, P, 9)              # kc k bl p t
    fc2wb = np.ascontiguousarray(f2.transpose(1, 2, 4, 0, 3)).reshape(P, 18 * 4 * P)
    awm = np.ascontiguousarray(aw[0, :C, 0, 0].reshape(2, P).T)      # [128, 2]
    awp = np.ascontiguousarray(aw[0, C:, 0, 0].reshape(2, P).T)
    wb_full = np.zeros((P, BCOLS), dtype=np.float32)
    wb_full[:, OFF_GW:OFF_GW + 512] = gwb
    wb_full[:, OFF_FC1:OFF_FC1 + 2048] = fc1wb
    wb_full[:, OFF_FC2:OFF_FC2 + 9216] = fc2wb
    wb_full[:, OFF_AWM:OFF_AWM + 2] = awm
    wb_full[:, OFF_AWP:OFF_AWP + 2] = awp
    wb_full = wb_full.astype(BF16)
    bsh_g = np.ascontiguousarray(
        wb_full.reshape(P, 8, BSH).transpose(1, 0, 2)).view(np.uint8).reshape(8, P * BSH * 2)

    fc1b4 = np.ascontiguousarray(fc1_b.reshape(4, P).T)              # [128, 4]
    b1c = b1.reshape(2, P).T
    b2c = b2.reshape(2, P).T
    gbc = gb.reshape(2, P).T
    fc2bT = np.ascontiguousarray(fc2_b.reshape(2, P, 9).transpose(1, 0, 2)).reshape(P, 18)
    cf1 = np.concatenate([b1c, b2c, gbc, fc1b4, fc2bT], axis=1).astype(np.float32)
    cf_g = np.ascontiguousarray(
        np.broadcast_to(cf1, (8,) + cf1.shape)).view(np.uint8).reshape(8, P * 28 * 4)
    abt_g = np.ascontiguousarray(np.broadcast_to(
        ab.reshape(1).astype(np.float32), (8, 1))).view(np.uint8).reshape(8, 4)
    return qsh_g, bsh_g, cf_g, abt_g


def kernel(f_tm2, f_tm1, f_t, w1, b1, w2, b2, gw, gb,
           fc1_w, fc1_b, fc2_w, fc2_b, aw, ab):
    import time

    args = [np.asarray(a, dtype=np.float32) for a in
            (f_tm2, f_tm1, f_t, w1, b1, w2, b2, gw, gb, fc1_w, fc1_b, fc2_w, fc2_b, aw, ab)]
    f_tm2, f_tm1, f_t = args[0], args[1], args[2]

    t0 = time.time()
    g = _get_exec()
    pool = _pool()
    t1 = time.time()

    blob = np.asarray(g["prep_jit"](*args))
    t2 = time.time()

    out_arrs = g["jitted"](blob)
    yv = np.asarray(out_arrs[0])                      # [8*2, 128, 56, 56] bf16
    t3 = time.time()

    out = np.empty((B, C, H, W), dtype=np.float32)

    def _post_row(bi):
        dst = out[bi].view(np.uint32).reshape(2, P, H, W)
        dst[:] = yv[2 * bi:2 * bi + 2].view(np.uint16)
        dst <<= np.uint32(16)

    for f in [pool.submit(_post_row, bi) for bi in range(B)]:
        f.result()
    t4 = time.time()
    LAST_INFO.update(dict(build_s=t1 - t0, prep_s=t2 - t1, run_s=t3 - t2,
                          post_s=t4 - t3, exec_time_ns=None))
    return out


# revision 31
# speedup vs baseline: 60.5671x; 1.4144x over previous
"""Trainium2 Bass kernel for nn_ContMixT (dense_cnn).

Data-parallel over batch: 8 samples -> 8 NeuronCores.

v3 design notes (tunnel-transfer-bound, so minimize bytes moved):
- Frames reparametrized: s=(f_tm2+f_tm1)/2 (bf16, also the blend base),
  d=(f_tm2-f_tm1)/2 (fp8, conv tower only), c=f_t (bf16).  conv1 weights
  are transformed host-side (W1s=W1a+W1b, W1d=W1a-W1b) so the tower
  consumes (s, d, c) directly.  5 B/pixel shipped vs 8 B/pixel in v2.
- All tensors ship tight (56x56); padding to the 64x64 conv frames is
  done on device (memzero + interior DMA).  fp8 tower copies of s and c
  are cast on device instead of shipped.
- The big weights (conv fp8 + fc bf16) are sharded 1/8 per core and
  AllGathered on device over NeuronLink: 5.4 MB total over the host
  tunnel instead of 43 MB replicated.
- identity / ones constants ride inside the NEFF (inline_tensor).
- Custom exec path (no donated zero output buffers -- the kernel writes
  every output element) with the compiled executable cached in a module
  global, so repeat calls skip build+compile.
- Conv tower runs fp8 DoubleRow as in v2; depthwise + blend read bf16.
"""

import sys

if "/opt/trn_rl_repo" not in sys.path:
    sys.path.insert(0, "/opt/trn_rl_repo")

import numpy as np
import ml_dtypes
import jax
from jax.sharding import Mesh, PartitionSpec
from jax.experimental.shard_map import shard_map

import concourse.bacc as bacc
import concourse.tile as tile
from concourse import mybir
from concourse import bass2jax

BF16 = ml_dtypes.bfloat16
FP8 = ml_dtypes.float8_e4m3

B, C, H, W = 8, 256, 56, 56
HID = 256
P = 128
FW = 64          # conv frame width (56 + 2*4 pad)
R0 = 4           # first image row in the padded frame
NR = 8           # rows per chunk
NCH = 7          # chunks (56 = 7*8)
WS = 64.0        # fp8 weight scale

QCOLS = 9 * 3 * 2 * 2 * P + 9 * 2 * 2 * P   # 13824 + 4608 = 18432
QSH = QCOLS // 8                             # 2304
W1SZ = 9 * 3 * 2 * 2 * P                     # 13824
BCOLS = 12288
BSH = BCOLS // 8                             # 1536
OFF_GW, OFF_FC1, OFF_FC2, OFF_AWM, OFF_AWP = 0, 512, 2560, 11776, 11778

# single-blob input layout (byte offsets per core); one jax array per call
# minimizes the per-array tunnel dispatch overhead (~85 ms/array)
FRB = 2 * P * H * W            # bytes of one fp8 frame pair = 802816
SOFF = 0                       # s fp8 (tower + alpha only; exact s blends on host)
COFF = SOFF + FRB              # c fp8 (f_t; depthwise reads the exact upcast)
DOFF = COFF + FRB              # d fp8
QOFF = DOFF + FRB              # qsh fp8 [128,2304]
BOFF = QOFF + P * QSH          # bsh bf16 [128,1536]
CFOFF = BOFF + P * BSH * 2     # cf f32 [128,28]
ABOFF = CFOFF + P * 28 * 4     # ab f32 [1,1]
NB = ABOFF + 4

# output blob: f_t_mod in fp8 + raw sigmoid(alpha pre-act) in f32;
# the final blend out = s + (0.3+0.4*sig) * (m - s) runs on host in f32
MOFF = 0                       # m fp8 [2,128,56,56]
AOFF = MOFF + FRB              # sigma f32 [56,56]
OB = AOFF + H * W * 4

LAST_INFO = {}
_CACHE = {}
_POOL = None


def _pool():
    global _POOL
    if _POOL is None:
        from concurrent.futures import ThreadPoolExecutor
        _POOL = ThreadPoolExecutor(max_workers=8)
    return _POOL


def _taps(d):
    return [(ky * 3 + kx, (ky - 1) * d, (kx - 1) * d) for ky in range(3) for kx in range(3)]


def build_nc():
    nc = bacc.Bacc()
    f32 = mybir.dt.float32
    bf16 = mybir.dt.bfloat16
    fp8 = mybir.dt.float8e4
    DR = mybir.MatmulPerfMode.DoubleRow

    Relu = mybir.ActivationFunctionType.Relu
    Sigmoid = mybir.ActivationFunctionType.Sigmoid
    mult = mybir.AluOpType.mult
    add = mybir.AluOpType.add

    # ---- dram I/O: ONE uint8 blob in, ONE uint8 blob out (regions bitcast) ----
    xin = nc.dram_tensor("xin", [NB], mybir.dt.uint8, kind="ExternalInput")

    y = nc.dram_tensor("y", [OB], mybir.dt.uint8, kind="ExternalOutput")
    ym_v = y[MOFF:MOFF + FRB].bitcast(fp8).rearrange(
        "(o p r c) -> o p r c", o=2, p=P, r=H)
    ya_v = y[AOFF:AOFF + H * W * 4].bitcast(f32).rearrange("(r c) -> r c", r=H)

    identc = nc.inline_tensor(np.eye(P, dtype=np.float32).astype(BF16), name="identc")

    with tile.TileContext(nc) as tc:
        with (
            tc.tile_pool(name="mp", bufs=1) as mp,
            tc.tile_pool(name="dp", bufs=1, space="DRAM") as dp,
            tc.tile_pool(name="psb", bufs=4, space="PSUM") as psb,
            tc.tile_pool(name="pss", bufs=2, space="PSUM") as pss,
            tc.tile_pool(name="psa", bufs=2, space="PSUM") as psa,
        ):
            xqs = mp.tile([P, 3, 2, FW, FW], fp8, name="xqs")   # tower input: s,d,c
            xcs = mp.tile([P, 2, FW, FW], bf16, name="xcs")     # c padded (depthwise in)
            m8 = mp.tile([P, 2, H, W], fp8, name="m8")          # f_t_mod, wire copy
            y1s = mp.tile([P, 2, FW, FW], fp8, name="y1s")
            fms = mp.tile([P, 2, H, FW], bf16, name="fms")
            w1s = mp.tile([P, 3, 9, 2, 2, P], fp8, name="w1s")
            w2s = mp.tile([P, 9, 2, 2, P], fp8, name="w2s")
            gws = mp.tile([P, 2, HID], bf16, name="gws")
            fc1ws = mp.tile([P, 4, 512], bf16, name="fc1ws")
            fc2ws = mp.tile([P, 18, 4, P], bf16, name="fc2ws")
            ident = mp.tile([P, P], bf16, name="ident")
            awm = mp.tile([P, 2], bf16, name="awm")
            awp = mp.tile([P, 2], bf16, name="awp")
            awpq = mp.tile([P, 2], fp8, name="awpq")
            cfs = mp.tile([P, 28], f32, name="cfs")
            abts = mp.tile([1, 1], f32, name="abts")
            pacc = [mp.tile([P, NCH], f32, name=f"pacc{o}") for o in range(2)]
            gsum = mp.tile([P, 2], f32, name="gsum")
            lsum = mp.tile([P, 2], f32, name="lsum")
            gsumb = mp.tile([P, 2], bf16, name="gsumb")
            fcinb = mp.tile([P, 4], bf16, name="fcinb")
            hb = mp.tile([P, 4], bf16, name="hb")
            wkt = mp.tile([P, 18], f32, name="wkt")
            wks = mp.tile([P, 18], f32, name="wks")
            diag = [mp.tile([P, 9, P], bf16, name=f"diag{o}") for o in range(2)]

            qb = dp.tile([P, QSH], fp8, name="qb")
            qg = dp.tile([8, P, QSH], fp8, name="qg")
            bb = dp.tile([P, BSH], bf16, name="bb")
            bg = dp.tile([8, P, BSH], bf16, name="bg")

            # ---------- weight AllGather (NeuronLink, not host tunnel) ----------
            nc.gpsimd.dma_start(qb[:], xin[QOFF:QOFF + P * QSH].bitcast(fp8))
            nc.gpsimd.collective_compute(
                "AllGather", mybir.AluOpType.bypass,
                replica_groups=[list(range(8))], ins=[qb.opt()], outs=[qg.opt()])
            nc.gpsimd.dma_start(bb[:], xin[BOFF:BOFF + P * BSH * 2].bitcast(bf16))
            nc.gpsimd.collective_compute(
                "AllGather", mybir.AluOpType.bypass,
                replica_groups=[list(range(8))], ins=[bb.opt()], outs=[bg.opt()])

            w1f = w1s.rearrange("p a b c d e -> p (a b c d e)")
            w2f = w2s.rearrange("p a b c d -> p (a b c d)")
            for r in range(6):
                nc.sync.dma_start(out=w1f[:, r * QSH:(r + 1) * QSH], in_=qg[r])
            for r in (6, 7):
                o0 = (r - 6) * QSH
                nc.sync.dma_start(out=w2f[:, o0:o0 + QSH], in_=qg[r])

            # ---------- frame staging ----------
            nc.scalar.memzero(xqs)
            HFR = FRB // 2
            for o in range(2):
                nc.sync.dma_start(
                    out=xqs[:, 0, o, R0:R0 + H, R0:R0 + W],
                    in_=xin[SOFF + o * HFR:SOFF + (o + 1) * HFR].bitcast(fp8))
                nc.sync.dma_start(
                    out=xqs[:, 2, o, R0:R0 + H, R0:R0 + W],
                    in_=xin[COFF + o * HFR:COFF + (o + 1) * HFR].bitcast(fp8))
                nc.sync.dma_start(
                    out=xqs[:, 1, o, R0:R0 + H, R0:R0 + W],
                    in_=xin[DOFF + o * HFR:DOFF + (o + 1) * HFR].bitcast(fp8))
            nc.vector.tensor_copy(xcs, xqs[:, 2])    # c fp8 -> bf16 (exact upcast)
            nc.sync.dma_start(out=cfs, in_=xin[CFOFF:CFOFF + P * 28 * 4].bitcast(f32))
            nc.sync.dma_start(out=abts, in_=xin[ABOFF:ABOFF + 4].bitcast(f32))
            nc.sync.dma_start(out=ident, in_=identc[:, :])
            nc.scalar.memzero(y1s)

            b1c = [cfs[:, 0:1], cfs[:, 1:2]]
            b2c = [cfs[:, 2:3], cfs[:, 3:4]]
            gbc = cfs[:, 4:6]
            fc1b = cfs[:, 6:10]
            fc2bT = cfs[:, 10:28]

            taps1 = _taps(2)
            taps2 = _taps(4)
            taps3 = _taps(1)

            # ---------- conv1 ----------
            for c in range(NCH):
                for o in range(2):
                    ps = psb.tile([P, NR, W], f32, name=f"c1_{o}_{c}", tag="psb")
                    psl = ps.rearrange("p a b -> p (a b)")
                    mms = []
                    for kp in range(3):
                        for (t, dy, dx) in taps1:
                            r = R0 + NR * c + dy
                            mms.append((w1s[:, kp, t, o, :, :],
                                        xqs[:, kp, :, r:r + NR, 4 + dx:60 + dx]))
                    for n, (wv, xv) in enumerate(mms):
                        nc.tensor.matmul(psl, wv, xv, start=(n == 0),
                                         stop=(n == len(mms) - 1), perf_mode=DR)
                    nc.scalar.activation(
                        out=y1s[:, o, R0 + NR * c:R0 + NR * c + NR, 4:60],
                        in_=ps, func=Relu,
                        bias=b1c[o], scale=1.0 / WS,
                    )

            # late-use loads: emitted after conv1 so they queue behind the
            # conv-critical transfers on the DMA engines
            regions = [
                (gws.rearrange("p a b -> p (a b)"), OFF_GW, 2 * HID),
                (fc1ws.rearrange("p a b -> p (a b)"), OFF_FC1, 4 * 512),
                (fc2ws.rearrange("p a b c -> p (a b c)"), OFF_FC2, 18 * 4 * P),
                (awm, OFF_AWM, 2),
                (awp, OFF_AWP, 2),
            ]
            for dst, roff, rlen in regions:
                for r in range(8):
                    a = max(roff, r * BSH)
                    b2 = min(roff + rlen, (r + 1) * BSH)
                    if a < b2:
                        nc.sync.dma_start(out=dst[:, a - roff:b2 - roff],
                                          in_=bg[r][:, a - r * BSH:b2 - r * BSH])
            # local pooling of f_t (DVE, idle during conv)
            for o in range(2):
                nc.vector.tensor_reduce(
                    out=lsum[:, o:o + 1], in_=xcs[:, o, 4:60, 4:60],
                    axis=mybir.AxisListType.XY, op=add,
                )

            # ---------- conv2 + pooled accumulation ----------
            for c in range(NCH):
                for o in range(2):
                    ps = psb.tile([P, NR, W], f32, name=f"c2_{o}_{c}", tag="psb")
                    psl = ps.rearrange("p a b -> p (a b)")
                    mms = []
                    for (t, dy, dx) in taps2:
                        r = R0 + NR * c + dy
                        mms.append((w2s[:, t, o, :, :],
                                    y1s[:, :, r:r + NR, 4 + dx:60 + dx]))
                    for n, (wv, xv) in enumerate(mms):
                        nc.tensor.matmul(psl, wv, xv, start=(n == 0),
                                         stop=(n == len(mms) - 1), perf_mode=DR)
                    sc2 = mp.tile([P, NR, 56], bf16, name=f"sc2_{o}_{c}", tag="sc2", bufs=2)
                    nc.scalar.activation(
                        out=sc2, in_=ps, func=Relu,
                        bias=b2c[o], scale=1.0 / WS,
                        accum_out=pacc[o][:, c:c + 1],
                    )

            # ---------- pools -> fc chain ----------
            for o in range(2):
                nc.vector.tensor_reduce(
                    out=gsum[:, o:o + 1], in_=pacc[o],
                    axis=mybir.AxisListType.X, op=add,
                )
            nc.vector.tensor_copy(gsumb, gsum)

            psg = pss.tile([P, 2], f32, name="psg", tag="pss")
            for m in range(2):
                for k in range(2):
                    nc.tensor.matmul(
                        psg[:, m:m + 1], gws[:, k, m * P:(m + 1) * P],
                        gsumb[:, k:k + 1], start=(k == 0), stop=(k == 1),
                    )
            nc.vector.tensor_add(fcinb[:, 0:2], psg, gbc)
            nc.vector.tensor_copy(fcinb[:, 2:4], lsum)

            psh = pss.tile([P, 4], f32, name="psh", tag="pss")
            for m in range(4):
                for k in range(4):
                    nc.tensor.matmul(
                        psh[:, m:m + 1], fc1ws[:, k, m * P:(m + 1) * P],
                        fcinb[:, k:k + 1], start=(k == 0), stop=(k == 3),
                    )
            nc.vector.tensor_add(hb, psh, fc1b)

            psT = pss.tile([P, 18], f32, name="psT", tag="pss")
            for j in range(18):
                for kc in range(4):
                    nc.tensor.matmul(
                        psT[:, j:j + 1], fc2ws[:, j, kc, :],
                        hb[:, kc:kc + 1], start=(kc == 0), stop=(kc == 3),
                    )
            nc.vector.tensor_add(wkt, psT, fc2bT)
            # silu(z) = z * sigmoid(z)
            nc.scalar.activation(out=wks, in_=wkt, func=Sigmoid)
            nc.vector.tensor_mul(wks, wks, wkt)

            for j in range(18):
                nc.vector.tensor_scalar_mul(diag[j // 9][:, j % 9, :], ident,
                                            wks[:, j:j + 1])
            nc.vector.tensor_copy(awpq, awp)         # fp8 copy for the s-side alpha dot

            # ---------- depthwise + alpha (valid cols only; blend on host) ----------
            for c in range(NCH):
                rows = slice(NR * c, NR * c + NR)
                for o in range(2):
                    ps = psb.tile([P, NR, W], f32, name=f"dw_{o}_{c}", tag="psb")
                    psl = ps.rearrange("p a b -> p (a b)")
                    for (t, dy, dx) in taps3:
                        nc.tensor.matmul(
                            psl, diag[o][:, t, :],
                            xcs[:, o, R0 + NR * c + dy:R0 + NR * c + dy + NR,
                                4 + dx:60 + dx],
                            start=(t == 0), stop=(t == 8),
                        )
                    nc.scalar.copy(fms[:, o, rows, 4:60], ps)
                    nc.scalar.copy(m8[:, o, rows, :], ps)
                    # stream this chunk's modulated rows out immediately
                    nc.sync.dma_start(out=ym_v[o, :, rows, :],
                                      in_=m8[:, o, rows, :])

                pa = psa.tile([1, NR, W], f32, name=f"pa{c}", tag="psa")
                pal = pa.rearrange("p a b -> p (a b)")
                for o in range(2):
                    nc.tensor.matmul(
                        pal, awm[:, o:o + 1], fms[:, o, rows, 4:60],
                        start=(o == 0), stop=False,
                    )
                for o in range(2):
                    nc.tensor.matmul(
                        pal, awpq[:, o:o + 1],
                        xqs[:, 0, o, R0 + NR * c:R0 + NR * c + NR, 4:60],
                        start=False, stop=(o == 1),
                    )
                arow = mp.tile([1, NR, W], f32, name=f"ar{c}", tag="ar", bufs=2)
                nc.scalar.activation(out=arow, in_=pa, func=Sigmoid, bias=abts[:, 0:1])
                nc.sync.dma_start(out=ya_v[rows, :],
                                  in_=arow.rearrange("p a b -> p (a b)"))

    nc.compile()
    return nc


def _get_exec():
    if "jitted" in _CACHE:
        return _CACHE
    nc = build_nc()
    bass2jax.install_neuronx_cc_hook()
    partition_name = nc.partition_id_tensor.name if nc.partition_id_tensor else None
    in_names, out_names, out_avals = [], [], []
    for alloc in nc.m.functions[0].allocations:
        if not isinstance(alloc, mybir.MemoryLocationSet):
            continue
        name = alloc.memorylocations[0].name
        if alloc.kind == "ExternalInput":
            if name != partition_name:
                in_names.append(name)
        elif alloc.kind == "ExternalOutput":
            out_names.append(name)
            out_avals.append(jax.core.ShapedArray(
                tuple(alloc.tensor_shape), mybir.dt.np(alloc.dtype)))
    n_params = len(in_names)
    param_names = list(in_names)
    if partition_name is not None:
        in_names.append(partition_name)

    def _body(*args):
        operands = list(args)
        if partition_name is not None:
            operands.append(bass2jax.partition_id_tensor())
        return tuple(bass2jax._bass_exec_p.bind(
            *operands, out_avals=tuple(out_avals),
            in_names=tuple(in_names), out_names=tuple(out_names),
            lowering_input_output_aliases=(), sim_require_finite=True,
            sim_require_nnan=True, nc=nc))

    devices = jax.devices()[:8]
    mesh = Mesh(np.asarray(devices), ("core",))
    jitted = jax.jit(shard_map(
        _body, mesh=mesh,
        in_specs=(PartitionSpec("core"),) * n_params,
        out_specs=(PartitionSpec("core"),) * len(out_names), check_rep=False))

    import jax.numpy as jnp

    def _bc(v):
        return jax.lax.bitcast_convert_type(v, jnp.uint8)

    def _prep_all(a, b, c, w1, b1, w2, b2, gw, gb, fc1_w, fc1_b, fc2_w, fc2_b, aw, ab):
        s = (a + b) * np.float32(0.5)
        d = (a - b) * np.float32(0.5)
        sb = _bc(s.astype(jnp.float8_e4m3fn)).reshape(B, FRB)
        cb = _bc(c.astype(jnp.float8_e4m3fn)).reshape(B, FRB)
        db = _bc(d.astype(jnp.float8_e4m3fn)).reshape(B, FRB)

        w1r = w1.reshape(2, P, 3, 2, P, 3, 3)        # o m kp i k ty tx
        w1t = jnp.stack([w1r[:, :, 0] + w1r[:, :, 1],
                         w1r[:, :, 0] - w1r[:, :, 1],
                         w1r[:, :, 2]], axis=2)
        w1q = w1t.transpose(4, 2, 5, 6, 0, 3, 1).reshape(P, W1SZ)   # k kp ty tx o i m
        w2q = w2.reshape(2, P, 2, P, 3, 3).transpose(3, 4, 5, 0, 2, 1).reshape(P, 9 * 2 * 2 * P)
        wq_full = (jnp.concatenate([w1q, w2q], axis=1) * WS).astype(jnp.float8_e4m3fn)
        qsh_g = _bc(wq_full.reshape(P, 8, QSH).transpose(1, 0, 2)).reshape(8, P * QSH)

        gwb = (gw[:, :, 0, 0] / 3136.0).T.reshape(2, P, HID).transpose(1, 0, 2).reshape(P, 2 * HID)
        fc1t = jnp.concatenate([fc1_w.T[:C], fc1_w.T[C:] / 3136.0], axis=0)
        fc1wb = fc1t.reshape(4, P, 512).transpose(1, 0, 2).reshape(P, 4 * 512)
        fc2wb = fc2_w.T.reshape(4, P, 2, P, 9).transpose(1, 2, 4, 0, 3).reshape(P, 18 * 4 * P)
        awm = aw[0, :C, 0, 0].reshape(2, P).T
        awp = aw[0, C:, 0, 0].reshape(2, P).T
        wb_full = jnp.concatenate(
            [gwb, fc1wb, fc2wb, awm, awp,
             jnp.zeros((P, BCOLS - (OFF_AWP + 2)), jnp.float32)], axis=1)
        bsh_g = _bc(wb_full.astype(jnp.bfloat16).reshape(P, 8, BSH).transpose(1, 0, 2)
                    ).reshape(8, P * BSH * 2)

        b1c = b1.reshape(2, P).T
        b2c = b2.reshape(2, P).T
        gbc = gb.reshape(2, P).T
        fc1b4 = fc1_b.reshape(4, P).T
        fc2bT = fc2_b.reshape(2, P, 9).transpose(1, 0, 2).reshape(P, 18)
        cf1 = jnp.concatenate([b1c, b2c, gbc, fc1b4, fc2bT], axis=1).astype(jnp.float32)
        cfb = _bc(jnp.broadcast_to(cf1, (8, P, 28))).reshape(8, P * 28 * 4)
        abb = _bc(jnp.broadcast_to(ab.reshape(1, 1).astype(jnp.float32), (8, 1))).reshape(8, 4)
        return jnp.concatenate([sb, cb, db, qsh_g, bsh_g, cfb, abb], axis=1).reshape(B * NB)

    def _post_all(yb, a, b):
        yb = yb.reshape(B, OB)
        m = jax.lax.bitcast_convert_type(
            yb[:, MOFF:MOFF + FRB], jnp.float8_e4m3fn).astype(jnp.float32)
        m = m.reshape(B, C, H, W)
        sig = jax.lax.bitcast_convert_type(
            yb[:, AOFF:OB].reshape(B, H * W, 4), jnp.float32)
        alpha = (np.float32(0.3) + np.float32(0.4) * sig).reshape(B, 1, H, W)
        s = (a + b) * np.float32(0.5)
        return s + alpha * (m - s)

    cpu = jax.devices("cpu")[0]
    prep_jit = jax.jit(_prep_all, device=cpu)
    post_jit = jax.jit(_post_all, device=cpu)
    _CACHE.update(dict(jitted=jitted, param_names=param_names,
                       out_names=out_names, out_avals=out_avals,
                       prep_jit=prep_jit, post_jit=post_jit))
    return _CACHE


def _prep_shared(w1, b1, w2, b2, gw, gb, fc1_w, fc1_b, fc2_w, fc2_b, aw, ab):
    """Returns per-region [8, nbytes] uint8 arrays for the shared weights."""
    # conv1 weights with the (a,b,c)->(s,d,c) frame transform on axis kp
    w1r = w1.reshape(2, P, 3, 2, P, 3, 3)            # o m kp i k ty tx
    w1t = np.empty_like(w1r)
    w1t[:, :, 0] = w1r[:, :, 0] + w1r[:, :, 1]       # applies to s
    w1t[:, :, 1] = w1r[:, :, 0] - w1r[:, :, 1]       # applies to d
    w1t[:, :, 2] = w1r[:, :, 2]                      # applies to c
    w1q = np.ascontiguousarray(w1t.transpose(4, 2, 5, 6, 0, 3, 1))  # k kp ty tx o i m
    w1q = w1q.reshape(P, W1SZ)
    w2r = w2.reshape(2, P, 2, P, 3, 3)               # o m i k ty tx
    w2q = np.ascontiguousarray(w2r.transpose(3, 4, 5, 0, 2, 1))     # k ty tx o i m
    w2q = w2q.reshape(P, 9 * 2 * 2 * P)
    wq_full = (np.concatenate([w1q, w2q], axis=1) * WS).astype(FP8)  # [P, QCOLS]
    qsh_g = np.ascontiguousarray(
        wq_full.reshape(P, 8, QSH).transpose(1, 0, 2)).view(np.uint8).reshape(8, P * QSH)

    gwt = np.ascontiguousarray((gw[:, :, 0, 0] / 3136.0).T).reshape(2, P, HID)
    gwb = np.ascontiguousarray(gwt.transpose(1, 0, 2)).reshape(P, 2 * HID)
    fc1t = fc1_w.T.copy()
    fc1t[C:, :] /= 3136.0
    fc1wb = np.ascontiguousarray(
        fc1t.reshape(4, P, 512).transpose(1, 0, 2)).reshape(P, 4 * 512)
    f2 = fc2_w.T.reshape(4, P, 2, P, 9)              # kc k bl p t
    fc2wb = np.ascontiguousarray(f2.transpose(1, 2, 4, 0, 3)).reshape(P, 18 * 4 * P)
    awm = np.ascontiguousarray(aw[0, :C, 0, 0].reshape(2, P).T)      # [128, 2]
    awp = np.ascontiguousarray(aw[0, C:, 0, 0].reshape(2, P).T)
    wb_full = np.zeros((P, BCOLS), dtype=np.float32)
    wb_full[:, OFF_GW:OFF_GW + 512] = gwb
    wb_full[:, OFF_FC1:OFF_FC1 + 2048] = fc1wb
    wb_full[:, OFF_FC2:OFF_FC2 + 9216] = fc2wb
    wb_full[:, OFF_AWM:OFF_AWM + 2] = awm
    wb_full[:, OFF_AWP:OFF_AWP + 2] = awp
    wb_full = wb_full.astype(BF16)
    bsh_g = np.ascontiguousarray(
        wb_full.reshape(P, 8, BSH).transpose(1, 0, 2)).view(np.uint8).reshape(8, P * BSH * 2)

    fc1b4 = np.ascontiguousarray(fc1_b.reshape(4, P).T)              # [128, 4]
    b1c = b1.reshape(2, P).T
    b2c = b2.reshape(2, P).T
    gbc = gb.reshape(2, P).T
    fc2bT = np.ascontiguousarray(fc2_b.reshape(2, P, 9).transpose(1, 0, 2)).reshape(P, 18)
    cf1 = np.concatenate([b1c, b2c, gbc, fc1b4, fc2bT], axis=1).astype(np.float32)
    cf_g = np.ascontiguousarray(
        np.broadcast_to(cf1, (8,) + cf1.shape)).view(np.uint8).reshape(8, P * 28 * 4)
    abt_g = np.ascontiguousarray(np.broadcast_to(
        ab.reshape(1).astype(np.float32), (8, 1))).view(np.uint8).reshape(8, 4)
    return qsh_g, bsh_g, cf_g, abt_g


def kernel(f_tm2, f_tm1, f_t, w1, b1, w2, b2, gw, gb,
           fc1_w, fc1_b, fc2_w, fc2_b, aw, ab):
    import time

    args = [np.asarray(a, dtype=np.float32) for a in
            (f_tm2, f_tm1, f_t, w1, b1, w2, b2, gw, gb, fc1_w, fc1_b, fc2_w, fc2_b, aw, ab)]
    f_tm2, f_tm1, f_t = args[0], args[1], args[2]

    t0 = time.time()
    g = _get_exec()
    t1 = time.time()

    blob = np.asarray(g["prep_jit"](*args))
    t2 = time.time()

    out_arrs = g["jitted"](blob)
    yv = np.asarray(out_arrs[0])                      # [8*OB] u8
    t3 = time.time()

    out = np.asarray(g["post_jit"](yv, f_tm2, f_tm1))
    t4 = time.time()
    LAST_INFO.update(dict(build_s=t1 - t0, prep_s=t2 - t1, run_s=t3 - t2,
                          post_s=t4 - t3, exec_time_ns=None))
    return out
